# revision 6
# baseline (speedup 1.0000x reference)
"""MiniGPT forward on 8 Trainium2 NeuronCores.

Sharding: core c handles sequence (c & 3) and vocab half (c >> 2).  The 6
transformer blocks are data-parallel over the 4 sequences (each pair of
cores duplicates block compute); the tied-embedding LM head is split over
the vocab.  No collectives.

All block linears (qkv, proj, fc, fc2) run as fp8(e4m3) DoubleRow matmuls
with 3-term error feedback: out = Wh@xh + Wh@xl + Wl@xh, which is
W/x-16-bit-equivalent accuracy at 0.75x the bf16 PE cost (DR packs a
256-deep contraction per instruction at 0.5 cycles/row).  Attention
(scores, softmax, AV) stays bf16.  LayerNorm gains are folded into the
following linear's weights on the host; LN stats use PE ones-matmuls with
the x^2 squares on gpsimd; causal masking is applied on the PE by
accumulating I @ (-240*(p>f)) into the score PSUM.  The LM head runs the
same 3-term fp8 scheme, fully hidden under the 105MB logits output DMA.
"""

import sys

sys.path.insert(0, "/opt/trn_rl_repo")

import numpy as np
import ml_dtypes

import concourse.bacc as bacc
import concourse.tile as tile
from concourse import mybir
from concourse.bass_utils import run_bass_kernel_spmd

F32 = mybir.dt.float32
F32R = mybir.dt.float32r
BF16 = mybir.dt.bfloat16
FP8 = mybir.dt.float8e4
ALU = mybir.AluOpType
ACT = mybir.ActivationFunctionType
DR = mybir.MatmulPerfMode.DoubleRow
E4 = ml_dtypes.float8_e4m3

B, T, C, H, HD, L, V = 4, 1024, 768, 12, 64, 6, 50257
CT = C // 128           # 6 c-tiles
KP = CT // 2            # 3 k-pairs
TT = T // 128           # 8 token tiles
AQ = 256                # attention query block
HT = 3072 // 128        # 24 hidden tiles
HP = HT // 2            # 12 hidden k-pairs
VS = 25600              # vocab shard per core
NVC = VS // 512         # 50
EPS = 1e-5
SX = 16.0               # fp8 scale for LN outputs (blocks + head)
SY = 32.0               # fp8 scale for attention output y
SH = 32.0               # fp8 scale for gelu output h

_CACHE = {}
LAST_RESULT = None
LAST_NC = None


def build_program(sc, n_layers=L):
    nc = bacc.Bacc(None, target_bir_lowering=False)

    def f8_in(name, shape):
        return nc.dram_tensor(name, list(shape), FP8, kind="ExternalInput")

    x0t_d = nc.dram_tensor("x0t", [128, CT, T], F32R, kind="ExternalInput")
    wqh, wql, wvh, wvl, wph, wpl = [], [], [], [], [], []
    wfh, wfl, w2h, w2l = [], [], [], []
    for l in range(n_layers):
        wqh.append(f8_in(f"qkwh{l}", (12 * 128, KP, 2, 128)))
        wql.append(f8_in(f"qkwl{l}", (12 * 128, KP, 2, 128)))
        wvh.append(f8_in(f"vwh{l}", (128, KP, 2, C)))
        wvl.append(f8_in(f"vwl{l}", (128, KP, 2, C)))
        wph.append(f8_in(f"pwh{l}", (CT * 128, KP, 2, 128)))
        wpl.append(f8_in(f"pwl{l}", (CT * 128, KP, 2, 128)))
        wfh.append(f8_in(f"fwh{l}", (HT * 128, KP, 2, 128)))
        wfl.append(f8_in(f"fwl{l}", (HT * 128, KP, 2, 128)))
        w2h.append(f8_in(f"f2wh{l}", (CT * 128, HP, 2, 128)))
        w2l.append(f8_in(f"f2wl{l}", (CT * 128, HP, 2, 128)))
    whh_d = nc.dram_tensor("whh", [NVC * 128, KP, 2, 512], FP8,
                           kind="ExternalInput")
    whl_d = nc.dram_tensor("whl", [NVC * 128, KP, 2, 512], FP8,
                           kind="ExternalInput")
    masks_d = nc.dram_tensor("masks", [128, 2, AQ], FP8,
                             kind="ExternalInput")
    iden_d = nc.dram_tensor("iden", [128, 2, 128], FP8,
                            kind="ExternalInput")
    logits_d = nc.dram_tensor("logits", [T, VS], F32, kind="ExternalOutput")

    ln_ctr = [0]

    with nc.allow_low_precision("fp8 3-term error-feedback intentional"), \
         tile.TileContext(nc) as tc:
        glob = tc.alloc_tile_pool(name="glob", bufs=1)
        gx = tc.alloc_tile_pool(name="gx", bufs=1)
        gx2 = tc.alloc_tile_pool(name="gx2", bufs=2)
        gu = tc.alloc_tile_pool(name="gu", bufs=2)
        gmicro = tc.alloc_tile_pool(name="gmicro", bufs=1)
        ps_big = tc.alloc_tile_pool(name="ps_big", bufs=3, space="PSUM")
        gxn = tc.alloc_tile_pool(name="gxn", bufs=1)
        gw = tc.alloc_tile_pool(name="gw", bufs=5)
        gwv = tc.alloc_tile_pool(name="gwv", bufs=1)
        gw24 = tc.alloc_tile_pool(name="gw24", bufs=2)

        ones_col = glob.tile([128, 1], F32R, tag="ones_col")
        ones_row = glob.tile([1, 128], F32R, tag="ones_row")
        epsh_t = glob.tile([1, 1], F32, tag="epsh")
        masks_t = glob.tile([128, 2, AQ], FP8, tag="masks")
        iden_t = glob.tile([128, 2, 128], FP8, tag="iden")
        nc.vector.memset(ones_col[:].bitcast(F32), 1.0)
        nc.vector.memset(ones_row[:].bitcast(F32), 1.0)
        nc.vector.memset(epsh_t[:], EPS / (SX * SX))
        nc.sync.dma_start(out=masks_t[:], in_=masks_d[:])
        nc.sync.dma_start(out=iden_t[:], in_=iden_d[:])

        xT = gx.tile([128, CT, T], F32R, tag="xT")
        nc.sync.dma_start(out=xT[:], in_=x0t_d[:])

        # persistent v tile: [p, tt, h, 0:64] = v ; [.., 64:128] = 1.0
        gv = tc.alloc_tile_pool(name="gv", bufs=1)
        vT = gv.tile([128, TT, H, 128], BF16, tag="vT")
        nc.gpsimd.memset(vT[:, :, :, 64:128], 1.0)

        def ln_qc(xin, xh, xl, qc, ps_stat, ps_bc):
            """one token-half of (xh + xl) ~= SX * (xin - mu) * rstd in fp8."""
            qs = slice(qc * 512, (qc + 1) * 512)
            s_ps = ps_stat.tile([1, 512], F32, space="PSUM", tag="stat")
            q_ps = ps_stat.tile([1, 512], F32, space="PSUM", tag="stat")
            for kt in range(CT):
                nc.tensor.matmul(s_ps[:], ones_col[:], xin[:, kt, qs],
                                 start=(kt == 0), stop=(kt == CT - 1))
            for kt in range(CT):
                x2 = gx2.tile([128, 512], F32R, tag="x2")
                nc.gpsimd.tensor_tensor(
                    out=x2[:], in0=xin[:, kt, qs],
                    in1=xin[:, kt, qs], op=ALU.mult)
                nc.tensor.matmul(q_ps[:], ones_col[:], x2[:],
                                 start=(kt == 0), stop=(kt == CT - 1))
            mu = gmicro.tile([1, 512], F32R, tag="mu")
            nc.scalar.mul(mu[:], s_ps[:], 1.0 / C)
            mu2 = gmicro.tile([1, 512], F32, tag="mu2")
            nc.scalar.activation(mu2[:], mu[:], ACT.Square)
            var = gmicro.tile([1, 512], F32, tag="var")
            nc.vector.scalar_tensor_tensor(
                out=var[:], in0=q_ps[:], scalar=1.0 / C, in1=mu2[:],
                op0=ALU.mult, op1=ALU.subtract)
            sd = gmicro.tile([1, 512], F32, tag="sd")
            nc.scalar.activation(sd[:], var[:], ACT.Sqrt, bias=epsh_t[:],
                                 scale=1.0 / (SX * SX))
            r = gmicro.tile([1, 512], F32R, tag="r")
            nc.vector.reciprocal(r[:], sd[:])            # SX/sd
            mr = gmicro.tile([1, 512], F32R, tag="mr")
            nc.vector.tensor_tensor(out=mr[:], in0=mu[:], in1=r[:],
                                    op=ALU.mult)
            bc = ps_bc.tile([128, 512], F32, space="PSUM", tag="bc")
            nc.tensor.matmul(bc[:], ones_row[:], r[:], start=True,
                             stop=True)
            bc2 = ps_bc.tile([128, 512], F32, space="PSUM", tag="bc")
            nc.tensor.matmul(bc2[:], ones_row[:], mr[:], start=True,
                             stop=True)
            for kt in range(CT):
                t = gx2.tile([128, 512], F32, tag="lnt")
                nc.vector.tensor_tensor(out=t[:],
                                        in0=xin[:, kt, qs].bitcast(F32),
                                        in1=bc[:], op=ALU.mult)
                u = gu.tile([128, 512], F32, tag="lnu")
                nc.vector.tensor_tensor(out=u[:], in0=t[:],
                                        in1=bc2[:], op=ALU.subtract)
                nc.scalar.copy(xh[:, kt, qs], u[:])
                nc.vector.tensor_tensor(out=xl[:, kt, qs], in0=u[:],
                                        in1=xh[:, kt, qs],
                                        op=ALU.subtract)

        def ln_pools():
            i = ln_ctr[0]
            ln_ctr[0] += 1
            ps_stat = tc.alloc_tile_pool(name=f"ps_st{i}", bufs=2,
                                         space="PSUM")
            ps_bc = tc.alloc_tile_pool(name=f"ps_bc{i}", bufs=2, space="PSUM")
            return ps_stat, ps_bc

        def dr_mm(ps, wh, wl, xh, xl, qs, kps, wsl=None):
            """accumulate 3-term fp8 DR: Wh@xh + Wh@xl + Wl@xh into ps."""
            n = 3 * kps
            i = 0
            for w_, x_ in ((wh, xh), (wh, xl), (wl, xh)):
                for kp in range(kps):
                    lhs = w_[:, kp, :, :] if wsl is None else \
                        w_[:, kp, :, wsl]
                    nc.tensor.matmul(ps, lhs, x_[:, 2 * kp:2 * kp + 2, qs],
                                     start=(i == 0), stop=(i == n - 1),
                                     perf_mode=DR)
                    i += 1

        for l in range(n_layers):
            pa = tc.alloc_tile_pool(name=f"pa{l}", bufs=1)

            # ---- LN1 -> xn8 h/l (fp8), interleaved with q,k projections ----
            xnh = gxn.tile([128, CT, T], FP8, tag="xnh")
            xnl = gxn.tile([128, CT, T], FP8, tag="xnl")
            qkT = pa.tile([128, 12, T], BF16, tag="qkT")
            st1, bc1 = ln_pools()

            def qkv_half(qc):
                qs = slice(qc * 512, (qc + 1) * 512)
                ln_qc(xT, xnh, xnl, qc, st1, bc1)
                for s in range(12):
                    wh = gw.tile([128, KP, 2, 128], FP8, tag="w6h", name="wh")
                    nc.sync.dma_start(out=wh[:],
                                      in_=wqh[l].ap()[s * 128:(s + 1) * 128])
                    wl = gw.tile([128, KP, 2, 128], FP8, tag="w6l", name="wl")
                    nc.sync.dma_start(out=wl[:],
                                      in_=wql[l].ap()[s * 128:(s + 1) * 128])
                    ps = ps_big.tile([128, 512], F32, space="PSUM", tag="px",
                                     name="ps")
                    dr_mm(ps[:], wh, wl, xnh, xnl, qs, KP)
                    nc.vector.tensor_scalar_mul(qkT[:, s, qs], ps[:],
                                                sc[f"dq_qk{l}"])

            # ---- v projection (tokens on PSUM partitions) ----
            wvht = gwv.tile([128, KP, 2, C], FP8, tag="vwh")
            nc.sync.dma_start(out=wvht[:], in_=wvh[l].ap()[:])
            wvlt = gwv.tile([128, KP, 2, C], FP8, tag="vwl")
            nc.sync.dma_start(out=wvlt[:], in_=wvl[l].ap()[:])

            def v_half(tts):
              for tt in tts:
                tsl = slice(tt * 128, (tt + 1) * 128)
                psA = ps_big.tile([128, 512], F32, space="PSUM", tag="px")
                psB = ps_big.tile([128, 256], F32, space="PSUM", tag="px")
                # lhsT = x (tokens on out partitions), rhs = w
                n = 3 * KP
                for half, ps_, csl in ((0, psA, slice(0, 512)),
                                       (1, psB, slice(512, 768))):
                    i = 0
                    for x_, w_ in ((xnh, wvht), (xnl, wvht), (xnh, wvlt)):
                        for kp in range(KP):
                            nc.tensor.matmul(ps_[:],
                                             x_[:, 2 * kp:2 * kp + 2, tsl],
                                             w_[:, kp, :, csl],
                                             start=(i == 0),
                                             stop=(i == n - 1),
                                             perf_mode=DR)
                            i += 1
                nc.vector.tensor_scalar_mul(
                    vT[:, tt, 0:8, 0:64],
                    psA[:].rearrange("p (h d) -> p h d", h=8),
                    sc[f"dq_v{l}"])
                nc.vector.tensor_scalar_mul(
                    vT[:, tt, 8:12, 0:64],
                    psB[:].rearrange("p (h d) -> p h d", h=4),
                    sc[f"dq_v{l}"])

            # ---- attention (bf16) ----
            yT = pa.tile([128, CT, T], BF16, tag="yT")

            def attn_part(jlist):
              for hp in range(6):
                for j in jlist:
                    js = slice(j * AQ, (j + 1) * AQ)
                    epairs = {}
                    for p_ in range(j + 1):
                        pstiles = {}
                        for h in (2 * hp, 2 * hp + 1):
                            par = h % 2
                            rows = slice(64 * par, 64 * par + 64)
                            sps = ps_sc.tile([128, 2 * AQ], F32,
                                             space="PSUM", tag="sc",
                                             name="sps")
                            diag = (p_ == j)
                            for half in range(2):
                                kt = 2 * p_ + half
                                nc.tensor.matmul(
                                    sps[:, half * AQ:(half + 1) * AQ],
                                    qkT[rows, 6 + hp,
                                        kt * 128:(kt + 1) * 128],
                                    qkT[rows, hp, js],
                                    start=True, stop=not diag,
                                    skip_group_check=diag)
                                if diag:
                                    nc.tensor.matmul(
                                        sps[:, half * AQ:(half + 1) * AQ],
                                        iden_t[:],
                                        masks_t[:, half, :].rearrange(
                                            "p (i q) -> p i q", i=1)
                                        .broadcast_to([128, 2, AQ]),
                                        start=False, stop=True, perf_mode=DR,
                                        skip_group_check=True)
                            pstiles[h] = sps
                        for h in (2 * hp, 2 * hp + 1):
                            e = pE.tile([128, 2 * AQ], BF16, tag="E",
                                        name="e")
                            nc.scalar.activation(e[:], pstiles[h][:],
                                                 ACT.Exp, scale=0.125)
                            epairs[(h, p_)] = e
                    for h in (2 * hp, 2 * hp + 1):
                        par = h % 2
                        yrow = slice(64 * par, 64 * par + 64)
                        yps = ps_av.tile([128, AQ], F32, space="PSUM",
                                         tag="av", name="yps")
                        for kt in range(2 * j + 2):
                            e = epairs[(h, kt // 2)]
                            nc.tensor.matmul(
                                yps[:], vT[:, kt, h, :],
                                e[:, (kt % 2) * AQ:(kt % 2 + 1) * AQ],
                                start=(kt == 0), stop=(kt == 2 * j + 1))
                        rec = prec.tile([64, AQ], F32, tag="rec")
                        nc.vector.reciprocal(rec[:], yps[64:128, :])
                        nc.vector.tensor_tensor(out=yT[yrow, hp, js],
                                                in0=yps[0:64, :], in1=rec[:],
                                                op=ALU.mult)

            qkv_half(0)
            qkv_half(1)
            for p in (bc1, st1):
                p.release()
            pE = tc.alloc_tile_pool(name=f"pE{l}", bufs=8)
            prec = tc.alloc_tile_pool(name=f"prec{l}", bufs=3)
            ps_sc = tc.alloc_tile_pool(name=f"ps_sc{l}", bufs=3, space="PSUM")
            ps_av = tc.alloc_tile_pool(name=f"ps_av{l}", bufs=2, space="PSUM")
            v_half(range(0, 4))
            attn_part([0, 1])
            v_half(range(4, 8))
            attn_part([2, 3])
            for p in (ps_av, ps_sc, prec, pE):
                p.release()

            # ---- proj + residual, then LN2 + MLP; weight tiles loaded once
            # (ot-outer, qc-inner) ----
            py8 = tc.alloc_tile_pool(name=f"py8{l}", bufs=1)
            ph8 = tc.alloc_tile_pool(name=f"ph8{l}", bufs=1)
            phbf = tc.alloc_tile_pool(name=f"phbf{l}", bufs=3)
            xn2h = gxn.tile([128, CT, T], FP8, tag="xnh")
            xn2l = gxn.tile([128, CT, T], FP8, tag="xnl")
            st2, bc2p = ln_pools()
            y8h = py8.tile([128, CT, T], FP8, tag="y8h")
            y8l = py8.tile([128, CT, T], FP8, tag="y8l")
            for kt in range(CT):
                nc.scalar.mul(y8h[:, kt, :], yT[:, kt, :], SY)
                nc.vector.scalar_tensor_tensor(
                    out=y8l[:, kt, :], in0=yT[:, kt, :], scalar=SY,
                    in1=y8h[:, kt, :], op0=ALU.mult, op1=ALU.subtract)
            for ot in range(CT):
                wh = gw.tile([128, KP, 2, 128], FP8, tag="w6h")
                nc.sync.dma_start(out=wh[:],
                                  in_=wph[l].ap()[ot * 128:(ot + 1) * 128])
                wl = gw.tile([128, KP, 2, 128], FP8, tag="w6l")
                nc.sync.dma_start(out=wl[:],
                                  in_=wpl[l].ap()[ot * 128:(ot + 1) * 128])
                for qc in range(2):
                    qs = slice(qc * 512, (qc + 1) * 512)
                    ps = ps_big.tile([128, 512], F32, space="PSUM", tag="px")
                    dr_mm(ps[:], wh, wl, y8h, y8l, qs, KP)
                    nc.vector.scalar_tensor_tensor(
                        out=xT[:, ot, qs], in0=ps[:], scalar=sc[f"dq_p{l}"],
                        in1=xT[:, ot, qs], op0=ALU.mult, op1=ALU.add)
            for qc in range(2):
                qs = slice(qc * 512, (qc + 1) * 512)
                ln_qc(xT, xn2h, xn2l, qc, st2, bc2p)
                h8h = ph8.tile([128, HT, 512], FP8, tag="h8h")
                h8l = ph8.tile([128, HT, 512], FP8, tag="h8l")
                for ot in range(HT):
                    wh = gw.tile([128, KP, 2, 128], FP8, tag="w6h")
                    nc.sync.dma_start(out=wh[:],
                                      in_=wfh[l].ap()[ot * 128:(ot + 1) * 128])
                    wl = gw.tile([128, KP, 2, 128], FP8, tag="w6l")
                    nc.sync.dma_start(out=wl[:],
                                      in_=wfl[l].ap()[ot * 128:(ot + 1) * 128])
                    ps = ps_big.tile([128, 512], F32, space="PSUM", tag="px")
                    dr_mm(ps[:], wh, wl, xn2h, xn2l, qs, KP)
                    hbf = phbf.tile([128, 512], BF16, tag="hbf")
                    nc.scalar.activation(hbf[:], ps[:], ACT.Gelu,
                                         scale=sc[f"dq_fc{l}"])
                    nc.scalar.mul(h8h[:, ot, :], hbf[:], SH)
                    nc.vector.scalar_tensor_tensor(
                        out=h8l[:, ot, :], in0=hbf[:], scalar=SH,
                        in1=h8h[:, ot, :], op0=ALU.mult, op1=ALU.subtract)
                for ot in range(CT):
                    wh = gw24.tile([128, HP, 2, 128], FP8, tag="w24h")
                    nc.sync.dma_start(out=wh[:],
                                      in_=w2h[l].ap()[ot * 128:(ot + 1) * 128])
                    wl = gw24.tile([128, HP, 2, 128], FP8, tag="w24l")
                    nc.sync.dma_start(out=wl[:],
                                      in_=w2l[l].ap()[ot * 128:(ot + 1) * 128])
                    ps = ps_big.tile([128, 512], F32, space="PSUM", tag="px")
                    dr_mm(ps[:], wh, wl, h8h, h8l, slice(0, 512), HP)
                    nc.vector.scalar_tensor_tensor(
                        out=xT[:, ot, qs], in0=ps[:], scalar=sc[f"dq_f2{l}"],
                        in1=xT[:, ot, qs], op0=ALU.mult, op1=ALU.add)
            for p in (bc2p, st2, phbf, ph8, py8, pa):
                p.release()

        # ---- final LN (fp8 h/l, scale SX) + LM head ----
        for p in (gv, gw24, gwv, gw, gxn):
            p.release()
        pf = tc.alloc_tile_pool(name="pf", bufs=1)
        xf = pf.tile([128, CT, T], FP8, tag="xf")
        xl = pf.tile([128, CT, T], FP8, tag="xl")
        stf, bcf = ln_pools()
        for qc in range(2):
            ln_qc(xT, xf, xl, qc, stf, bcf)
        for p in (bcf, stf):
            p.release()

        ph = tc.alloc_tile_pool(name="ph", bufs=4)
        pout = tc.alloc_tile_pool(name="pout", bufs=2)
        for vc in range(NVC):
            wh8 = ph.tile([128, KP, 2, 512], FP8, tag="wh")
            nc.sync.dma_start(out=wh8[:],
                              in_=whh_d.ap()[vc * 128:(vc + 1) * 128])
            wl8 = ph.tile([128, KP, 2, 512], FP8, tag="whl")
            nc.sync.dma_start(out=wl8[:],
                              in_=whl_d.ap()[vc * 128:(vc + 1) * 128])
            o = pout.tile([128, TT, 512], F32, tag="out")
            for tt in range(TT):
                tsl = slice(tt * 128, (tt + 1) * 128)
                ps = ps_big.tile([128, 512], F32, space="PSUM", tag="px")
                terms = [(xf, wh8), (xl, wh8), (xf, wl8)]
                for cc in range(2):
                    i = 0
                    for xsrc, wsrc in terms:
                        for kp in range(KP):
                            nc.tensor.matmul(
                                ps[:, cc * 256:(cc + 1) * 256],
                                xsrc[:, 2 * kp:2 * kp + 2, tsl],
                                wsrc[:, kp, :, cc * 256:cc * 256 + 256],
                                start=(i == 0), stop=(i == 3 * KP - 1),
                                perf_mode=DR)
                            i += 1
                if tt % 2 == 0:
                    nc.vector.tensor_scalar_mul(o[:, tt, :], ps[:],
                                                sc["dq_h"])
                else:
                    nc.scalar.mul(o[:, tt, :], ps[:], sc["dq_h"])
            nc.sync.dma_start(
                out=logits_d.ap()[:, vc * 512:(vc + 1) * 512].rearrange(
                    "(t p) v -> p t v", p=128),
                in_=o[:])
        for p in (pout, ph, pf, ps_big, gmicro, gu, gx2, gx, glob):
            p.release()

    nc.compile()
    return nc


# ---------------------------------------------------------------------------
# host side
# ---------------------------------------------------------------------------

def _pow2_scale(m, target=224.0):
    if m == 0:
        return 1.0
    return float(2.0 ** np.floor(np.log2(target / m)))


def _hi_lo(w, s):
    """w*s split into e4m3 hi + lo parts."""
    ws = w * s
    hi = ws.astype(E4)
    lo = (ws - hi.astype(np.float32)).astype(E4)
    return hi, lo


def _prep_inputs(inputs, n_layers):
    f32 = np.float32
    idx = np.asarray(inputs["idx"])
    wte = np.asarray(inputs["wte"], f32)
    wpe = np.asarray(inputs["wpe"], f32)

    sc = {}
    common = {}
    for l in range(n_layers):
        ln1w = np.asarray(inputs["ln1_w"][l], f32)
        ln1b = np.asarray(inputs["ln1_b"][l], f32)
        aw = np.asarray(inputs["attn_w"][l], f32)
        ab = np.asarray(inputs["attn_b"][l], f32)
        awf = ln1w[:, None] * aw
        abf = ab + ln1b @ aw
        assert not np.any(abf), "nonzero attn bias not supported"
        # LN output is scaled by SX on device; fold 1/SX into weights so
        # qkT comes out in true units after the dq mult.
        qk = awf[:, :1536]                       # [C, 1536], c_in-major
        s_qk = _pow2_scale(float(np.abs(qk).max()))
        qh, ql = _hi_lo(qk, s_qk)
        # c_in = kp*256 + d*128 + p  ->  [s*128+co_p, kp, d, co]
        def qk_layout(a):
            return np.ascontiguousarray(
                a.reshape(KP, 2, 128, 12, 128).transpose(3, 2, 0, 1, 4)
            ).reshape(12 * 128, KP, 2, 128)
        common[f"qkwh{l}"] = qk_layout(qh)
        common[f"qkwl{l}"] = qk_layout(ql)
        sc[f"dq_qk{l}"] = 1.0 / (SX * s_qk)
        vw = awf[:, 1536:]                       # [C, C]
        s_v = _pow2_scale(float(np.abs(vw).max()))
        vh, vl = _hi_lo(vw, s_v)
        def v_layout(a):
            return np.ascontiguousarray(
                a.reshape(KP, 2, 128, C).transpose(2, 0, 1, 3))
        common[f"vwh{l}"] = v_layout(vh)
        common[f"vwl{l}"] = v_layout(vl)
        sc[f"dq_v{l}"] = 1.0 / (SX * s_v)

        pw = np.asarray(inputs["proj_w"][l], f32)
        assert not np.any(np.asarray(inputs["proj_b"][l])), "proj bias"
        s_p = _pow2_scale(float(np.abs(pw).max()))
        ph_, pl_ = _hi_lo(pw, s_p)
        def p_layout(a):
            return np.ascontiguousarray(
                a.reshape(KP, 2, 128, CT, 128).transpose(3, 2, 0, 1, 4)
            ).reshape(CT * 128, KP, 2, 128)
        common[f"pwh{l}"] = p_layout(ph_)
        common[f"pwl{l}"] = p_layout(pl_)
        sc[f"dq_p{l}"] = 1.0 / (SY * s_p)

        ln2w = np.asarray(inputs["ln2_w"][l], f32)
        ln2b = np.asarray(inputs["ln2_b"][l], f32)
        fw = np.asarray(inputs["fc_w"][l], f32)
        fbv = np.asarray(inputs["fc_b"][l], f32)
        fwf = ln2w[:, None] * fw
        fbf = fbv + ln2b @ fw
        assert not np.any(fbf), "nonzero fc bias not supported"
        s_fc = _pow2_scale(float(np.abs(fwf).max()))
        fh, fl = _hi_lo(fwf, s_fc)
        def f_layout(a):
            return np.ascontiguousarray(
                a.reshape(KP, 2, 128, HT, 128).transpose(3, 2, 0, 1, 4)
            ).reshape(HT * 128, KP, 2, 128)
        common[f"fwh{l}"] = f_layout(fh)
        common[f"fwl{l}"] = f_layout(fl)
        sc[f"dq_fc{l}"] = 1.0 / (SX * s_fc)

        f2w = np.asarray(inputs["fc2_w"][l], f32)
        assert not np.any(np.asarray(inputs["fc2_b"][l])), "fc2 bias"
        s_f2 = _pow2_scale(float(np.abs(f2w).max()))
        f2h, f2l = _hi_lo(f2w, s_f2)
        def f2_layout(a):
            return np.ascontiguousarray(
                a.reshape(HP, 2, 128, CT, 128).transpose(3, 2, 0, 1, 4)
            ).reshape(CT * 128, HP, 2, 128)
        common[f"f2wh{l}"] = f2_layout(f2h)
        common[f"f2wl{l}"] = f2_layout(f2l)
        sc[f"dq_f2{l}"] = 1.0 / (SH * s_f2)

    p = np.arange(128)[:, None]
    f = np.arange(AQ)[None, :]
    masks = np.zeros((128, 2, AQ), f32)
    masks[:, 0, :] = np.where(p > f, -240.0, 0.0)
    masks[:, 1, :] = np.where(p + 128 > f, -240.0, 0.0)
    common["masks"] = masks.astype(E4)
    iden = np.zeros((128, 2, 128), f32)
    iden[:, 0, :] = 128.0 * np.eye(128)
    common["iden"] = iden.astype(E4)

    lnfw = np.asarray(inputs["lnf_w"], f32)
    lnfb = np.asarray(inputs["lnf_b"], f32)
    assert not np.any(lnfb @ wte.T), "nonzero head bias not supported"
    wh = lnfw[:, None] * wte.T                     # [768, V]
    whp = np.zeros((C, 2 * VS), f32)
    whp[:, :V] = wh
    m = float(np.abs(wh).max())
    s_h = float(2.0 ** np.floor(np.log2(240.0 / m)))
    sc["dq_h"] = 1.0 / (s_h * SX)

    whead, wheadl = {}, {}
    for vh in range(2):
        sl = whp[:, vh * VS:(vh + 1) * VS] * s_h
        hi = sl.astype(E4)
        lo = (sl - hi.astype(f32)).astype(E4)
        whead[vh] = np.ascontiguousarray(
            hi.reshape(KP, 2, 128, NVC, 512).transpose(3, 2, 0, 1, 4)
        ).reshape(NVC * 128, KP, 2, 512)
        wheadl[vh] = np.ascontiguousarray(
            lo.reshape(KP, 2, 128, NVC, 512).transpose(3, 2, 0, 1, 4)
        ).reshape(NVC * 128, KP, 2, 512)

    def t6(a):          # [768, T] -> [128, 6, T]
        return np.ascontiguousarray(
            a.reshape(CT, 128, a.shape[1]).transpose(1, 0, 2))

    x0 = wte[idx] + wpe[None, :T]                  # [B, T, C]
    in_maps = []
    for c in range(8):
        s, vh = c & 3, c >> 2
        m2 = dict(common)
        m2["x0t"] = t6(np.ascontiguousarray(x0[s].T))
        m2["whh"] = whead[vh]
        m2["whl"] = wheadl[vh]
        in_maps.append(m2)
    return in_maps, sc


def kernel(**inputs):
    global LAST_RESULT, LAST_NC
    n_layers = L
    in_maps, sc = _prep_inputs(inputs, n_layers)
    key = (n_layers, tuple(sorted(sc.items())))
    if key not in _CACHE:
        _CACHE[key] = build_program(sc, n_layers)
    nc = _CACHE[key]
    LAST_NC = nc
    res = run_bass_kernel_spmd(nc, in_maps, core_ids=list(range(8)))
    LAST_RESULT = res
    out = np.empty((B, T, V), np.float32)
    for c in range(8):
        s, vh = c & 3, c >> 2
        part = res.results[c]["logits"]
        if vh == 0:
            out[s, :, :VS] = part
        else:
            out[s, :, VS:] = part[:, :V - VS]
    return out


if __name__ == "__main__":
    rng = np.random.default_rng(0)
    ins = {
        "idx": rng.integers(0, V, (B, T)).astype(np.int32),
        "wte": (rng.standard_normal((V, C)) * 0.02).astype(np.float32),
        "wpe": (rng.standard_normal((T, C)) * 0.02).astype(np.float32),
        "ln1_w": np.ones((L, C), np.float32),
        "ln1_b": np.zeros((L, C), np.float32),
        "attn_w": (rng.standard_normal((L, C, 3 * C)) * 0.02).astype(np.float32),
        "attn_b": np.zeros((L, 3 * C), np.float32),
        "proj_w": (rng.standard_normal((L, C, C)) * 0.02).astype(np.float32),
        "proj_b": np.zeros((L, C), np.float32),
        "ln2_w": np.ones((L, C), np.float32),
        "ln2_b": np.zeros((L, C), np.float32),
        "fc_w": (rng.standard_normal((L, C, 4 * C)) * 0.02).astype(np.float32),
        "fc_b": np.zeros((L, 4 * C), np.float32),
        "fc2_w": (rng.standard_normal((L, 4 * C, C)) * 0.02).astype(np.float32),
        "fc2_b": np.zeros((L, C), np.float32),
        "lnf_w": np.ones((C,), np.float32),
        "lnf_b": np.zeros((C,), np.float32),
    }
    out = kernel(**ins)
    print("out", out.shape, out.dtype, float(np.abs(out).max()))


# revision 16
# speedup vs baseline: 1.1994x; 1.1994x over previous
"""MiniGPT forward on 8 Trainium2 NeuronCores — sequence-split variant.

Core c: sequence (c & 3), token-half (c >> 2).  In AQ=256 blocks of the
1024-token sequence, half 0 owns blocks {0, 3}, half 1 owns {1, 2} (equal
causal-attention load).  Each core runs the 6 transformer blocks for its
OWN 512 tokens only — no duplicated block compute.  Per layer the pair
exchanges K and V via two ReduceScatters (k first): each core stages
[k*m0, k*m1] with per-core {0,1} masks so the RS output is exactly the
PEER's k/v at a uniform address (SPMD-safe, no rank branching).
Attention = own-block phase (overlaps the RS) + remote phase; causal /
validity masks are per-core input data applied on the PE via the
iden @ mask DoubleRow trick.  One remote block per core is fully masked
waste, keeping the instruction stream identical across cores.

Block linears are fp8(e4m3) DoubleRow 3-term (Wh@xh + Wh@xl + Wl@xh,
0.75x bf16 PE cost); attention stays bf16.  The LM head is token-split:
each core computes its own 512 tokens x the FULL vocab with the same
3-term fp8 scheme, logits stream out in bf16.
"""

import sys

sys.path.insert(0, "/opt/trn_rl_repo")

import numpy as np
import ml_dtypes

import concourse.bacc as bacc
import concourse.tile as tile
from concourse import mybir
from concourse.bass_utils import run_bass_kernel_spmd

F32 = mybir.dt.float32
F32R = mybir.dt.float32r
BF16 = mybir.dt.bfloat16
FP8 = mybir.dt.float8e4
ALU = mybir.AluOpType
ACT = mybir.ActivationFunctionType
DR = mybir.MatmulPerfMode.DoubleRow
E4 = ml_dtypes.float8_e4m3

B, T, C, H, HD, L, V = 4, 1024, 768, 12, 64, 6, 50257
TO = 512                # own tokens per core
CT = C // 128           # 6 c-tiles
KP = CT // 2            # 3 k-pairs
TT = TO // 128          # 4 own token tiles
AQ = 256                # attention query block
HT = 3072 // 128        # 24 hidden tiles
HP = HT // 2            # 12 hidden k-pairs
VS2 = 51200             # padded vocab
NVC = VS2 // 512        # 100
EPS = 1e-5
SX = 16.0               # fp8 scale for LN outputs (blocks + head)
SY = 32.0               # fp8 scale for attention output y (folded in ones)
SH = 1.0                # gelu output used unscaled in fp8

RG = [[0, 4], [1, 5], [2, 6], [3, 7]]   # pair replica groups

_CACHE = {}
LAST_RESULT = None
LAST_NC = None


def build_program(sc, n_layers=L):
    nc = bacc.Bacc(None, target_bir_lowering=False)

    def f8_in(name, shape):
        return nc.dram_tensor(name, list(shape), FP8, kind="ExternalInput")

    x0t_d = nc.dram_tensor("x0t", [128, CT, TO], F32R, kind="ExternalInput")
    wqh, wql, wvh, wvl, wph, wpl = [], [], [], [], [], []
    wfh, wfl, w2h, w2l = [], [], [], []
    for l in range(n_layers):
        wqh.append(f8_in(f"qkwh{l}", (12 * 128, KP, 2, 128)))
        wql.append(f8_in(f"qkwl{l}", (12 * 128, KP, 2, 128)))
        wvh.append(f8_in(f"vwh{l}", (128, KP, 2, C)))
        wvl.append(f8_in(f"vwl{l}", (128, KP, 2, C)))
        wph.append(f8_in(f"pwh{l}", (CT * 128, KP, 2, 128)))
        wpl.append(f8_in(f"pwl{l}", (CT * 128, KP, 2, 128)))
        wfh.append(f8_in(f"fwh{l}", (HT * 128, KP, 2, 128)))
        wfl.append(f8_in(f"fwl{l}", (HT * 128, KP, 2, 128)))
        w2h.append(f8_in(f"f2wh{l}", (CT * 128, HP, 2, 128)))
        w2l.append(f8_in(f"f2wl{l}", (CT * 128, HP, 2, 128)))
    whh_d = f8_in("whh", (NVC * 128, KP, 2, 512))
    whl_d = f8_in("whl", (NVC * 128, KP, 2, 512))
    masks_d = f8_in("masks", (128, 2, AQ))          # local diag (shared)
    masks2_d = f8_in("masks2", (128, 2, 2, AQ))     # [mA|mB, half, q] per-core
    iden_d = f8_in("iden", (128, 2, 128))
    mm_d = nc.dram_tensor("mm", [128, 2], F32, kind="ExternalInput")
    logits_d = nc.dram_tensor("logits", [TO, VS2], BF16,
                              kind="ExternalOutput")

    ln_ctr = [0]

    with nc.allow_low_precision("fp8 3-term error-feedback intentional"), \
         tile.TileContext(nc) as tc:
        glob = tc.alloc_tile_pool(name="glob", bufs=1)
        gx = tc.alloc_tile_pool(name="gx", bufs=1)
        gx2 = tc.alloc_tile_pool(name="gx2", bufs=2)
        gu = tc.alloc_tile_pool(name="gu", bufs=2)
        gmicro = tc.alloc_tile_pool(name="gmicro", bufs=1)
        ps_big = tc.alloc_tile_pool(name="ps_big", bufs=3, space="PSUM")
        gxn = tc.alloc_tile_pool(name="gxn", bufs=1)
        gw = tc.alloc_tile_pool(name="gw", bufs=5)
        gwv = tc.alloc_tile_pool(name="gwv", bufs=1)
        gw24 = tc.alloc_tile_pool(name="gw24", bufs=2)
        gkv = tc.alloc_tile_pool(name="gkv", bufs=1)
        gst = tc.alloc_tile_pool(name="gst", bufs=1)
        dram = tc.alloc_tile_pool(name="dram", bufs=2, space="DRAM")

        ones_col = glob.tile([128, 1], F32R, tag="ones_col")
        ones_row = glob.tile([1, 128], F32R, tag="ones_row")
        epsh_t = glob.tile([1, 1], F32, tag="epsh")
        masks_t = glob.tile([128, 2, AQ], FP8, tag="masks")
        masks2_t = glob.tile([128, 2, 2, AQ], FP8, tag="masks2")
        iden_t = glob.tile([128, 2, 128], FP8, tag="iden")
        mm_t = glob.tile([128, 2], F32, tag="mm")
        nc.vector.memset(ones_col[:].bitcast(F32), 1.0)
        nc.vector.memset(ones_row[:].bitcast(F32), 1.0)
        nc.vector.memset(epsh_t[:], EPS / (SX * SX))
        nc.sync.dma_start(out=masks_t[:], in_=masks_d[:])
        nc.sync.dma_start(out=masks2_t[:], in_=masks2_d[:])
        nc.sync.dma_start(out=iden_t[:], in_=iden_d[:])
        nc.sync.dma_start(out=mm_t[:], in_=mm_d[:])

        xT = gx.tile([128, CT, TO], F32R, tag="xT")
        nc.sync.dma_start(out=xT[:], in_=x0t_d[:])

        # persistent v tiles: [p, tt, h, 0:64] = v ; [.., 64:128] = 1/SY
        gv = tc.alloc_tile_pool(name="gv", bufs=1)
        vOwn = gv.tile([128, TT, H, 128], BF16, tag="vOwn")
        vRem = gv.tile([128, TT, H, 128], BF16, tag="vRem")
        kRem = gkv.tile([128, CT, TO], BF16, tag="kRem")
        nc.gpsimd.memset(vOwn[:, :, :, 64:128], 1.0 / SY)
        nc.gpsimd.memset(vRem[:, :, :, 64:128], 1.0 / SY)

        def ln_full(xin, xh, xl, ps_stat, ps_bc):
            """(xh + xl) ~= SX * (xin - mu) * rstd in fp8, all 512 tokens."""
            qs = slice(0, TO)
            s_ps = ps_stat.tile([1, TO], F32, space="PSUM", tag="stat")
            q_ps = ps_stat.tile([1, TO], F32, space="PSUM", tag="stat")
            for kt in range(CT):
                nc.tensor.matmul(s_ps[:], ones_col[:], xin[:, kt, qs],
                                 start=(kt == 0), stop=(kt == CT - 1))
            for kt in range(CT):
                x2 = gx2.tile([128, TO], F32R, tag="x2")
                nc.gpsimd.tensor_tensor(
                    out=x2[:], in0=xin[:, kt, qs],
                    in1=xin[:, kt, qs], op=ALU.mult)
                nc.tensor.matmul(q_ps[:], ones_col[:], x2[:],
                                 start=(kt == 0), stop=(kt == CT - 1))
            mu = gmicro.tile([1, TO], F32R, tag="mu")
            nc.scalar.mul(mu[:], s_ps[:], 1.0 / C)
            mu2 = gmicro.tile([1, TO], F32, tag="mu2")
            nc.scalar.activation(mu2[:], mu[:], ACT.Square)
            var = gmicro.tile([1, TO], F32, tag="var")
            nc.vector.scalar_tensor_tensor(
                out=var[:], in0=q_ps[:], scalar=1.0 / C, in1=mu2[:],
                op0=ALU.mult, op1=ALU.subtract)
            sd = gmicro.tile([1, TO], F32, tag="sd")
            nc.scalar.activation(sd[:], var[:], ACT.Sqrt, bias=epsh_t[:],
                                 scale=1.0 / (SX * SX))
            r = gmicro.tile([1, TO], F32R, tag="r")
            nc.vector.reciprocal(r[:], sd[:])            # SX/sd
            mr = gmicro.tile([1, TO], F32R, tag="mr")
            nc.vector.tensor_tensor(out=mr[:], in0=mu[:], in1=r[:],
                                    op=ALU.mult)
            bc = ps_bc.tile([128, TO], F32, space="PSUM", tag="bc")
            nc.tensor.matmul(bc[:], ones_row[:], r[:], start=True, stop=True)
            bc2 = ps_bc.tile([128, TO], F32, space="PSUM", tag="bc")
            nc.tensor.matmul(bc2[:], ones_row[:], mr[:], start=True,
                             stop=True)
            for kt in range(CT):
                t = gx2.tile([128, TO], F32, tag="lnt")
                nc.vector.tensor_tensor(out=t[:],
                                        in0=xin[:, kt, qs].bitcast(F32),
                                        in1=bc[:], op=ALU.mult)
                u = gu.tile([128, TO], F32, tag="lnu")
                nc.vector.tensor_tensor(out=u[:], in0=t[:],
                                        in1=bc2[:], op=ALU.subtract)
                nc.scalar.copy(xh[:, kt, :], u[:])
                nc.gpsimd.tensor_tensor(out=xl[:, kt, :], in0=u[:],
                                        in1=xh[:, kt, :],
                                        op=ALU.subtract)

        def ln_pools():
            i = ln_ctr[0]
            ln_ctr[0] += 1
            ps_stat = tc.alloc_tile_pool(name=f"ps_st{i}", bufs=2,
                                         space="PSUM")
            ps_bc = tc.alloc_tile_pool(name=f"ps_bc{i}", bufs=2, space="PSUM")
            return ps_stat, ps_bc

        def dr_mm(ps, wh, wl, xh, xl, qs, kps):
            """accumulate 3-term fp8 DR: Wh@xh + Wh@xl + Wl@xh into ps."""
            n = 3 * kps
            i = 0
            for w_, x_ in ((wh, xh), (wh, xl), (wl, xh)):
                for kp in range(kps):
                    nc.tensor.matmul(ps, w_[:, kp, :, :],
                                     x_[:, 2 * kp:2 * kp + 2, qs],
                                     start=(i == 0), stop=(i == n - 1),
                                     perf_mode=DR)
                    i += 1

        def mask_mm(sps, half, msrc):
            nc.tensor.matmul(
                sps[:, half * AQ:(half + 1) * AQ],
                iden_t[:],
                msrc.rearrange("p (i q) -> p i q", i=1)
                .broadcast_to([128, 2, AQ]),
                start=False, stop=True, perf_mode=DR,
                skip_group_check=True)

        def score_block(hp, rows, ksrc, kbase, qsl, msrc):
            """one AQ x AQ*?? score block: k tiles (kbase, kbase+1) of ksrc
            vs q columns qsl; optional additive mask (None | AP)."""
            sps = ps_sc.tile([128, 2 * AQ], F32, space="PSUM", tag="sc",
                             name="sps")
            for half in range(2):
                kt = kbase + half
                nc.tensor.matmul(
                    sps[:, half * AQ:(half + 1) * AQ],
                    ksrc[rows, 6 + hp, kt * 128:(kt + 1) * 128]
                    if ksrc is qkT else
                    ksrc[rows, hp, kt * 128:(kt + 1) * 128],
                    qkT[rows, hp, qsl],
                    start=True, stop=(msrc is None),
                    skip_group_check=(msrc is not None))
                if msrc is not None:
                    mask_mm(sps, half, msrc[:, half, :])
            e = pE.tile([128, 2 * AQ], BF16, tag="E", name="e")
            nc.scalar.activation(e[:], sps[:], ACT.Exp, scale=0.125)
            return e

        for l in range(n_layers):
            pa = tc.alloc_tile_pool(name=f"pa{l}", bufs=1)

            # ---- LN1 -> xn8 h/l (fp8) ----
            xnh = gxn.tile([128, CT, TO], FP8, tag="xnh")
            xnl = gxn.tile([128, CT, TO], FP8, tag="xnl")
            qkT = pa.tile([128, 12, TO], BF16, tag="qkT")
            st1, bc1 = ln_pools()
            ln_full(xT, xnh, xnl, st1, bc1)

            # ---- k projections first so the k-RS starts early ----
            def qk_slots(slots):
                for s_ in slots:
                    wh = gw.tile([128, KP, 2, 128], FP8, tag="w6h",
                                 name="wh")
                    nc.sync.dma_start(
                        out=wh[:], in_=wqh[l].ap()[s_ * 128:(s_ + 1) * 128])
                    wl = gw.tile([128, KP, 2, 128], FP8, tag="w6l",
                                 name="wl")
                    nc.sync.dma_start(
                        out=wl[:], in_=wql[l].ap()[s_ * 128:(s_ + 1) * 128])
                    ps = ps_big.tile([128, TO], F32, space="PSUM", tag="px",
                                     name="ps")
                    dr_mm(ps[:], wh, wl, xnh, xnl, slice(0, TO), KP)
                    nc.vector.tensor_scalar_mul(qkT[:, s_, :], ps[:],
                                                sc[f"dq_qk{l}"])

            qk_slots(range(6, 12))
            qk_slots(range(6))
            for p in (bc1, st1):
                p.release()

            # ---- k export: stage [k*m0, k*m1], RS, import peer k ----
            kst = gst.tile([128, 2, CT, TO], BF16, tag="kst")
            for sl in range(2):
                nc.vector.tensor_scalar_mul(kst[:, sl], qkT[:, 6:12, :],
                                            mm_t[:, sl:sl + 1])
            k_in = dram.tile([2, 128, CT, TO], BF16, tag="k_in")
            k_out = dram.tile([128, CT, TO], BF16, tag="k_out")
            nc.sync.dma_start(
                out=k_in[:].rearrange("a p c t -> p a c t"), in_=kst[:])
            nc.gpsimd.collective_compute(
                "ReduceScatter", ALU.add, replica_groups=RG,
                ins=[k_in.opt()], outs=[k_out.opt()])
            nc.sync.dma_start(out=kRem[:], in_=k_out[:])

            # ---- v projection (tokens on PSUM partitions) ----
            wvht = gwv.tile([128, KP, 2, C], FP8, tag="vwh")
            nc.sync.dma_start(out=wvht[:], in_=wvh[l].ap()[:])
            wvlt = gwv.tile([128, KP, 2, C], FP8, tag="vwl")
            nc.sync.dma_start(out=wvlt[:], in_=wvl[l].ap()[:])
            for tt in range(TT):
                tsl = slice(tt * 128, (tt + 1) * 128)
                psA = ps_big.tile([128, 512], F32, space="PSUM", tag="px")
                psB = ps_big.tile([128, 256], F32, space="PSUM", tag="px")
                n = 3 * KP
                for ps_, csl in ((psA, slice(0, 512)),
                                 (psB, slice(512, 768))):
                    i = 0
                    for x_, w_ in ((xnh, wvht), (xnl, wvht), (xnh, wvlt)):
                        for kp in range(KP):
                            nc.tensor.matmul(ps_[:],
                                             x_[:, 2 * kp:2 * kp + 2, tsl],
                                             w_[:, kp, :, csl],
                                             start=(i == 0),
                                             stop=(i == n - 1),
                                             perf_mode=DR)
                            i += 1
                nc.vector.tensor_scalar_mul(
                    vOwn[:, tt, 0:8, 0:64],
                    psA[:].rearrange("p (h d) -> p h d", h=8),
                    sc[f"dq_v{l}"])
                nc.vector.tensor_scalar_mul(
                    vOwn[:, tt, 8:12, 0:64],
                    psB[:].rearrange("p (h d) -> p h d", h=4),
                    sc[f"dq_v{l}"])

            # ---- v export ----
            vst = gst.tile([128, 2, TT, H, 64], BF16, tag="vst")
            for sl in range(2):
                nc.vector.tensor_scalar_mul(vst[:, sl],
                                            vOwn[:, :, :, 0:64],
                                            mm_t[:, sl:sl + 1])
            v_in = dram.tile([2, 128, TT, H, 64], BF16, tag="v_in")
            v_out = dram.tile([128, TT, H, 64], BF16, tag="v_out")
            nc.sync.dma_start(
                out=v_in[:].rearrange("a p t h d -> p a t h d"), in_=vst[:])
            nc.gpsimd.collective_compute(
                "ReduceScatter", ALU.add, replica_groups=RG,
                ins=[v_in.opt()], outs=[v_out.opt()])
            nc.sync.dma_start(out=vRem[:, :, :, 0:64], in_=v_out[:])

            # ---- attention ----
            yT = pa.tile([128, CT, TO], BF16, tag="yT")
            pE = tc.alloc_tile_pool(name=f"pE{l}", bufs=10)
            prec = tc.alloc_tile_pool(name=f"prec{l}", bufs=4)
            gyo = tc.alloc_tile_pool(name=f"gyo{l}", bufs=1)
            ps_sc = tc.alloc_tile_pool(name=f"ps_sc{l}", bufs=2, space="PSUM")
            ps_av = tc.alloc_tile_pool(name=f"ps_av{l}", bufs=2, space="PSUM")
            yo = gyo.tile([128, 12, 2, AQ], F32, tag="yo")  # own partials

            qA, qB = slice(0, AQ), slice(AQ, 2 * AQ)
            # phase 1: own blocks (independent of the RS)
            for hp in range(6):
                for h in (2 * hp, 2 * hp + 1):
                    par = h % 2
                    rows = slice(64 * par, 64 * par + 64)
                    eA = score_block(hp, rows, qkT, 0, qA, masks_t)
                    eB1 = score_block(hp, rows, qkT, 0, qB, None)
                    eB2 = score_block(hp, rows, qkT, 2, qB, masks_t)
                    ya = ps_av.tile([128, AQ], F32, space="PSUM", tag="av")
                    for kt in range(2):
                        nc.tensor.matmul(ya[:], vOwn[:, kt, h, :],
                                         eA[:, kt * AQ:(kt + 1) * AQ],
                                         start=(kt == 0), stop=(kt == 1))
                    nc.vector.tensor_copy(out=yo[:, h, 0, :], in_=ya[:])
                    yb = ps_av.tile([128, AQ], F32, space="PSUM", tag="av")
                    for kt in range(4):
                        e = eB1 if kt < 2 else eB2
                        nc.tensor.matmul(yb[:], vOwn[:, kt, h, :],
                                         e[:, (kt % 2) * AQ:(kt % 2 + 1) * AQ],
                                         start=(kt == 0), stop=(kt == 3))
                    nc.vector.tensor_copy(out=yo[:, h, 1, :], in_=yb[:])
            # phase 2: remote blocks + combine
            for hp in range(6):
                for h in (2 * hp, 2 * hp + 1):
                    par = h % 2
                    rows = slice(64 * par, 64 * par + 64)
                    eAr = score_block(hp, rows, kRem, 0, qA, masks2_t[:, 0])
                    eBr1 = score_block(hp, rows, kRem, 0, qB, None)
                    eBr2 = score_block(hp, rows, kRem, 2, qB,
                                       masks2_t[:, 1])
                    for qi, es in ((0, (eAr, eAr)), (1, (eBr1, eBr2))):
                        nkt = 2 if qi == 0 else 4
                        yr = ps_av.tile([128, AQ], F32, space="PSUM",
                                        tag="av")
                        for kt in range(nkt):
                            e = es[0] if kt < 2 else es[1]
                            nc.tensor.matmul(
                                yr[:], vRem[:, kt, h, :],
                                e[:, (kt % 2) * AQ:(kt % 2 + 1) * AQ],
                                start=(kt == 0), stop=(kt == nkt - 1))
                        yc = prec.tile([128, AQ], F32, tag="yc")
                        nc.vector.tensor_tensor(out=yc[:], in0=yr[:],
                                                in1=yo[:, h, qi, :],
                                                op=ALU.add)
                        rec = prec.tile([64, AQ], F32, tag="rec")
                        nc.vector.reciprocal(rec[:], yc[64:128, :])
                        qsl = slice(qi * AQ, (qi + 1) * AQ)
                        yrow = slice(64 * par, 64 * par + 64)
                        nc.vector.tensor_tensor(out=yT[yrow, hp, qsl],
                                                in0=yc[0:64, :], in1=rec[:],
                                                op=ALU.mult)
            for p in (ps_av, ps_sc, gyo, prec, pE):
                p.release()

            # ---- y8 split, proj + residual, LN2 + MLP ----
            py8 = tc.alloc_tile_pool(name=f"py8{l}", bufs=1)
            ph8 = tc.alloc_tile_pool(name=f"ph8{l}", bufs=1)
            phbf = tc.alloc_tile_pool(name=f"phbf{l}", bufs=3)
            y8h = py8.tile([128, CT, TO], FP8, tag="y8h")
            y8l = py8.tile([128, CT, TO], FP8, tag="y8l")
            for kt in range(CT):
                nc.gpsimd.tensor_copy(out=y8h[:, kt, :], in_=yT[:, kt, :])
                nc.gpsimd.tensor_tensor(
                    out=y8l[:, kt, :], in0=yT[:, kt, :],
                    in1=y8h[:, kt, :], op=ALU.subtract)
            for ot in range(CT):
                wh = gw.tile([128, KP, 2, 128], FP8, tag="w6h")
                nc.sync.dma_start(out=wh[:],
                                  in_=wph[l].ap()[ot * 128:(ot + 1) * 128])
                wl = gw.tile([128, KP, 2, 128], FP8, tag="w6l")
                nc.sync.dma_start(out=wl[:],
                                  in_=wpl[l].ap()[ot * 128:(ot + 1) * 128])
                ps = ps_big.tile([128, TO], F32, space="PSUM", tag="px")
                dr_mm(ps[:], wh, wl, y8h, y8l, slice(0, TO), KP)
                nc.vector.scalar_tensor_tensor(
                    out=xT[:, ot, :], in0=ps[:], scalar=sc[f"dq_p{l}"],
                    in1=xT[:, ot, :], op0=ALU.mult, op1=ALU.add)
            xn2h = gxn.tile([128, CT, TO], FP8, tag="xnh")
            xn2l = gxn.tile([128, CT, TO], FP8, tag="xnl")
            st2, bc2p = ln_pools()
            ln_full(xT, xn2h, xn2l, st2, bc2p)
            h8h = ph8.tile([128, HT, TO], FP8, tag="h8h")
            h8l = ph8.tile([128, HT, TO], FP8, tag="h8l")
            for ot in range(HT):
                wh = gw.tile([128, KP, 2, 128], FP8, tag="w6h")
                nc.sync.dma_start(out=wh[:],
                                  in_=wfh[l].ap()[ot * 128:(ot + 1) * 128])
                wl = gw.tile([128, KP, 2, 128], FP8, tag="w6l")
                nc.sync.dma_start(out=wl[:],
                                  in_=wfl[l].ap()[ot * 128:(ot + 1) * 128])
                ps = ps_big.tile([128, TO], F32, space="PSUM", tag="px")
                dr_mm(ps[:], wh, wl, xn2h, xn2l, slice(0, TO), KP)
                hbf = phbf.tile([128, TO], BF16, tag="hbf")
                nc.scalar.activation(h8h[:, ot, :], ps[:], ACT.Gelu,
                                     scale=sc[f"dq_fc{l}"])
                nc.scalar.activation(hbf[:], ps[:], ACT.Gelu,
                                     scale=sc[f"dq_fc{l}"])
                nc.gpsimd.tensor_tensor(
                    out=h8l[:, ot, :], in0=hbf[:],
                    in1=h8h[:, ot, :], op=ALU.subtract)
            for ot in range(CT):
                wh = gw24.tile([128, HP, 2, 128], FP8, tag="w24h")
                nc.sync.dma_start(out=wh[:],
                                  in_=w2h[l].ap()[ot * 128:(ot + 1) * 128])
                wl = gw24.tile([128, HP, 2, 128], FP8, tag="w24l")
                nc.sync.dma_start(out=wl[:],
                                  in_=w2l[l].ap()[ot * 128:(ot + 1) * 128])
                ps = ps_big.tile([128, TO], F32, space="PSUM", tag="px")
                dr_mm(ps[:], wh, wl, h8h, h8l, slice(0, TO), HP)
                nc.vector.scalar_tensor_tensor(
                    out=xT[:, ot, :], in0=ps[:], scalar=sc[f"dq_f2{l}"],
                    in1=xT[:, ot, :], op0=ALU.mult, op1=ALU.add)
            for p in (bc2p, st2, phbf, ph8, py8, pa):
                p.release()

        # ---- final LN (fp8 h/l, scale SX) + token-split LM head ----
        for p in (gv, dram, gst, gkv, gw24, gwv, gw, gxn):
            p.release()
        pf = tc.alloc_tile_pool(name="pf", bufs=1)
        xf = pf.tile([128, CT, TO], FP8, tag="xf")
        xl = pf.tile([128, CT, TO], FP8, tag="xl")
        stf, bcf = ln_pools()
        ln_full(xT, xf, xl, stf, bcf)
        for p in (bcf, stf):
            p.release()

        ph = tc.alloc_tile_pool(name="ph", bufs=4)
        pout = tc.alloc_tile_pool(name="pout", bufs=2)
        for vc in range(NVC):
            wh8 = ph.tile([128, KP, 2, 512], FP8, tag="wh")
            nc.sync.dma_start(out=wh8[:],
                              in_=whh_d.ap()[vc * 128:(vc + 1) * 128])
            wl8 = ph.tile([128, KP, 2, 512], FP8, tag="whl")
            nc.sync.dma_start(out=wl8[:],
                              in_=whl_d.ap()[vc * 128:(vc + 1) * 128])
            o = pout.tile([128, TT, 512], BF16, tag="out")
            for tt in range(TT):
                tsl = slice(tt * 128, (tt + 1) * 128)
                ps = ps_big.tile([128, 512], F32, space="PSUM", tag="px")
                terms = [(xf, wh8), (xl, wh8), (xf, wl8)]
                for cc in range(2):
                    i = 0
                    for xsrc, wsrc in terms:
                        for kp in range(KP):
                            nc.tensor.matmul(
                                ps[:, cc * 256:(cc + 1) * 256],
                                xsrc[:, 2 * kp:2 * kp + 2, tsl],
                                wsrc[:, kp, :, cc * 256:cc * 256 + 256],
                                start=(i == 0), stop=(i == 3 * KP - 1),
                                perf_mode=DR)
                            i += 1
                if tt % 2 == 0:
                    nc.vector.tensor_scalar_mul(o[:, tt, :], ps[:],
                                                sc["dq_h"])
                else:
                    nc.scalar.mul(o[:, tt, :], ps[:], sc["dq_h"])
            nc.sync.dma_start(
                out=logits_d.ap()[:, vc * 512:(vc + 1) * 512].rearrange(
                    "(t p) v -> p t v", p=128),
                in_=o[:])
        for p in (pout, ph, pf, ps_big, gmicro, gu, gx2, gx, glob):
            p.release()

    nc.compile()
    return nc

# ---------------------------------------------------------------------------
# host side
# ---------------------------------------------------------------------------

def _pow2_scale(m, target=224.0):
    if m == 0:
        return 1.0
    return float(2.0 ** np.floor(np.log2(target / m)))


def _hi_lo(w, s):
    ws = w * s
    hi = ws.astype(E4)
    lo = (ws - hi.astype(np.float32)).astype(E4)
    return hi, lo


# own global AQ-blocks per half, in local order
OWN_BLOCKS = {0: (0, 3), 1: (1, 2)}


def _prep_inputs(inputs, n_layers):
    f32 = np.float32
    idx = np.asarray(inputs["idx"])
    wte = np.asarray(inputs["wte"], f32)
    wpe = np.asarray(inputs["wpe"], f32)

    sc = {}
    common = {}
    for l in range(n_layers):
        ln1w = np.asarray(inputs["ln1_w"][l], f32)
        ln1b = np.asarray(inputs["ln1_b"][l], f32)
        aw = np.asarray(inputs["attn_w"][l], f32)
        ab = np.asarray(inputs["attn_b"][l], f32)
        awf = ln1w[:, None] * aw
        abf = ab + ln1b @ aw
        assert not np.any(abf), "nonzero attn bias not supported"
        qk = awf[:, :1536]
        s_qk = _pow2_scale(float(np.abs(qk).max()))
        qh, ql = _hi_lo(qk, s_qk)

        def qk_layout(a):
            return np.ascontiguousarray(
                a.reshape(KP, 2, 128, 12, 128).transpose(3, 2, 0, 1, 4)
            ).reshape(12 * 128, KP, 2, 128)
        common[f"qkwh{l}"] = qk_layout(qh)
        common[f"qkwl{l}"] = qk_layout(ql)
        sc[f"dq_qk{l}"] = 1.0 / (SX * s_qk)
        vw = awf[:, 1536:]
        s_v = _pow2_scale(float(np.abs(vw).max()))
        vh, vl = _hi_lo(vw, s_v)

        def v_layout(a):
            return np.ascontiguousarray(
                a.reshape(KP, 2, 128, C).transpose(2, 0, 1, 3))
        common[f"vwh{l}"] = v_layout(vh)
        common[f"vwl{l}"] = v_layout(vl)
        sc[f"dq_v{l}"] = 1.0 / (SX * s_v)

        pw = np.asarray(inputs["proj_w"][l], f32)
        assert not np.any(np.asarray(inputs["proj_b"][l])), "proj bias"
        s_p = _pow2_scale(float(np.abs(pw).max()))
        ph_, pl_ = _hi_lo(pw, s_p)

        def p_layout(a):
            return np.ascontiguousarray(
                a.reshape(KP, 2, 128, CT, 128).transpose(3, 2, 0, 1, 4)
            ).reshape(CT * 128, KP, 2, 128)
        common[f"pwh{l}"] = p_layout(ph_)
        common[f"pwl{l}"] = p_layout(pl_)
        sc[f"dq_p{l}"] = 1.0 / (SY * s_p)

        ln2w = np.asarray(inputs["ln2_w"][l], f32)
        ln2b = np.asarray(inputs["ln2_b"][l], f32)
        fw = np.asarray(inputs["fc_w"][l], f32)
        fbv = np.asarray(inputs["fc_b"][l], f32)
        fwf = ln2w[:, None] * fw
        fbf = fbv + ln2b @ fw
        assert not np.any(fbf), "nonzero fc bias not supported"
        s_fc = _pow2_scale(float(np.abs(fwf).max()))
        fh, fl = _hi_lo(fwf, s_fc)

        def f_layout(a):
            return np.ascontiguousarray(
                a.reshape(KP, 2, 128, HT, 128).transpose(3, 2, 0, 1, 4)
            ).reshape(HT * 128, KP, 2, 128)
        common[f"fwh{l}"] = f_layout(fh)
        common[f"fwl{l}"] = f_layout(fl)
        sc[f"dq_fc{l}"] = 1.0 / (SX * s_fc)

        f2w = np.asarray(inputs["fc2_w"][l], f32)
        assert not np.any(np.asarray(inputs["fc2_b"][l])), "fc2 bias"
        s_f2 = _pow2_scale(float(np.abs(f2w).max()))
        f2h, f2l = _hi_lo(f2w, s_f2)

        def f2_layout(a):
            return np.ascontiguousarray(
                a.reshape(HP, 2, 128, CT, 128).transpose(3, 2, 0, 1, 4)
            ).reshape(CT * 128, HP, 2, 128)
        common[f"f2wh{l}"] = f2_layout(f2h)
        common[f"f2wl{l}"] = f2_layout(f2l)
        sc[f"dq_f2{l}"] = 1.0 / (SH * s_f2)

    p = np.arange(128)[:, None]
    f = np.arange(AQ)[None, :]
    masks = np.zeros((128, 2, AQ), f32)
    masks[:, 0, :] = np.where(p > f, -240.0, 0.0)
    masks[:, 1, :] = np.where(p + 128 > f, -240.0, 0.0)
    common["masks"] = masks.astype(E4)
    iden = np.zeros((128, 2, 128), f32)
    iden[:, 0, :] = 128.0 * np.eye(128)
    common["iden"] = iden.astype(E4)

    lnfw = np.asarray(inputs["lnf_w"], f32)
    lnfb = np.asarray(inputs["lnf_b"], f32)
    assert not np.any(lnfb @ wte.T), "nonzero head bias not supported"
    wh = lnfw[:, None] * wte.T                     # [768, V]
    whp = np.zeros((C, VS2), f32)
    whp[:, :V] = wh
    m = float(np.abs(wh).max())
    s_h = float(2.0 ** np.floor(np.log2(240.0 / m)))
    sc["dq_h"] = 1.0 / (s_h * SX)

    slh = whp * s_h
    hih = slh.astype(E4)
    loh = (slh - hih.astype(f32)).astype(E4)
    common["whh"] = np.ascontiguousarray(
        hih.reshape(KP, 2, 128, NVC, 512).transpose(3, 2, 0, 1, 4)
    ).reshape(NVC * 128, KP, 2, 512)
    common["whl"] = np.ascontiguousarray(
        loh.reshape(KP, 2, 128, NVC, 512).transpose(3, 2, 0, 1, 4)
    ).reshape(NVC * 128, KP, 2, 512)

    def t6(a):          # [768, TO] -> [128, 6, TO]
        return np.ascontiguousarray(
            a.reshape(CT, 128, a.shape[1]).transpose(1, 0, 2))

    x0 = wte[idx] + wpe[None, :T]                  # [B, T, C]
    in_maps = []
    for c in range(8):
        s, half = c & 3, c >> 2
        bA, bB = OWN_BLOCKS[half]
        m2 = dict(common)
        xo = np.concatenate([x0[s, bA * AQ:(bA + 1) * AQ],
                             x0[s, bB * AQ:(bB + 1) * AQ]], axis=0)
        m2["x0t"] = t6(np.ascontiguousarray(xo.T))
        # RS staging masks: slot s2 carries my data iff my rank != s2
        mm = np.zeros((128, 2), f32)
        mm[:, 1 - half] = 1.0
        m2["mm"] = mm
        # data masks: mA for qA x remA ; mB for qB x remB
        m2d = np.zeros((128, 2, 2, AQ), f32)
        if half == 0:
            m2d[:, 0, :, :] = -240.0        # qA(blk0) x rA(blk1): future
            # qB(blk3) x rB(blk2): full attend -> 0
        else:
            # qA(blk1) x rA(blk0): full attend -> 0
            m2d[:, 1, :, :] = -240.0        # qB(blk2) x rB(blk3): future
        m2["masks2"] = m2d.astype(E4)
        in_maps.append(m2)
    return in_maps, sc


def kernel(**inputs):
    global LAST_RESULT, LAST_NC
    n_layers = L
    in_maps, sc = _prep_inputs(inputs, n_layers)
    key = (n_layers, tuple(sorted(sc.items())))
    if key not in _CACHE:
        _CACHE[key] = build_program(sc, n_layers)
    nc = _CACHE[key]
    LAST_NC = nc
    res = run_bass_kernel_spmd(nc, in_maps, core_ids=list(range(8)))
    LAST_RESULT = res
    out = np.empty((B, T, V), np.float32)
    for c in range(8):
        s, half = c & 3, c >> 2
        bA, bB = OWN_BLOCKS[half]
        part = np.asarray(res.results[c]["logits"]).astype(np.float32)
        out[s, bA * AQ:(bA + 1) * AQ] = part[0:AQ, :V]
        out[s, bB * AQ:(bB + 1) * AQ] = part[AQ:2 * AQ, :V]
    return out


if __name__ == "__main__":
    rng = np.random.default_rng(0)
    ins = {
        "idx": rng.integers(0, V, (B, T)).astype(np.int32),
        "wte": (rng.standard_normal((V, C)) * 0.02).astype(np.float32),
        "wpe": (rng.standard_normal((T, C)) * 0.02).astype(np.float32),
        "ln1_w": np.ones((L, C), np.float32),
        "ln1_b": np.zeros((L, C), np.float32),
        "attn_w": (rng.standard_normal((L, C, 3 * C)) * 0.02).astype(np.float32),
        "attn_b": np.zeros((L, 3 * C), np.float32),
        "proj_w": (rng.standard_normal((L, C, C)) * 0.02).astype(np.float32),
        "proj_b": np.zeros((L, C), np.float32),
        "ln2_w": np.ones((L, C), np.float32),
        "ln2_b": np.zeros((L, C), np.float32),
        "fc_w": (rng.standard_normal((L, C, 4 * C)) * 0.02).astype(np.float32),
        "fc_b": np.zeros((L, 4 * C), np.float32),
        "fc2_w": (rng.standard_normal((L, 4 * C, C)) * 0.02).astype(np.float32),
        "fc2_b": np.zeros((L, C), np.float32),
        "lnf_w": np.ones((C,), np.float32),
        "lnf_b": np.zeros((C,), np.float32),
    }
    out = kernel(**ins)
    print("out", out.shape, out.dtype, float(np.abs(out).max()))


# revision 17
# speedup vs baseline: 1.2175x; 1.0152x over previous
"""MiniGPT forward on 8 Trainium2 NeuronCores — sequence-split variant.

Core c: sequence (c & 3), token-half (c >> 2).  In AQ=256 blocks of the
1024-token sequence, half 0 owns blocks {0, 3}, half 1 owns {1, 2} (equal
causal-attention load).  Each core runs the 6 transformer blocks for its
OWN 512 tokens only — no duplicated block compute.  Per layer the pair
exchanges K and V via two ReduceScatters (k first): each core stages
[k*m0, k*m1] with per-core {0,1} masks so the RS output is exactly the
PEER's k/v at a uniform address (SPMD-safe, no rank branching).
Attention = own-block phase (overlaps the RS) + remote phase; causal /
validity masks are per-core input data applied on the PE via the
iden @ mask DoubleRow trick.  One remote block per core is fully masked
waste, keeping the instruction stream identical across cores.

Block linears are fp8(e4m3) DoubleRow 3-term (Wh@xh + Wh@xl + Wl@xh,
0.75x bf16 PE cost); attention stays bf16.  The LM head is token-split:
each core computes its own 512 tokens x the FULL vocab with the same
3-term fp8 scheme, logits stream out in bf16.
"""

import sys

sys.path.insert(0, "/opt/trn_rl_repo")

import numpy as np
import ml_dtypes

import concourse.bacc as bacc
import concourse.tile as tile
from concourse import mybir
from concourse.bass_utils import run_bass_kernel_spmd

F32 = mybir.dt.float32
F32R = mybir.dt.float32r
BF16 = mybir.dt.bfloat16
FP8 = mybir.dt.float8e4
ALU = mybir.AluOpType
ACT = mybir.ActivationFunctionType
DR = mybir.MatmulPerfMode.DoubleRow
E4 = ml_dtypes.float8_e4m3

B, T, C, H, HD, L, V = 4, 1024, 768, 12, 64, 6, 50257
TO = 512                # own tokens per core
CT = C // 128           # 6 c-tiles
KP = CT // 2            # 3 k-pairs
TT = TO // 128          # 4 own token tiles
AQ = 256                # attention query block
HT = 3072 // 128        # 24 hidden tiles
HP = HT // 2            # 12 hidden k-pairs
VS2 = 51200             # padded vocab
NVC = VS2 // 512        # 100
EPS = 1e-5
SX = 16.0               # fp8 scale for LN outputs (blocks + head)
SY = 32.0               # fp8 scale for attention output y (folded in ones)
SH = 1.0                # gelu output used unscaled in fp8

RG = [[0, 4], [1, 5], [2, 6], [3, 7]]   # pair replica groups

_CACHE = {}
LAST_RESULT = None
LAST_NC = None


def build_program(sc, n_layers=L):
    nc = bacc.Bacc(None, target_bir_lowering=False)

    def f8_in(name, shape):
        return nc.dram_tensor(name, list(shape), FP8, kind="ExternalInput")

    x0t_d = nc.dram_tensor("x0t", [128, CT, TO], F32R, kind="ExternalInput")
    wqh, wql, wvh, wvl, wph, wpl = [], [], [], [], [], []
    wfh, wfl, w2h, w2l = [], [], [], []
    for l in range(n_layers):
        wqh.append(f8_in(f"qkwh{l}", (12 * 128, KP, 2, 128)))
        wql.append(f8_in(f"qkwl{l}", (12 * 128, KP, 2, 128)))
        wvh.append(f8_in(f"vwh{l}", (128, KP, 2, C)))
        wvl.append(f8_in(f"vwl{l}", (128, KP, 2, C)))
        wph.append(f8_in(f"pwh{l}", (CT * 128, KP, 2, 128)))
        wpl.append(f8_in(f"pwl{l}", (CT * 128, KP, 2, 128)))
        wfh.append(f8_in(f"fwh{l}", (HT * 128, KP, 2, 128)))
        wfl.append(f8_in(f"fwl{l}", (HT * 128, KP, 2, 128)))
        w2h.append(f8_in(f"f2wh{l}", (CT * 128, HP, 2, 128)))
        w2l.append(f8_in(f"f2wl{l}", (CT * 128, HP, 2, 128)))
    whh_d = f8_in("whh", (NVC * 128, KP, 2, 512))
    whl_d = f8_in("whl", (NVC * 128, KP, 2, 512))
    masks_d = f8_in("masks", (128, 2, AQ))          # local diag (shared)
    masks2_d = f8_in("masks2", (128, 2, 2, AQ))     # [mA|mB, half, q] per-core
    iden_d = f8_in("iden", (128, 2, 128))
    mm_d = nc.dram_tensor("mm", [128, 2], F32, kind="ExternalInput")
    logits_d = nc.dram_tensor("logits", [TO, VS2], BF16,
                              kind="ExternalOutput")

    ln_ctr = [0]

    with nc.allow_low_precision("fp8 3-term error-feedback intentional"), \
         tile.TileContext(nc) as tc:
        glob = tc.alloc_tile_pool(name="glob", bufs=1)
        gx = tc.alloc_tile_pool(name="gx", bufs=1)
        gx2 = tc.alloc_tile_pool(name="gx2", bufs=2)
        gu = tc.alloc_tile_pool(name="gu", bufs=2)
        gmicro = tc.alloc_tile_pool(name="gmicro", bufs=1)
        ps_big = tc.alloc_tile_pool(name="ps_big", bufs=4, space="PSUM")
        gxn = tc.alloc_tile_pool(name="gxn", bufs=1)
        gw = tc.alloc_tile_pool(name="gw", bufs=5)
        gwv = tc.alloc_tile_pool(name="gwv", bufs=1)
        gw24 = tc.alloc_tile_pool(name="gw24", bufs=2)
        gkv = tc.alloc_tile_pool(name="gkv", bufs=1)
        gst = tc.alloc_tile_pool(name="gst", bufs=1)
        dram = tc.alloc_tile_pool(name="dram", bufs=2, space="DRAM")

        ones_col = glob.tile([128, 1], F32R, tag="ones_col")
        ones_row = glob.tile([1, 128], F32R, tag="ones_row")
        epsh_t = glob.tile([1, 1], F32, tag="epsh")
        masks_t = glob.tile([128, 2, AQ], FP8, tag="masks")
        masks2_t = glob.tile([128, 2, 2, AQ], FP8, tag="masks2")
        iden_t = glob.tile([128, 2, 128], FP8, tag="iden")
        mm_t = glob.tile([128, 2], F32, tag="mm")
        nc.vector.memset(ones_col[:].bitcast(F32), 1.0)
        nc.vector.memset(ones_row[:].bitcast(F32), 1.0)
        nc.vector.memset(epsh_t[:], EPS / (SX * SX))
        nc.sync.dma_start(out=masks_t[:], in_=masks_d[:])
        nc.sync.dma_start(out=masks2_t[:], in_=masks2_d[:])
        nc.sync.dma_start(out=iden_t[:], in_=iden_d[:])
        nc.sync.dma_start(out=mm_t[:], in_=mm_d[:])

        xT = gx.tile([128, CT, TO], F32R, tag="xT")
        nc.sync.dma_start(out=xT[:], in_=x0t_d[:])

        # persistent v tiles: [p, tt, h, 0:64] = v ; [.., 64:128] = 1/SY
        gv = tc.alloc_tile_pool(name="gv", bufs=1)
        vOwn = gv.tile([128, TT, H, 128], BF16, tag="vOwn")
        vRem = gv.tile([128, TT, H, 128], BF16, tag="vRem")
        kRem = gkv.tile([128, CT, TO], BF16, tag="kRem")
        nc.gpsimd.memset(vOwn[:, :, :, 64:128], 1.0 / SY)
        nc.gpsimd.memset(vRem[:, :, :, 64:128], 1.0 / SY)

        def ln_full(xin, xh, xl, ps_stat, ps_bc):
            """(xh + xl) ~= SX * (xin - mu) * rstd in fp8, all 512 tokens."""
            qs = slice(0, TO)
            s_ps = ps_stat.tile([1, TO], F32, space="PSUM", tag="stat")
            q_ps = ps_stat.tile([1, TO], F32, space="PSUM", tag="stat")
            for kt in range(CT):
                nc.tensor.matmul(s_ps[:], ones_col[:], xin[:, kt, qs],
                                 start=(kt == 0), stop=(kt == CT - 1))
            for kt in range(CT):
                x2 = gx2.tile([128, TO], F32R, tag="x2")
                nc.gpsimd.tensor_tensor(
                    out=x2[:], in0=xin[:, kt, qs],
                    in1=xin[:, kt, qs], op=ALU.mult)
                nc.tensor.matmul(q_ps[:], ones_col[:], x2[:],
                                 start=(kt == 0), stop=(kt == CT - 1))
            mu = gmicro.tile([1, TO], F32R, tag="mu")
            nc.scalar.mul(mu[:], s_ps[:], 1.0 / C)
            mu2 = gmicro.tile([1, TO], F32, tag="mu2")
            nc.scalar.activation(mu2[:], mu[:], ACT.Square)
            var = gmicro.tile([1, TO], F32, tag="var")
            nc.vector.scalar_tensor_tensor(
                out=var[:], in0=q_ps[:], scalar=1.0 / C, in1=mu2[:],
                op0=ALU.mult, op1=ALU.subtract)
            sd = gmicro.tile([1, TO], F32, tag="sd")
            nc.scalar.activation(sd[:], var[:], ACT.Sqrt, bias=epsh_t[:],
                                 scale=1.0 / (SX * SX))
            r = gmicro.tile([1, TO], F32R, tag="r")
            nc.vector.reciprocal(r[:], sd[:])            # SX/sd
            mr = gmicro.tile([1, TO], F32R, tag="mr")
            nc.vector.tensor_tensor(out=mr[:], in0=mu[:], in1=r[:],
                                    op=ALU.mult)
            bc = ps_bc.tile([128, TO], F32, space="PSUM", tag="bc")
            nc.tensor.matmul(bc[:], ones_row[:], r[:], start=True, stop=True)
            bc2 = ps_bc.tile([128, TO], F32, space="PSUM", tag="bc")
            nc.tensor.matmul(bc2[:], ones_row[:], mr[:], start=True,
                             stop=True)
            for kt in range(CT):
                t = gx2.tile([128, TO], F32, tag="lnt")
                nc.vector.tensor_tensor(out=t[:],
                                        in0=xin[:, kt, qs].bitcast(F32),
                                        in1=bc[:], op=ALU.mult)
                u = gu.tile([128, TO], F32, tag="lnu")
                nc.vector.tensor_tensor(out=u[:], in0=t[:],
                                        in1=bc2[:], op=ALU.subtract)
                nc.scalar.copy(xh[:, kt, :], u[:])
                nc.gpsimd.tensor_tensor(out=xl[:, kt, :], in0=u[:],
                                        in1=xh[:, kt, :],
                                        op=ALU.subtract)

        def ln_pools():
            i = ln_ctr[0]
            ln_ctr[0] += 1
            ps_stat = tc.alloc_tile_pool(name=f"ps_st{i}", bufs=2,
                                         space="PSUM")
            ps_bc = tc.alloc_tile_pool(name=f"ps_bc{i}", bufs=2, space="PSUM")
            return ps_stat, ps_bc

        def dr_mm(ps, wh, wl, xh, xl, qs, kps):
            """accumulate 3-term fp8 DR: Wh@xh + Wh@xl + Wl@xh into ps."""
            n = 3 * kps
            i = 0
            for w_, x_ in ((wh, xh), (wh, xl), (wl, xh)):
                for kp in range(kps):
                    nc.tensor.matmul(ps, w_[:, kp, :, :],
                                     x_[:, 2 * kp:2 * kp + 2, qs],
                                     start=(i == 0), stop=(i == n - 1),
                                     perf_mode=DR)
                    i += 1

        def mask_mm(sps, half, msrc):
            nc.tensor.matmul(
                sps[:, half * AQ:(half + 1) * AQ],
                iden_t[:],
                msrc.rearrange("p (i q) -> p i q", i=1)
                .broadcast_to([128, 2, AQ]),
                start=False, stop=True, perf_mode=DR,
                skip_group_check=True)

        def score_block(hp, rows, ksrc, kbase, qsl, msrc):
            """one AQ x AQ*?? score block: k tiles (kbase, kbase+1) of ksrc
            vs q columns qsl; optional additive mask (None | AP)."""
            sps = ps_sc.tile([128, 2 * AQ], F32, space="PSUM", tag="sc",
                             name="sps")
            for half in range(2):
                kt = kbase + half
                nc.tensor.matmul(
                    sps[:, half * AQ:(half + 1) * AQ],
                    ksrc[rows, 6 + hp, kt * 128:(kt + 1) * 128]
                    if ksrc is qkT else
                    ksrc[rows, hp, kt * 128:(kt + 1) * 128],
                    qkT[rows, hp, qsl],
                    start=True, stop=(msrc is None),
                    skip_group_check=(msrc is not None))
                if msrc is not None:
                    mask_mm(sps, half, msrc[:, half, :])
            e = pE.tile([128, 2 * AQ], BF16, tag="E", name="e")
            nc.scalar.activation(e[:], sps[:], ACT.Exp, scale=0.125)
            return e

        for l in range(n_layers):
            pa = tc.alloc_tile_pool(name=f"pa{l}", bufs=1)

            # ---- LN1 -> xn8 h/l (fp8) ----
            xnh = gxn.tile([128, CT, TO], FP8, tag="xnh")
            xnl = gxn.tile([128, CT, TO], FP8, tag="xnl")
            qkT = pa.tile([128, 12, TO], BF16, tag="qkT")
            st1, bc1 = ln_pools()
            ln_full(xT, xnh, xnl, st1, bc1)

            # ---- k projections first so the k-RS starts early ----
            def qk_slots(slots):
                for s_ in slots:
                    wh = gw.tile([128, KP, 2, 128], FP8, tag="w6h",
                                 name="wh")
                    nc.sync.dma_start(
                        out=wh[:], in_=wqh[l].ap()[s_ * 128:(s_ + 1) * 128])
                    wl = gw.tile([128, KP, 2, 128], FP8, tag="w6l",
                                 name="wl")
                    nc.sync.dma_start(
                        out=wl[:], in_=wql[l].ap()[s_ * 128:(s_ + 1) * 128])
                    ps = ps_big.tile([128, TO], F32, space="PSUM", tag="px",
                                     name="ps")
                    dr_mm(ps[:], wh, wl, xnh, xnl, slice(0, TO), KP)
                    nc.vector.tensor_scalar_mul(qkT[:, s_, :], ps[:],
                                                sc[f"dq_qk{l}"])

            qk_slots(range(6, 12))
            qk_slots(range(6))
            for p in (bc1, st1):
                p.release()

            # ---- k export: stage [k*m0, k*m1], RS, import peer k ----
            kst = gst.tile([128, 2, CT, TO], BF16, tag="kst")
            for sl in range(2):
                nc.vector.tensor_scalar_mul(kst[:, sl], qkT[:, 6:12, :],
                                            mm_t[:, sl:sl + 1])
            k_in = dram.tile([2, 128, CT, TO], BF16, tag="k_in")
            k_out = dram.tile([128, CT, TO], BF16, tag="k_out")
            nc.sync.dma_start(
                out=k_in[:].rearrange("a p c t -> p a c t"), in_=kst[:])
            nc.gpsimd.collective_compute(
                "ReduceScatter", ALU.add, replica_groups=RG,
                ins=[k_in.opt()], outs=[k_out.opt()])
            nc.sync.dma_start(out=kRem[:], in_=k_out[:])

            # ---- v projection (tokens on PSUM partitions) ----
            wvht = gwv.tile([128, KP, 2, C], FP8, tag="vwh")
            nc.sync.dma_start(out=wvht[:], in_=wvh[l].ap()[:])
            wvlt = gwv.tile([128, KP, 2, C], FP8, tag="vwl")
            nc.sync.dma_start(out=wvlt[:], in_=wvl[l].ap()[:])
            for tt in range(TT):
                tsl = slice(tt * 128, (tt + 1) * 128)
                psA = ps_big.tile([128, 512], F32, space="PSUM", tag="px")
                psB = ps_big.tile([128, 256], F32, space="PSUM", tag="px")
                n = 3 * KP
                for ps_, csl in ((psA, slice(0, 512)),
                                 (psB, slice(512, 768))):
                    i = 0
                    for x_, w_ in ((xnh, wvht), (xnl, wvht), (xnh, wvlt)):
                        for kp in range(KP):
                            nc.tensor.matmul(ps_[:],
                                             x_[:, 2 * kp:2 * kp + 2, tsl],
                                             w_[:, kp, :, csl],
                                             start=(i == 0),
                                             stop=(i == n - 1),
                                             perf_mode=DR)
                            i += 1
                nc.vector.tensor_scalar_mul(
                    vOwn[:, tt, 0:8, 0:64],
                    psA[:].rearrange("p (h d) -> p h d", h=8),
                    sc[f"dq_v{l}"])
                nc.vector.tensor_scalar_mul(
                    vOwn[:, tt, 8:12, 0:64],
                    psB[:].rearrange("p (h d) -> p h d", h=4),
                    sc[f"dq_v{l}"])

            # ---- v export ----
            vst = gst.tile([128, 2, TT, H, 64], BF16, tag="vst")
            for sl in range(2):
                nc.vector.tensor_scalar_mul(vst[:, sl],
                                            vOwn[:, :, :, 0:64],
                                            mm_t[:, sl:sl + 1])
            v_in = dram.tile([2, 128, TT, H, 64], BF16, tag="v_in")
            v_out = dram.tile([128, TT, H, 64], BF16, tag="v_out")
            nc.sync.dma_start(
                out=v_in[:].rearrange("a p t h d -> p a t h d"), in_=vst[:])
            nc.gpsimd.collective_compute(
                "ReduceScatter", ALU.add, replica_groups=RG,
                ins=[v_in.opt()], outs=[v_out.opt()])
            nc.sync.dma_start(out=vRem[:, :, :, 0:64], in_=v_out[:])

            # ---- attention ----
            yT = pa.tile([128, CT, TO], BF16, tag="yT")
            pE = tc.alloc_tile_pool(name=f"pE{l}", bufs=10)
            prec = tc.alloc_tile_pool(name=f"prec{l}", bufs=4)
            gyo = tc.alloc_tile_pool(name=f"gyo{l}", bufs=1)
            ps_sc = tc.alloc_tile_pool(name=f"ps_sc{l}", bufs=2, space="PSUM")
            ps_av = tc.alloc_tile_pool(name=f"ps_av{l}", bufs=2, space="PSUM")
            yo = gyo.tile([128, 12, 2, AQ], F32, tag="yo")  # own partials

            qA, qB = slice(0, AQ), slice(AQ, 2 * AQ)
            # phase 1: own blocks (independent of the RS)
            for hp in range(6):
                for h in (2 * hp, 2 * hp + 1):
                    par = h % 2
                    rows = slice(64 * par, 64 * par + 64)
                    eA = score_block(hp, rows, qkT, 0, qA, masks_t)
                    eB1 = score_block(hp, rows, qkT, 0, qB, None)
                    eB2 = score_block(hp, rows, qkT, 2, qB, masks_t)
                    ya = ps_av.tile([128, AQ], F32, space="PSUM", tag="av")
                    for kt in range(2):
                        nc.tensor.matmul(ya[:], vOwn[:, kt, h, :],
                                         eA[:, kt * AQ:(kt + 1) * AQ],
                                         start=(kt == 0), stop=(kt == 1))
                    nc.vector.tensor_copy(out=yo[:, h, 0, :], in_=ya[:])
                    yb = ps_av.tile([128, AQ], F32, space="PSUM", tag="av")
                    for kt in range(4):
                        e = eB1 if kt < 2 else eB2
                        nc.tensor.matmul(yb[:], vOwn[:, kt, h, :],
                                         e[:, (kt % 2) * AQ:(kt % 2 + 1) * AQ],
                                         start=(kt == 0), stop=(kt == 3))
                    nc.vector.tensor_copy(out=yo[:, h, 1, :], in_=yb[:])
            # phase 2: remote blocks + combine
            for hp in range(6):
                for h in (2 * hp, 2 * hp + 1):
                    par = h % 2
                    rows = slice(64 * par, 64 * par + 64)
                    eAr = score_block(hp, rows, kRem, 0, qA, masks2_t[:, 0])
                    eBr1 = score_block(hp, rows, kRem, 0, qB, None)
                    eBr2 = score_block(hp, rows, kRem, 2, qB,
                                       masks2_t[:, 1])
                    for qi, es in ((0, (eAr, eAr)), (1, (eBr1, eBr2))):
                        nkt = 2 if qi == 0 else 4
                        yr = ps_av.tile([128, AQ], F32, space="PSUM",
                                        tag="av")
                        for kt in range(nkt):
                            e = es[0] if kt < 2 else es[1]
                            nc.tensor.matmul(
                                yr[:], vRem[:, kt, h, :],
                                e[:, (kt % 2) * AQ:(kt % 2 + 1) * AQ],
                                start=(kt == 0), stop=(kt == nkt - 1))
                        yc = prec.tile([128, AQ], F32, tag="yc")
                        nc.vector.tensor_tensor(out=yc[:], in0=yr[:],
                                                in1=yo[:, h, qi, :],
                                                op=ALU.add)
                        rec = prec.tile([64, AQ], F32, tag="rec")
                        nc.vector.reciprocal(rec[:], yc[64:128, :])
                        qsl = slice(qi * AQ, (qi + 1) * AQ)
                        yrow = slice(64 * par, 64 * par + 64)
                        nc.vector.tensor_tensor(out=yT[yrow, hp, qsl],
                                                in0=yc[0:64, :], in1=rec[:],
                                                op=ALU.mult)
            for p in (ps_av, ps_sc, gyo, prec, pE):
                p.release()

            # ---- y8 split, proj + residual, LN2 + MLP ----
            py8 = tc.alloc_tile_pool(name=f"py8{l}", bufs=1)
            ph8 = tc.alloc_tile_pool(name=f"ph8{l}", bufs=1)
            phbf = tc.alloc_tile_pool(name=f"phbf{l}", bufs=3)
            y8h = py8.tile([128, CT, TO], FP8, tag="y8h")
            y8l = py8.tile([128, CT, TO], FP8, tag="y8l")
            for kt in range(CT):
                nc.gpsimd.tensor_copy(out=y8h[:, kt, :], in_=yT[:, kt, :])
                nc.gpsimd.tensor_tensor(
                    out=y8l[:, kt, :], in0=yT[:, kt, :],
                    in1=y8h[:, kt, :], op=ALU.subtract)
            for ot in range(CT):
                wh = gw.tile([128, KP, 2, 128], FP8, tag="w6h")
                nc.sync.dma_start(out=wh[:],
                                  in_=wph[l].ap()[ot * 128:(ot + 1) * 128])
                wl = gw.tile([128, KP, 2, 128], FP8, tag="w6l")
                nc.sync.dma_start(out=wl[:],
                                  in_=wpl[l].ap()[ot * 128:(ot + 1) * 128])
                ps = ps_big.tile([128, TO], F32, space="PSUM", tag="px")
                dr_mm(ps[:], wh, wl, y8h, y8l, slice(0, TO), KP)
                nc.vector.scalar_tensor_tensor(
                    out=xT[:, ot, :], in0=ps[:], scalar=sc[f"dq_p{l}"],
                    in1=xT[:, ot, :], op0=ALU.mult, op1=ALU.add)
            xn2h = gxn.tile([128, CT, TO], FP8, tag="xnh")
            xn2l = gxn.tile([128, CT, TO], FP8, tag="xnl")
            st2, bc2p = ln_pools()
            ln_full(xT, xn2h, xn2l, st2, bc2p)
            h8h = ph8.tile([128, HT, TO], FP8, tag="h8h")
            h8l = ph8.tile([128, HT, TO], FP8, tag="h8l")
            for ot in range(HT):
                wh = gw.tile([128, KP, 2, 128], FP8, tag="w6h")
                nc.sync.dma_start(out=wh[:],
                                  in_=wfh[l].ap()[ot * 128:(ot + 1) * 128])
                wl = gw.tile([128, KP, 2, 128], FP8, tag="w6l")
                nc.sync.dma_start(out=wl[:],
                                  in_=wfl[l].ap()[ot * 128:(ot + 1) * 128])
                ps = ps_big.tile([128, TO], F32, space="PSUM", tag="px")
                dr_mm(ps[:], wh, wl, xn2h, xn2l, slice(0, TO), KP)
                hbf = phbf.tile([128, TO], BF16, tag="hbf")
                nc.scalar.activation(h8h[:, ot, :], ps[:], ACT.Gelu,
                                     scale=sc[f"dq_fc{l}"])
                nc.scalar.activation(hbf[:], ps[:], ACT.Gelu,
                                     scale=sc[f"dq_fc{l}"])
                nc.gpsimd.tensor_tensor(
                    out=h8l[:, ot, :], in0=hbf[:],
                    in1=h8h[:, ot, :], op=ALU.subtract)
            for ot in range(CT):
                wh = gw24.tile([128, HP, 2, 128], FP8, tag="w24h")
                nc.sync.dma_start(out=wh[:],
                                  in_=w2h[l].ap()[ot * 128:(ot + 1) * 128])
                wl = gw24.tile([128, HP, 2, 128], FP8, tag="w24l")
                nc.sync.dma_start(out=wl[:],
                                  in_=w2l[l].ap()[ot * 128:(ot + 1) * 128])
                ps = ps_big.tile([128, TO], F32, space="PSUM", tag="px")
                dr_mm(ps[:], wh, wl, h8h, h8l, slice(0, TO), HP)
                nc.vector.scalar_tensor_tensor(
                    out=xT[:, ot, :], in0=ps[:], scalar=sc[f"dq_f2{l}"],
                    in1=xT[:, ot, :], op0=ALU.mult, op1=ALU.add)
            for p in (bc2p, st2, phbf, ph8, py8, pa):
                p.release()

        # ---- final LN (fp8 h/l, scale SX) + token-split LM head ----
        for p in (gv, dram, gst, gkv, gw24, gwv, gw, gxn):
            p.release()
        pf = tc.alloc_tile_pool(name="pf", bufs=1)
        xf = pf.tile([128, CT, TO], FP8, tag="xf")
        xl = pf.tile([128, CT, TO], FP8, tag="xl")
        stf, bcf = ln_pools()
        ln_full(xT, xf, xl, stf, bcf)
        for p in (bcf, stf):
            p.release()

        ph = tc.alloc_tile_pool(name="ph", bufs=4)
        pout = tc.alloc_tile_pool(name="pout", bufs=2)
        for vc in range(NVC):
            wh8 = ph.tile([128, KP, 2, 512], FP8, tag="wh")
            nc.sync.dma_start(out=wh8[:],
                              in_=whh_d.ap()[vc * 128:(vc + 1) * 128])
            wl8 = ph.tile([128, KP, 2, 512], FP8, tag="whl")
            nc.sync.dma_start(out=wl8[:],
                              in_=whl_d.ap()[vc * 128:(vc + 1) * 128])
            o = pout.tile([128, TT, 512], BF16, tag="out")
            for tt in range(TT):
                tsl = slice(tt * 128, (tt + 1) * 128)
                ps = ps_big.tile([128, 512], F32, space="PSUM", tag="px")
                terms = [(xf, wh8), (xl, wh8), (xf, wl8)]
                for cc in range(2):
                    i = 0
                    for xsrc, wsrc in terms:
                        for kp in range(KP):
                            nc.tensor.matmul(
                                ps[:, cc * 256:(cc + 1) * 256],
                                xsrc[:, 2 * kp:2 * kp + 2, tsl],
                                wsrc[:, kp, :, cc * 256:cc * 256 + 256],
                                start=(i == 0), stop=(i == 3 * KP - 1),
                                perf_mode=DR)
                            i += 1
                if tt % 2 == 0:
                    nc.vector.tensor_scalar_mul(o[:, tt, :], ps[:],
                                                sc["dq_h"])
                else:
                    nc.scalar.mul(o[:, tt, :], ps[:], sc["dq_h"])
            nc.sync.dma_start(
                out=logits_d.ap()[:, vc * 512:(vc + 1) * 512].rearrange(
                    "(t p) v -> p t v", p=128),
                in_=o[:])
        for p in (pout, ph, pf, ps_big, gmicro, gu, gx2, gx, glob):
            p.release()

    nc.compile()
    return nc

# ---------------------------------------------------------------------------
# host side
# ---------------------------------------------------------------------------

def _pow2_scale(m, target=224.0):
    if m == 0:
        return 1.0
    return float(2.0 ** np.floor(np.log2(target / m)))


def _hi_lo(w, s):
    ws = w * s
    hi = ws.astype(E4)
    lo = (ws - hi.astype(np.float32)).astype(E4)
    return hi, lo


# own global AQ-blocks per half, in local order
OWN_BLOCKS = {0: (0, 3), 1: (1, 2)}


def _prep_inputs(inputs, n_layers):
    f32 = np.float32
    idx = np.asarray(inputs["idx"])
    wte = np.asarray(inputs["wte"], f32)
    wpe = np.asarray(inputs["wpe"], f32)

    sc = {}
    common = {}
    for l in range(n_layers):
        ln1w = np.asarray(inputs["ln1_w"][l], f32)
        ln1b = np.asarray(inputs["ln1_b"][l], f32)
        aw = np.asarray(inputs["attn_w"][l], f32)
        ab = np.asarray(inputs["attn_b"][l], f32)
        awf = ln1w[:, None] * aw
        abf = ab + ln1b @ aw
        assert not np.any(abf), "nonzero attn bias not supported"
        qk = awf[:, :1536]
        s_qk = _pow2_scale(float(np.abs(qk).max()))
        qh, ql = _hi_lo(qk, s_qk)

        def qk_layout(a):
            return np.ascontiguousarray(
                a.reshape(KP, 2, 128, 12, 128).transpose(3, 2, 0, 1, 4)
            ).reshape(12 * 128, KP, 2, 128)
        common[f"qkwh{l}"] = qk_layout(qh)
        common[f"qkwl{l}"] = qk_layout(ql)
        sc[f"dq_qk{l}"] = 1.0 / (SX * s_qk)
        vw = awf[:, 1536:]
        s_v = _pow2_scale(float(np.abs(vw).max()))
        vh, vl = _hi_lo(vw, s_v)

        def v_layout(a):
            return np.ascontiguousarray(
                a.reshape(KP, 2, 128, C).transpose(2, 0, 1, 3))
        common[f"vwh{l}"] = v_layout(vh)
        common[f"vwl{l}"] = v_layout(vl)
        sc[f"dq_v{l}"] = 1.0 / (SX * s_v)

        pw = np.asarray(inputs["proj_w"][l], f32)
        assert not np.any(np.asarray(inputs["proj_b"][l])), "proj bias"
        s_p = _pow2_scale(float(np.abs(pw).max()))
        ph_, pl_ = _hi_lo(pw, s_p)

        def p_layout(a):
            return np.ascontiguousarray(
                a.reshape(KP, 2, 128, CT, 128).transpose(3, 2, 0, 1, 4)
            ).reshape(CT * 128, KP, 2, 128)
        common[f"pwh{l}"] = p_layout(ph_)
        common[f"pwl{l}"] = p_layout(pl_)
        sc[f"dq_p{l}"] = 1.0 / (SY * s_p)

        ln2w = np.asarray(inputs["ln2_w"][l], f32)
        ln2b = np.asarray(inputs["ln2_b"][l], f32)
        fw = np.asarray(inputs["fc_w"][l], f32)
        fbv = np.asarray(inputs["fc_b"][l], f32)
        fwf = ln2w[:, None] * fw
        fbf = fbv + ln2b @ fw
        assert not np.any(fbf), "nonzero fc bias not supported"
        s_fc = _pow2_scale(float(np.abs(fwf).max()))
        fh, fl = _hi_lo(fwf, s_fc)

        def f_layout(a):
            return np.ascontiguousarray(
                a.reshape(KP, 2, 128, HT, 128).transpose(3, 2, 0, 1, 4)
            ).reshape(HT * 128, KP, 2, 128)
        common[f"fwh{l}"] = f_layout(fh)
        common[f"fwl{l}"] = f_layout(fl)
        sc[f"dq_fc{l}"] = 1.0 / (SX * s_fc)

        f2w = np.asarray(inputs["fc2_w"][l], f32)
        assert not np.any(np.asarray(inputs["fc2_b"][l])), "fc2 bias"
        s_f2 = _pow2_scale(float(np.abs(f2w).max()))
        f2h, f2l = _hi_lo(f2w, s_f2)

        def f2_layout(a):
            return np.ascontiguousarray(
                a.reshape(HP, 2, 128, CT, 128).transpose(3, 2, 0, 1, 4)
            ).reshape(CT * 128, HP, 2, 128)
        common[f"f2wh{l}"] = f2_layout(f2h)
        common[f"f2wl{l}"] = f2_layout(f2l)
        sc[f"dq_f2{l}"] = 1.0 / (SH * s_f2)

    p = np.arange(128)[:, None]
    f = np.arange(AQ)[None, :]
    masks = np.zeros((128, 2, AQ), f32)
    masks[:, 0, :] = np.where(p > f, -240.0, 0.0)
    masks[:, 1, :] = np.where(p + 128 > f, -240.0, 0.0)
    common["masks"] = masks.astype(E4)
    iden = np.zeros((128, 2, 128), f32)
    iden[:, 0, :] = 128.0 * np.eye(128)
    common["iden"] = iden.astype(E4)

    lnfw = np.asarray(inputs["lnf_w"], f32)
    lnfb = np.asarray(inputs["lnf_b"], f32)
    assert not np.any(lnfb @ wte.T), "nonzero head bias not supported"
    wh = lnfw[:, None] * wte.T                     # [768, V]
    whp = np.zeros((C, VS2), f32)
    whp[:, :V] = wh
    m = float(np.abs(wh).max())
    s_h = float(2.0 ** np.floor(np.log2(240.0 / m)))
    sc["dq_h"] = 1.0 / (s_h * SX)

    slh = whp * s_h
    hih = slh.astype(E4)
    loh = (slh - hih.astype(f32)).astype(E4)
    common["whh"] = np.ascontiguousarray(
        hih.reshape(KP, 2, 128, NVC, 512).transpose(3, 2, 0, 1, 4)
    ).reshape(NVC * 128, KP, 2, 512)
    common["whl"] = np.ascontiguousarray(
        loh.reshape(KP, 2, 128, NVC, 512).transpose(3, 2, 0, 1, 4)
    ).reshape(NVC * 128, KP, 2, 512)

    def t6(a):          # [768, TO] -> [128, 6, TO]
        return np.ascontiguousarray(
            a.reshape(CT, 128, a.shape[1]).transpose(1, 0, 2))

    x0 = wte[idx] + wpe[None, :T]                  # [B, T, C]
    in_maps = []
    for c in range(8):
        s, half = c & 3, c >> 2
        bA, bB = OWN_BLOCKS[half]
        m2 = dict(common)
        xo = np.concatenate([x0[s, bA * AQ:(bA + 1) * AQ],
                             x0[s, bB * AQ:(bB + 1) * AQ]], axis=0)
        m2["x0t"] = t6(np.ascontiguousarray(xo.T))
        # RS staging masks: slot s2 carries my data iff my rank != s2
        mm = np.zeros((128, 2), f32)
        mm[:, 1 - half] = 1.0
        m2["mm"] = mm
        # data masks: mA for qA x remA ; mB for qB x remB
        m2d = np.zeros((128, 2, 2, AQ), f32)
        if half == 0:
            m2d[:, 0, :, :] = -240.0        # qA(blk0) x rA(blk1): future
            # qB(blk3) x rB(blk2): full attend -> 0
        else:
            # qA(blk1) x rA(blk0): full attend -> 0
            m2d[:, 1, :, :] = -240.0        # qB(blk2) x rB(blk3): future
        m2["masks2"] = m2d.astype(E4)
        in_maps.append(m2)
    return in_maps, sc


def kernel(**inputs):
    global LAST_RESULT, LAST_NC
    n_layers = L
    in_maps, sc = _prep_inputs(inputs, n_layers)
    key = (n_layers, tuple(sorted(sc.items())))
    if key not in _CACHE:
        _CACHE[key] = build_program(sc, n_layers)
    nc = _CACHE[key]
    LAST_NC = nc
    res = run_bass_kernel_spmd(nc, in_maps, core_ids=list(range(8)))
    LAST_RESULT = res
    out = np.empty((B, T, V), np.float32)
    for c in range(8):
        s, half = c & 3, c >> 2
        bA, bB = OWN_BLOCKS[half]
        part = np.asarray(res.results[c]["logits"]).astype(np.float32)
        out[s, bA * AQ:(bA + 1) * AQ] = part[0:AQ, :V]
        out[s, bB * AQ:(bB + 1) * AQ] = part[AQ:2 * AQ, :V]
    return out


if __name__ == "__main__":
    rng = np.random.default_rng(0)
    ins = {
        "idx": rng.integers(0, V, (B, T)).astype(np.int32),
        "wte": (rng.standard_normal((V, C)) * 0.02).astype(np.float32),
        "wpe": (rng.standard_normal((T, C)) * 0.02).astype(np.float32),
        "ln1_w": np.ones((L, C), np.float32),
        "ln1_b": np.zeros((L, C), np.float32),
        "attn_w": (rng.standard_normal((L, C, 3 * C)) * 0.02).astype(np.float32),
        "attn_b": np.zeros((L, 3 * C), np.float32),
        "proj_w": (rng.standard_normal((L, C, C)) * 0.02).astype(np.float32),
        "proj_b": np.zeros((L, C), np.float32),
        "ln2_w": np.ones((L, C), np.float32),
        "ln2_b": np.zeros((L, C), np.float32),
        "fc_w": (rng.standard_normal((L, C, 4 * C)) * 0.02).astype(np.float32),
        "fc_b": np.zeros((L, 4 * C), np.float32),
        "fc2_w": (rng.standard_normal((L, 4 * C, C)) * 0.02).astype(np.float32),
        "fc2_b": np.zeros((L, C), np.float32),
        "lnf_w": np.ones((C,), np.float32),
        "lnf_b": np.zeros((C,), np.float32),
    }
    out = kernel(**ins)
    print("out", out.shape, out.dtype, float(np.abs(out).max()))


# revision 18
# speedup vs baseline: 1.2275x; 1.0082x over previous
"""MiniGPT forward on 8 Trainium2 NeuronCores — sequence-split variant.

Core c: sequence (c & 3), token-half (c >> 2).  In AQ=256 blocks of the
1024-token sequence, half 0 owns blocks {0, 3}, half 1 owns {1, 2} (equal
causal-attention load).  Each core runs the 6 transformer blocks for its
OWN 512 tokens only — no duplicated block compute.  Per layer the pair
exchanges K and V via two ReduceScatters (k first): each core stages
[k*m0, k*m1] with per-core {0,1} masks so the RS output is exactly the
PEER's k/v at a uniform address (SPMD-safe, no rank branching).
Attention = own-block phase (overlaps the RS) + remote phase; causal /
validity masks are per-core input data applied on the PE via the
iden @ mask DoubleRow trick.  One remote block per core is fully masked
waste, keeping the instruction stream identical across cores.

Block linears are fp8(e4m3) DoubleRow 3-term (Wh@xh + Wh@xl + Wl@xh,
0.75x bf16 PE cost); attention stays bf16.  The LM head is token-split:
each core computes its own 512 tokens x the FULL vocab with the same
3-term fp8 scheme, logits stream out in bf16.
"""

import sys

sys.path.insert(0, "/opt/trn_rl_repo")

import numpy as np
import ml_dtypes

import concourse.bacc as bacc
import concourse.tile as tile
from concourse import mybir
from concourse.bass_utils import run_bass_kernel_spmd

F32 = mybir.dt.float32
F32R = mybir.dt.float32r
BF16 = mybir.dt.bfloat16
FP8 = mybir.dt.float8e4
ALU = mybir.AluOpType
ACT = mybir.ActivationFunctionType
DR = mybir.MatmulPerfMode.DoubleRow
E4 = ml_dtypes.float8_e4m3

B, T, C, H, HD, L, V = 4, 1024, 768, 12, 64, 6, 50257
TO = 512                # own tokens per core
CT = C // 128           # 6 c-tiles
KP = CT // 2            # 3 k-pairs
TT = TO // 128          # 4 own token tiles
AQ = 256                # attention query block
HT = 3072 // 128        # 24 hidden tiles
HP = HT // 2            # 12 hidden k-pairs
VS2 = 51200             # padded vocab
NVC = VS2 // 512        # 100
EPS = 1e-5
SX = 16.0               # fp8 scale for LN outputs (blocks + head)
SY = 32.0               # fp8 scale for attention output y (folded in ones)
SH = 1.0                # gelu output used unscaled in fp8

RG = [[0, 4], [1, 5], [2, 6], [3, 7]]   # pair replica groups

_CACHE = {}
LAST_RESULT = None
LAST_NC = None


def build_program(sc, n_layers=L):
    nc = bacc.Bacc(None, target_bir_lowering=False)

    def f8_in(name, shape):
        return nc.dram_tensor(name, list(shape), FP8, kind="ExternalInput")

    x0t_d = nc.dram_tensor("x0t", [128, CT, TO], F32R, kind="ExternalInput")
    wqh, wql, wvh, wvl, wph, wpl = [], [], [], [], [], []
    wfh, wfl, w2h, w2l = [], [], [], []
    for l in range(n_layers):
        wqh.append(f8_in(f"qkwh{l}", (12 * 128, KP, 2, 128)))
        wql.append(f8_in(f"qkwl{l}", (12 * 128, KP, 2, 128)))
        wvh.append(f8_in(f"vwh{l}", (128, KP, 2, C)))
        wvl.append(f8_in(f"vwl{l}", (128, KP, 2, C)))
        wph.append(f8_in(f"pwh{l}", (CT * 128, KP, 2, 128)))
        wpl.append(f8_in(f"pwl{l}", (CT * 128, KP, 2, 128)))
        wfh.append(f8_in(f"fwh{l}", (HT * 128, KP, 2, 128)))
        wfl.append(f8_in(f"fwl{l}", (HT * 128, KP, 2, 128)))
        w2h.append(f8_in(f"f2wh{l}", (CT * 128, HP, 2, 128)))
        w2l.append(f8_in(f"f2wl{l}", (CT * 128, HP, 2, 128)))
    whh_d = f8_in("whh", (NVC * 128, KP, 2, 512))
    whl_d = f8_in("whl", (NVC * 128, KP, 2, 512))
    masks_d = f8_in("masks", (128, 2, AQ))          # local diag (shared)
    masks2_d = f8_in("masks2", (128, 2, 2, AQ))     # [mA|mB, half, q] per-core
    iden_d = f8_in("iden", (128, 2, 128))
    mm_d = nc.dram_tensor("mm", [128, 2], F32, kind="ExternalInput")
    logits_d = nc.dram_tensor("logits", [TO, VS2], BF16,
                              kind="ExternalOutput")

    ln_ctr = [0]

    with nc.allow_low_precision("fp8 3-term error-feedback intentional"), \
         tile.TileContext(nc) as tc:
        glob = tc.alloc_tile_pool(name="glob", bufs=1)
        gx = tc.alloc_tile_pool(name="gx", bufs=1)
        gx2 = tc.alloc_tile_pool(name="gx2", bufs=2)
        gu = tc.alloc_tile_pool(name="gu", bufs=2)
        gmicro = tc.alloc_tile_pool(name="gmicro", bufs=1)
        ps_big = tc.alloc_tile_pool(name="ps_big", bufs=4, space="PSUM")
        gxn = tc.alloc_tile_pool(name="gxn", bufs=1)
        gw = tc.alloc_tile_pool(name="gw", bufs=8)
        gwv = tc.alloc_tile_pool(name="gwv", bufs=1)
        gw24 = tc.alloc_tile_pool(name="gw24", bufs=3)
        gkv = tc.alloc_tile_pool(name="gkv", bufs=1)
        gst = tc.alloc_tile_pool(name="gst", bufs=1)
        dram = tc.alloc_tile_pool(name="dram", bufs=2, space="DRAM")

        ones_col = glob.tile([128, 1], F32R, tag="ones_col")
        ones_row = glob.tile([1, 128], F32R, tag="ones_row")
        epsh_t = glob.tile([1, 1], F32, tag="epsh")
        masks_t = glob.tile([128, 2, AQ], FP8, tag="masks")
        masks2_t = glob.tile([128, 2, 2, AQ], FP8, tag="masks2")
        iden_t = glob.tile([128, 2, 128], FP8, tag="iden")
        mm_t = glob.tile([128, 2], F32, tag="mm")
        nc.vector.memset(ones_col[:].bitcast(F32), 1.0)
        nc.vector.memset(ones_row[:].bitcast(F32), 1.0)
        nc.vector.memset(epsh_t[:], EPS / (SX * SX))
        nc.sync.dma_start(out=masks_t[:], in_=masks_d[:])
        nc.sync.dma_start(out=masks2_t[:], in_=masks2_d[:])
        nc.sync.dma_start(out=iden_t[:], in_=iden_d[:])
        nc.sync.dma_start(out=mm_t[:], in_=mm_d[:])

        xT = gx.tile([128, CT, TO], F32R, tag="xT")
        nc.sync.dma_start(out=xT[:], in_=x0t_d[:])

        # persistent v tiles: [p, tt, h, 0:64] = v ; [.., 64:128] = 1/SY
        gv = tc.alloc_tile_pool(name="gv", bufs=1)
        vOwn = gv.tile([128, TT, H, 128], BF16, tag="vOwn")
        vRem = gv.tile([128, TT, H, 128], BF16, tag="vRem")
        kRem = gkv.tile([128, CT, TO], BF16, tag="kRem")
        nc.gpsimd.memset(vOwn[:, :, :, 64:128], 1.0 / SY)
        nc.gpsimd.memset(vRem[:, :, :, 64:128], 1.0 / SY)

        def ln_full(xin, xh, xl, ps_stat, ps_bc):
            """(xh + xl) ~= SX * (xin - mu) * rstd in fp8, all 512 tokens."""
            qs = slice(0, TO)
            s_ps = ps_stat.tile([1, TO], F32, space="PSUM", tag="stat")
            q_ps = ps_stat.tile([1, TO], F32, space="PSUM", tag="stat")
            for kt in range(CT):
                nc.tensor.matmul(s_ps[:], ones_col[:], xin[:, kt, qs],
                                 start=(kt == 0), stop=(kt == CT - 1))
            for kt in range(CT):
                x2 = gx2.tile([128, TO], F32R, tag="x2")
                nc.gpsimd.tensor_tensor(
                    out=x2[:], in0=xin[:, kt, qs],
                    in1=xin[:, kt, qs], op=ALU.mult)
                nc.tensor.matmul(q_ps[:], ones_col[:], x2[:],
                                 start=(kt == 0), stop=(kt == CT - 1))
            mu = gmicro.tile([1, TO], F32R, tag="mu")
            nc.scalar.mul(mu[:], s_ps[:], 1.0 / C)
            mu2 = gmicro.tile([1, TO], F32, tag="mu2")
            nc.scalar.activation(mu2[:], mu[:], ACT.Square)
            var = gmicro.tile([1, TO], F32, tag="var")
            nc.vector.scalar_tensor_tensor(
                out=var[:], in0=q_ps[:], scalar=1.0 / C, in1=mu2[:],
                op0=ALU.mult, op1=ALU.subtract)
            sd = gmicro.tile([1, TO], F32, tag="sd")
            nc.scalar.activation(sd[:], var[:], ACT.Sqrt, bias=epsh_t[:],
                                 scale=1.0 / (SX * SX))
            r = gmicro.tile([1, TO], F32R, tag="r")
            nc.vector.reciprocal(r[:], sd[:])            # SX/sd
            mr = gmicro.tile([1, TO], F32R, tag="mr")
            nc.vector.tensor_tensor(out=mr[:], in0=mu[:], in1=r[:],
                                    op=ALU.mult)
            bc = ps_bc.tile([128, TO], F32, space="PSUM", tag="bc")
            nc.tensor.matmul(bc[:], ones_row[:], r[:], start=True, stop=True)
            bc2 = ps_bc.tile([128, TO], F32, space="PSUM", tag="bc")
            nc.tensor.matmul(bc2[:], ones_row[:], mr[:], start=True,
                             stop=True)
            for kt in range(CT):
                t = gx2.tile([128, TO], F32, tag="lnt")
                nc.vector.tensor_tensor(out=t[:],
                                        in0=xin[:, kt, qs].bitcast(F32),
                                        in1=bc[:], op=ALU.mult)
                u = gu.tile([128, TO], F32, tag="lnu")
                nc.vector.tensor_tensor(out=u[:], in0=t[:],
                                        in1=bc2[:], op=ALU.subtract)
                nc.scalar.copy(xh[:, kt, :], u[:])
                nc.gpsimd.tensor_tensor(out=xl[:, kt, :], in0=u[:],
                                        in1=xh[:, kt, :],
                                        op=ALU.subtract)

        def ln_pools():
            i = ln_ctr[0]
            ln_ctr[0] += 1
            ps_stat = tc.alloc_tile_pool(name=f"ps_st{i}", bufs=2,
                                         space="PSUM")
            ps_bc = tc.alloc_tile_pool(name=f"ps_bc{i}", bufs=2, space="PSUM")
            return ps_stat, ps_bc

        def dr_mm(ps, wh, wl, xh, xl, qs, kps):
            """accumulate 3-term fp8 DR: Wh@xh + Wh@xl + Wl@xh into ps."""
            n = 3 * kps
            i = 0
            for w_, x_ in ((wh, xh), (wh, xl), (wl, xh)):
                for kp in range(kps):
                    nc.tensor.matmul(ps, w_[:, kp, :, :],
                                     x_[:, 2 * kp:2 * kp + 2, qs],
                                     start=(i == 0), stop=(i == n - 1),
                                     perf_mode=DR)
                    i += 1

        def mask_mm(sps, half, msrc):
            nc.tensor.matmul(
                sps[:, half * AQ:(half + 1) * AQ],
                iden_t[:],
                msrc.rearrange("p (i q) -> p i q", i=1)
                .broadcast_to([128, 2, AQ]),
                start=False, stop=True, perf_mode=DR,
                skip_group_check=True)

        def score_block(hp, rows, ksrc, kbase, qsl, msrc):
            """one AQ x AQ*?? score block: k tiles (kbase, kbase+1) of ksrc
            vs q columns qsl; optional additive mask (None | AP)."""
            sps = ps_sc.tile([128, 2 * AQ], F32, space="PSUM", tag="sc",
                             name="sps")
            for half in range(2):
                kt = kbase + half
                nc.tensor.matmul(
                    sps[:, half * AQ:(half + 1) * AQ],
                    ksrc[rows, 6 + hp, kt * 128:(kt + 1) * 128]
                    if ksrc is qkT else
                    ksrc[rows, hp, kt * 128:(kt + 1) * 128],
                    qkT[rows, hp, qsl],
                    start=True, stop=(msrc is None),
                    skip_group_check=(msrc is not None))
                if msrc is not None:
                    mask_mm(sps, half, msrc[:, half, :])
            e = pE.tile([128, 2 * AQ], BF16, tag="E", name="e")
            nc.scalar.activation(e[:], sps[:], ACT.Exp, scale=0.125)
            return e

        for l in range(n_layers):
            pa = tc.alloc_tile_pool(name=f"pa{l}", bufs=1)

            # ---- LN1 -> xn8 h/l (fp8) ----
            xnh = gxn.tile([128, CT, TO], FP8, tag="xnh")
            xnl = gxn.tile([128, CT, TO], FP8, tag="xnl")
            qkT = pa.tile([128, 12, TO], BF16, tag="qkT")
            st1, bc1 = ln_pools()
            ln_full(xT, xnh, xnl, st1, bc1)

            # ---- k projections first so the k-RS starts early ----
            def qk_slots(slots):
                for s_ in slots:
                    wh = gw.tile([128, KP, 2, 128], FP8, tag="w6h",
                                 name="wh")
                    nc.sync.dma_start(
                        out=wh[:], in_=wqh[l].ap()[s_ * 128:(s_ + 1) * 128])
                    wl = gw.tile([128, KP, 2, 128], FP8, tag="w6l",
                                 name="wl")
                    nc.sync.dma_start(
                        out=wl[:], in_=wql[l].ap()[s_ * 128:(s_ + 1) * 128])
                    ps = ps_big.tile([128, TO], F32, space="PSUM", tag="px",
                                     name="ps")
                    dr_mm(ps[:], wh, wl, xnh, xnl, slice(0, TO), KP)
                    nc.vector.tensor_scalar_mul(qkT[:, s_, :], ps[:],
                                                sc[f"dq_qk{l}"])

            qk_slots(range(6, 12))
            qk_slots(range(6))
            for p in (bc1, st1):
                p.release()

            # ---- k export: stage [k*m0, k*m1], RS, import peer k ----
            kst = gst.tile([128, 2, CT, TO], BF16, tag="kst")
            for sl in range(2):
                nc.vector.tensor_scalar_mul(kst[:, sl], qkT[:, 6:12, :],
                                            mm_t[:, sl:sl + 1])
            k_in = dram.tile([2, 128, CT, TO], BF16, tag="k_in")
            k_out = dram.tile([128, CT, TO], BF16, tag="k_out")
            nc.sync.dma_start(
                out=k_in[:].rearrange("a p c t -> p a c t"), in_=kst[:])
            nc.gpsimd.collective_compute(
                "ReduceScatter", ALU.add, replica_groups=RG,
                ins=[k_in.opt()], outs=[k_out.opt()])
            nc.sync.dma_start(out=kRem[:], in_=k_out[:])

            # ---- v projection (tokens on PSUM partitions) ----
            wvht = gwv.tile([128, KP, 2, C], FP8, tag="vwh")
            nc.sync.dma_start(out=wvht[:], in_=wvh[l].ap()[:])
            wvlt = gwv.tile([128, KP, 2, C], FP8, tag="vwl")
            nc.sync.dma_start(out=wvlt[:], in_=wvl[l].ap()[:])
            for tt in range(TT):
                tsl = slice(tt * 128, (tt + 1) * 128)
                psA = ps_big.tile([128, 512], F32, space="PSUM", tag="px")
                psB = ps_big.tile([128, 256], F32, space="PSUM", tag="px")
                n = 3 * KP
                for ps_, csl in ((psA, slice(0, 512)),
                                 (psB, slice(512, 768))):
                    i = 0
                    for x_, w_ in ((xnh, wvht), (xnl, wvht), (xnh, wvlt)):
                        for kp in range(KP):
                            nc.tensor.matmul(ps_[:],
                                             x_[:, 2 * kp:2 * kp + 2, tsl],
                                             w_[:, kp, :, csl],
                                             start=(i == 0),
                                             stop=(i == n - 1),
                                             perf_mode=DR)
                            i += 1
                nc.vector.tensor_scalar_mul(
                    vOwn[:, tt, 0:8, 0:64],
                    psA[:].rearrange("p (h d) -> p h d", h=8),
                    sc[f"dq_v{l}"])
                nc.vector.tensor_scalar_mul(
                    vOwn[:, tt, 8:12, 0:64],
                    psB[:].rearrange("p (h d) -> p h d", h=4),
                    sc[f"dq_v{l}"])

            # ---- v export ----
            vst = gst.tile([128, 2, TT, H, 64], BF16, tag="vst")
            for sl in range(2):
                nc.vector.tensor_scalar_mul(vst[:, sl],
                                            vOwn[:, :, :, 0:64],
                                            mm_t[:, sl:sl + 1])
            v_in = dram.tile([2, 128, TT, H, 64], BF16, tag="v_in")
            v_out = dram.tile([128, TT, H, 64], BF16, tag="v_out")
            nc.sync.dma_start(
                out=v_in[:].rearrange("a p t h d -> p a t h d"), in_=vst[:])
            nc.gpsimd.collective_compute(
                "ReduceScatter", ALU.add, replica_groups=RG,
                ins=[v_in.opt()], outs=[v_out.opt()])
            nc.sync.dma_start(out=vRem[:, :, :, 0:64], in_=v_out[:])

            # ---- attention ----
            yT = pa.tile([128, CT, TO], BF16, tag="yT")
            pE = tc.alloc_tile_pool(name=f"pE{l}", bufs=10)
            prec = tc.alloc_tile_pool(name=f"prec{l}", bufs=4)
            gyo = tc.alloc_tile_pool(name=f"gyo{l}", bufs=1)
            ps_sc = tc.alloc_tile_pool(name=f"ps_sc{l}", bufs=2, space="PSUM")
            ps_av = tc.alloc_tile_pool(name=f"ps_av{l}", bufs=2, space="PSUM")
            yo = gyo.tile([128, 12, 2, AQ], F32, tag="yo")  # own partials

            qA, qB = slice(0, AQ), slice(AQ, 2 * AQ)
            # phase 1: own blocks (independent of the RS)
            for hp in range(6):
                for h in (2 * hp, 2 * hp + 1):
                    par = h % 2
                    rows = slice(64 * par, 64 * par + 64)
                    eA = score_block(hp, rows, qkT, 0, qA, masks_t)
                    eB1 = score_block(hp, rows, qkT, 0, qB, None)
                    eB2 = score_block(hp, rows, qkT, 2, qB, masks_t)
                    ya = ps_av.tile([128, AQ], F32, space="PSUM", tag="av")
                    for kt in range(2):
                        nc.tensor.matmul(ya[:], vOwn[:, kt, h, :],
                                         eA[:, kt * AQ:(kt + 1) * AQ],
                                         start=(kt == 0), stop=(kt == 1))
                    nc.vector.tensor_copy(out=yo[:, h, 0, :], in_=ya[:])
                    yb = ps_av.tile([128, AQ], F32, space="PSUM", tag="av")
                    for kt in range(4):
                        e = eB1 if kt < 2 else eB2
                        nc.tensor.matmul(yb[:], vOwn[:, kt, h, :],
                                         e[:, (kt % 2) * AQ:(kt % 2 + 1) * AQ],
                                         start=(kt == 0), stop=(kt == 3))
                    nc.vector.tensor_copy(out=yo[:, h, 1, :], in_=yb[:])
            # phase 2: remote blocks + combine
            for hp in range(6):
                for h in (2 * hp, 2 * hp + 1):
                    par = h % 2
                    rows = slice(64 * par, 64 * par + 64)
                    eAr = score_block(hp, rows, kRem, 0, qA, masks2_t[:, 0])
                    eBr1 = score_block(hp, rows, kRem, 0, qB, None)
                    eBr2 = score_block(hp, rows, kRem, 2, qB,
                                       masks2_t[:, 1])
                    for qi, es in ((0, (eAr, eAr)), (1, (eBr1, eBr2))):
                        nkt = 2 if qi == 0 else 4
                        yr = ps_av.tile([128, AQ], F32, space="PSUM",
                                        tag="av")
                        for kt in range(nkt):
                            e = es[0] if kt < 2 else es[1]
                            nc.tensor.matmul(
                                yr[:], vRem[:, kt, h, :],
                                e[:, (kt % 2) * AQ:(kt % 2 + 1) * AQ],
                                start=(kt == 0), stop=(kt == nkt - 1))
                        yc = prec.tile([128, AQ], F32, tag="yc")
                        nc.vector.tensor_tensor(out=yc[:], in0=yr[:],
                                                in1=yo[:, h, qi, :],
                                                op=ALU.add)
                        rec = prec.tile([64, AQ], F32, tag="rec")
                        nc.vector.reciprocal(rec[:], yc[64:128, :])
                        qsl = slice(qi * AQ, (qi + 1) * AQ)
                        yrow = slice(64 * par, 64 * par + 64)
                        nc.vector.tensor_tensor(out=yT[yrow, hp, qsl],
                                                in0=yc[0:64, :], in1=rec[:],
                                                op=ALU.mult)
            for p in (ps_av, ps_sc, gyo, prec, pE):
                p.release()

            # ---- y8 split, proj + residual, LN2 + MLP ----
            py8 = tc.alloc_tile_pool(name=f"py8{l}", bufs=1)
            ph8 = tc.alloc_tile_pool(name=f"ph8{l}", bufs=1)
            phbf = tc.alloc_tile_pool(name=f"phbf{l}", bufs=3)
            y8h = py8.tile([128, CT, TO], FP8, tag="y8h")
            y8l = py8.tile([128, CT, TO], FP8, tag="y8l")
            for kt in range(CT):
                nc.gpsimd.tensor_copy(out=y8h[:, kt, :], in_=yT[:, kt, :])
                nc.gpsimd.tensor_tensor(
                    out=y8l[:, kt, :], in0=yT[:, kt, :],
                    in1=y8h[:, kt, :], op=ALU.subtract)
            for ot in range(CT):
                wh = gw.tile([128, KP, 2, 128], FP8, tag="w6h")
                nc.sync.dma_start(out=wh[:],
                                  in_=wph[l].ap()[ot * 128:(ot + 1) * 128])
                wl = gw.tile([128, KP, 2, 128], FP8, tag="w6l")
                nc.sync.dma_start(out=wl[:],
                                  in_=wpl[l].ap()[ot * 128:(ot + 1) * 128])
                ps = ps_big.tile([128, TO], F32, space="PSUM", tag="px")
                dr_mm(ps[:], wh, wl, y8h, y8l, slice(0, TO), KP)
                nc.vector.scalar_tensor_tensor(
                    out=xT[:, ot, :], in0=ps[:], scalar=sc[f"dq_p{l}"],
                    in1=xT[:, ot, :], op0=ALU.mult, op1=ALU.add)
            xn2h = gxn.tile([128, CT, TO], FP8, tag="xnh")
            xn2l = gxn.tile([128, CT, TO], FP8, tag="xnl")
            st2, bc2p = ln_pools()
            ln_full(xT, xn2h, xn2l, st2, bc2p)
            h8h = ph8.tile([128, HT, TO], FP8, tag="h8h")
            h8l = ph8.tile([128, HT, TO], FP8, tag="h8l")
            for ot in range(HT):
                wh = gw.tile([128, KP, 2, 128], FP8, tag="w6h")
                nc.sync.dma_start(out=wh[:],
                                  in_=wfh[l].ap()[ot * 128:(ot + 1) * 128])
                wl = gw.tile([128, KP, 2, 128], FP8, tag="w6l")
                nc.sync.dma_start(out=wl[:],
                                  in_=wfl[l].ap()[ot * 128:(ot + 1) * 128])
                ps = ps_big.tile([128, TO], F32, space="PSUM", tag="px")
                dr_mm(ps[:], wh, wl, xn2h, xn2l, slice(0, TO), KP)
                hbf = phbf.tile([128, TO], BF16, tag="hbf")
                nc.scalar.activation(h8h[:, ot, :], ps[:], ACT.Gelu,
                                     scale=sc[f"dq_fc{l}"])
                nc.scalar.activation(hbf[:], ps[:], ACT.Gelu,
                                     scale=sc[f"dq_fc{l}"])
                nc.gpsimd.tensor_tensor(
                    out=h8l[:, ot, :], in0=hbf[:],
                    in1=h8h[:, ot, :], op=ALU.subtract)
            for ot in range(CT):
                wh = gw24.tile([128, HP, 2, 128], FP8, tag="w24h")
                nc.sync.dma_start(out=wh[:],
                                  in_=w2h[l].ap()[ot * 128:(ot + 1) * 128])
                wl = gw24.tile([128, HP, 2, 128], FP8, tag="w24l")
                nc.sync.dma_start(out=wl[:],
                                  in_=w2l[l].ap()[ot * 128:(ot + 1) * 128])
                ps = ps_big.tile([128, TO], F32, space="PSUM", tag="px")
                dr_mm(ps[:], wh, wl, h8h, h8l, slice(0, TO), HP)
                nc.vector.scalar_tensor_tensor(
                    out=xT[:, ot, :], in0=ps[:], scalar=sc[f"dq_f2{l}"],
                    in1=xT[:, ot, :], op0=ALU.mult, op1=ALU.add)
            for p in (bc2p, st2, phbf, ph8, py8, pa):
                p.release()

        # ---- final LN (fp8 h/l, scale SX) + token-split LM head ----
        for p in (gv, dram, gst, gkv, gw24, gwv, gw, gxn):
            p.release()
        pf = tc.alloc_tile_pool(name="pf", bufs=1)
        xf = pf.tile([128, CT, TO], FP8, tag="xf")
        xl = pf.tile([128, CT, TO], FP8, tag="xl")
        stf, bcf = ln_pools()
        ln_full(xT, xf, xl, stf, bcf)
        for p in (bcf, stf):
            p.release()

        ph = tc.alloc_tile_pool(name="ph", bufs=8)
        pout = tc.alloc_tile_pool(name="pout", bufs=4)
        for vc in range(NVC):
            wh8 = ph.tile([128, KP, 2, 512], FP8, tag="wh")
            nc.sync.dma_start(out=wh8[:],
                              in_=whh_d.ap()[vc * 128:(vc + 1) * 128])
            wl8 = ph.tile([128, KP, 2, 512], FP8, tag="whl")
            nc.sync.dma_start(out=wl8[:],
                              in_=whl_d.ap()[vc * 128:(vc + 1) * 128])
            o = pout.tile([128, TT, 512], BF16, tag="out")
            for tt in range(TT):
                tsl = slice(tt * 128, (tt + 1) * 128)
                ps = ps_big.tile([128, 512], F32, space="PSUM", tag="px")
                terms = [(xf, wh8), (xl, wh8), (xf, wl8)]
                for cc in range(2):
                    i = 0
                    for xsrc, wsrc in terms:
                        for kp in range(KP):
                            nc.tensor.matmul(
                                ps[:, cc * 256:(cc + 1) * 256],
                                xsrc[:, 2 * kp:2 * kp + 2, tsl],
                                wsrc[:, kp, :, cc * 256:cc * 256 + 256],
                                start=(i == 0), stop=(i == 3 * KP - 1),
                                perf_mode=DR)
                            i += 1
                if tt % 2 == 0:
                    nc.vector.tensor_scalar_mul(o[:, tt, :], ps[:],
                                                sc["dq_h"])
                else:
                    nc.scalar.mul(o[:, tt, :], ps[:], sc["dq_h"])
            nc.sync.dma_start(
                out=logits_d.ap()[:, vc * 512:(vc + 1) * 512].rearrange(
                    "(t p) v -> p t v", p=128),
                in_=o[:])
        for p in (pout, ph, pf, ps_big, gmicro, gu, gx2, gx, glob):
            p.release()

    nc.compile()
    return nc

# ---------------------------------------------------------------------------
# host side
# ---------------------------------------------------------------------------

def _pow2_scale(m, target=224.0):
    if m == 0:
        return 1.0
    return float(2.0 ** np.floor(np.log2(target / m)))


def _hi_lo(w, s):
    ws = w * s
    hi = ws.astype(E4)
    lo = (ws - hi.astype(np.float32)).astype(E4)
    return hi, lo


# own global AQ-blocks per half, in local order
OWN_BLOCKS = {0: (0, 3), 1: (1, 2)}


def _prep_inputs(inputs, n_layers):
    f32 = np.float32
    idx = np.asarray(inputs["idx"])
    wte = np.asarray(inputs["wte"], f32)
    wpe = np.asarray(inputs["wpe"], f32)

    sc = {}
    common = {}
    for l in range(n_layers):
        ln1w = np.asarray(inputs["ln1_w"][l], f32)
        ln1b = np.asarray(inputs["ln1_b"][l], f32)
        aw = np.asarray(inputs["attn_w"][l], f32)
        ab = np.asarray(inputs["attn_b"][l], f32)
        awf = ln1w[:, None] * aw
        abf = ab + ln1b @ aw
        assert not np.any(abf), "nonzero attn bias not supported"
        qk = awf[:, :1536]
        s_qk = _pow2_scale(float(np.abs(qk).max()))
        qh, ql = _hi_lo(qk, s_qk)

        def qk_layout(a):
            return np.ascontiguousarray(
                a.reshape(KP, 2, 128, 12, 128).transpose(3, 2, 0, 1, 4)
            ).reshape(12 * 128, KP, 2, 128)
        common[f"qkwh{l}"] = qk_layout(qh)
        common[f"qkwl{l}"] = qk_layout(ql)
        sc[f"dq_qk{l}"] = 1.0 / (SX * s_qk)
        vw = awf[:, 1536:]
        s_v = _pow2_scale(float(np.abs(vw).max()))
        vh, vl = _hi_lo(vw, s_v)

        def v_layout(a):
            return np.ascontiguousarray(
                a.reshape(KP, 2, 128, C).transpose(2, 0, 1, 3))
        common[f"vwh{l}"] = v_layout(vh)
        common[f"vwl{l}"] = v_layout(vl)
        sc[f"dq_v{l}"] = 1.0 / (SX * s_v)

        pw = np.asarray(inputs["proj_w"][l], f32)
        assert not np.any(np.asarray(inputs["proj_b"][l])), "proj bias"
        s_p = _pow2_scale(float(np.abs(pw).max()))
        ph_, pl_ = _hi_lo(pw, s_p)

        def p_layout(a):
            return np.ascontiguousarray(
                a.reshape(KP, 2, 128, CT, 128).transpose(3, 2, 0, 1, 4)
            ).reshape(CT * 128, KP, 2, 128)
        common[f"pwh{l}"] = p_layout(ph_)
        common[f"pwl{l}"] = p_layout(pl_)
        sc[f"dq_p{l}"] = 1.0 / (SY * s_p)

        ln2w = np.asarray(inputs["ln2_w"][l], f32)
        ln2b = np.asarray(inputs["ln2_b"][l], f32)
        fw = np.asarray(inputs["fc_w"][l], f32)
        fbv = np.asarray(inputs["fc_b"][l], f32)
        fwf = ln2w[:, None] * fw
        fbf = fbv + ln2b @ fw
        assert not np.any(fbf), "nonzero fc bias not supported"
        s_fc = _pow2_scale(float(np.abs(fwf).max()))
        fh, fl = _hi_lo(fwf, s_fc)

        def f_layout(a):
            return np.ascontiguousarray(
                a.reshape(KP, 2, 128, HT, 128).transpose(3, 2, 0, 1, 4)
            ).reshape(HT * 128, KP, 2, 128)
        common[f"fwh{l}"] = f_layout(fh)
        common[f"fwl{l}"] = f_layout(fl)
        sc[f"dq_fc{l}"] = 1.0 / (SX * s_fc)

        f2w = np.asarray(inputs["fc2_w"][l], f32)
        assert not np.any(np.asarray(inputs["fc2_b"][l])), "fc2 bias"
        s_f2 = _pow2_scale(float(np.abs(f2w).max()))
        f2h, f2l = _hi_lo(f2w, s_f2)

        def f2_layout(a):
            return np.ascontiguousarray(
                a.reshape(HP, 2, 128, CT, 128).transpose(3, 2, 0, 1, 4)
            ).reshape(CT * 128, HP, 2, 128)
        common[f"f2wh{l}"] = f2_layout(f2h)
        common[f"f2wl{l}"] = f2_layout(f2l)
        sc[f"dq_f2{l}"] = 1.0 / (SH * s_f2)

    p = np.arange(128)[:, None]
    f = np.arange(AQ)[None, :]
    masks = np.zeros((128, 2, AQ), f32)
    masks[:, 0, :] = np.where(p > f, -240.0, 0.0)
    masks[:, 1, :] = np.where(p + 128 > f, -240.0, 0.0)
    common["masks"] = masks.astype(E4)
    iden = np.zeros((128, 2, 128), f32)
    iden[:, 0, :] = 128.0 * np.eye(128)
    common["iden"] = iden.astype(E4)

    lnfw = np.asarray(inputs["lnf_w"], f32)
    lnfb = np.asarray(inputs["lnf_b"], f32)
    assert not np.any(lnfb @ wte.T), "nonzero head bias not supported"
    wh = lnfw[:, None] * wte.T                     # [768, V]
    whp = np.zeros((C, VS2), f32)
    whp[:, :V] = wh
    m = float(np.abs(wh).max())
    s_h = float(2.0 ** np.floor(np.log2(240.0 / m)))
    sc["dq_h"] = 1.0 / (s_h * SX)

    slh = whp * s_h
    hih = slh.astype(E4)
    loh = (slh - hih.astype(f32)).astype(E4)
    common["whh"] = np.ascontiguousarray(
        hih.reshape(KP, 2, 128, NVC, 512).transpose(3, 2, 0, 1, 4)
    ).reshape(NVC * 128, KP, 2, 512)
    common["whl"] = np.ascontiguousarray(
        loh.reshape(KP, 2, 128, NVC, 512).transpose(3, 2, 0, 1, 4)
    ).reshape(NVC * 128, KP, 2, 512)

    def t6(a):          # [768, TO] -> [128, 6, TO]
        return np.ascontiguousarray(
            a.reshape(CT, 128, a.shape[1]).transpose(1, 0, 2))

    x0 = wte[idx] + wpe[None, :T]                  # [B, T, C]
    in_maps = []
    for c in range(8):
        s, half = c & 3, c >> 2
        bA, bB = OWN_BLOCKS[half]
        m2 = dict(common)
        xo = np.concatenate([x0[s, bA * AQ:(bA + 1) * AQ],
                             x0[s, bB * AQ:(bB + 1) * AQ]], axis=0)
        m2["x0t"] = t6(np.ascontiguousarray(xo.T))
        # RS staging masks: slot s2 carries my data iff my rank != s2
        mm = np.zeros((128, 2), f32)
        mm[:, 1 - half] = 1.0
        m2["mm"] = mm
        # data masks: mA for qA x remA ; mB for qB x remB
        m2d = np.zeros((128, 2, 2, AQ), f32)
        if half == 0:
            m2d[:, 0, :, :] = -240.0        # qA(blk0) x rA(blk1): future
            # qB(blk3) x rB(blk2): full attend -> 0
        else:
            # qA(blk1) x rA(blk0): full attend -> 0
            m2d[:, 1, :, :] = -240.0        # qB(blk2) x rB(blk3): future
        m2["masks2"] = m2d.astype(E4)
        in_maps.append(m2)
    return in_maps, sc


def kernel(**inputs):
    global LAST_RESULT, LAST_NC
    n_layers = L
    in_maps, sc = _prep_inputs(inputs, n_layers)
    key = (n_layers, tuple(sorted(sc.items())))
    if key not in _CACHE:
        _CACHE[key] = build_program(sc, n_layers)
    nc = _CACHE[key]
    LAST_NC = nc
    res = run_bass_kernel_spmd(nc, in_maps, core_ids=list(range(8)))
    LAST_RESULT = res
    out = np.empty((B, T, V), np.float32)
    for c in range(8):
        s, half = c & 3, c >> 2
        bA, bB = OWN_BLOCKS[half]
        part = np.asarray(res.results[c]["logits"]).astype(np.float32)
        out[s, bA * AQ:(bA + 1) * AQ] = part[0:AQ, :V]
        out[s, bB * AQ:(bB + 1) * AQ] = part[AQ:2 * AQ, :V]
    return out


if __name__ == "__main__":
    rng = np.random.default_rng(0)
    ins = {
        "idx": rng.integers(0, V, (B, T)).astype(np.int32),
        "wte": (rng.standard_normal((V, C)) * 0.02).astype(np.float32),
        "wpe": (rng.standard_normal((T, C)) * 0.02).astype(np.float32),
        "ln1_w": np.ones((L, C), np.float32),
        "ln1_b": np.zeros((L, C), np.float32),
        "attn_w": (rng.standard_normal((L, C, 3 * C)) * 0.02).astype(np.float32),
        "attn_b": np.zeros((L, 3 * C), np.float32),
        "proj_w": (rng.standard_normal((L, C, C)) * 0.02).astype(np.float32),
        "proj_b": np.zeros((L, C), np.float32),
        "ln2_w": np.ones((L, C), np.float32),
        "ln2_b": np.zeros((L, C), np.float32),
        "fc_w": (rng.standard_normal((L, C, 4 * C)) * 0.02).astype(np.float32),
        "fc_b": np.zeros((L, 4 * C), np.float32),
        "fc2_w": (rng.standard_normal((L, 4 * C, C)) * 0.02).astype(np.float32),
        "fc2_b": np.zeros((L, C), np.float32),
        "lnf_w": np.ones((C,), np.float32),
        "lnf_b": np.zeros((C,), np.float32),
    }
    out = kernel(**ins)
    print("out", out.shape, out.dtype, float(np.abs(out).max()))


# revision 19
# speedup vs baseline: 1.2304x; 1.0024x over previous
"""MiniGPT forward on 8 Trainium2 NeuronCores — sequence-split variant.

Core c: sequence (c & 3), token-half (c >> 2).  In AQ=256 blocks of the
1024-token sequence, half 0 owns blocks {0, 3}, half 1 owns {1, 2} (equal
causal-attention load).  Each core runs the 6 transformer blocks for its
OWN 512 tokens only — no duplicated block compute.  Per layer the pair
exchanges K and V via two ReduceScatters (k first): each core stages
[k*m0, k*m1] with per-core {0,1} masks so the RS output is exactly the
PEER's k/v at a uniform address (SPMD-safe, no rank branching).
Attention = own-block phase (overlaps the RS) + remote phase; causal /
validity masks are per-core input data applied on the PE via the
iden @ mask DoubleRow trick.  One remote block per core is fully masked
waste, keeping the instruction stream identical across cores.

Block linears are fp8(e4m3) DoubleRow 3-term (Wh@xh + Wh@xl + Wl@xh,
0.75x bf16 PE cost); attention stays bf16.  The LM head is token-split:
each core computes its own 512 tokens x the FULL vocab with the same
3-term fp8 scheme, logits stream out in bf16.
"""

import sys

sys.path.insert(0, "/opt/trn_rl_repo")

import numpy as np
import ml_dtypes

import concourse.bacc as bacc
import concourse.tile as tile
from concourse import mybir
from concourse.bass_utils import run_bass_kernel_spmd

F32 = mybir.dt.float32
F32R = mybir.dt.float32r
BF16 = mybir.dt.bfloat16
FP8 = mybir.dt.float8e4
ALU = mybir.AluOpType
ACT = mybir.ActivationFunctionType
DR = mybir.MatmulPerfMode.DoubleRow
E4 = ml_dtypes.float8_e4m3

B, T, C, H, HD, L, V = 4, 1024, 768, 12, 64, 6, 50257
TO = 512                # own tokens per core
CT = C // 128           # 6 c-tiles
KP = CT // 2            # 3 k-pairs
TT = TO // 128          # 4 own token tiles
AQ = 256                # attention query block
HT = 3072 // 128        # 24 hidden tiles
HP = HT // 2            # 12 hidden k-pairs
VS2 = 51200             # padded vocab
NVC = VS2 // 512        # 100
EPS = 1e-5
SX = 16.0               # fp8 scale for LN outputs (blocks + head)
SY = 32.0               # fp8 scale for attention output y (folded in ones)
SH = 1.0                # gelu output used unscaled in fp8

RG = [[0, 4], [1, 5], [2, 6], [3, 7]]   # pair replica groups

_CACHE = {}
LAST_RESULT = None
LAST_NC = None


def build_program(sc, n_layers=L):
    nc = bacc.Bacc(None, target_bir_lowering=False)

    def f8_in(name, shape):
        return nc.dram_tensor(name, list(shape), FP8, kind="ExternalInput")

    x0t_d = nc.dram_tensor("x0t", [128, CT, TO], F32R, kind="ExternalInput")
    wqh, wql, wvh, wvl, wph, wpl = [], [], [], [], [], []
    wfh, wfl, w2h, w2l = [], [], [], []
    for l in range(n_layers):
        wqh.append(f8_in(f"qkwh{l}", (12 * 128, KP, 2, 128)))
        wql.append(f8_in(f"qkwl{l}", (12 * 128, KP, 2, 128)))
        wvh.append(f8_in(f"vwh{l}", (128, KP, 2, C)))
        wvl.append(f8_in(f"vwl{l}", (128, KP, 2, C)))
        wph.append(f8_in(f"pwh{l}", (CT * 128, KP, 2, 128)))
        wpl.append(f8_in(f"pwl{l}", (CT * 128, KP, 2, 128)))
        wfh.append(f8_in(f"fwh{l}", (HT * 128, KP, 2, 128)))
        wfl.append(f8_in(f"fwl{l}", (HT * 128, KP, 2, 128)))
        w2h.append(f8_in(f"f2wh{l}", (CT * 128, HP, 2, 128)))
        w2l.append(f8_in(f"f2wl{l}", (CT * 128, HP, 2, 128)))
    whh_d = f8_in("whh", (NVC * 128, KP, 2, 512))
    whl_d = f8_in("whl", (NVC * 128, KP, 2, 512))
    masks_d = f8_in("masks", (128, 2, AQ))          # local diag (shared)
    masks2_d = f8_in("masks2", (128, 2, 2, AQ))     # [mA|mB, half, q] per-core
    iden_d = f8_in("iden", (128, 2, 128))
    mm_d = nc.dram_tensor("mm", [128, 2], F32, kind="ExternalInput")
    logits_d = nc.dram_tensor("logits", [TO, VS2], BF16,
                              kind="ExternalOutput")

    ln_ctr = [0]

    with nc.allow_low_precision("fp8 3-term error-feedback intentional"), \
         tile.TileContext(nc) as tc:
        glob = tc.alloc_tile_pool(name="glob", bufs=1)
        gx = tc.alloc_tile_pool(name="gx", bufs=1)
        gx2 = tc.alloc_tile_pool(name="gx2", bufs=3)
        gu = tc.alloc_tile_pool(name="gu", bufs=3)
        gmicro = tc.alloc_tile_pool(name="gmicro", bufs=1)
        ps_big = tc.alloc_tile_pool(name="ps_big", bufs=4, space="PSUM")
        gxn = tc.alloc_tile_pool(name="gxn", bufs=1)
        gw = tc.alloc_tile_pool(name="gw", bufs=8)
        gwv = tc.alloc_tile_pool(name="gwv", bufs=1)
        gw24 = tc.alloc_tile_pool(name="gw24", bufs=3)
        gkv = tc.alloc_tile_pool(name="gkv", bufs=1)
        gst = tc.alloc_tile_pool(name="gst", bufs=1)
        dram = tc.alloc_tile_pool(name="dram", bufs=2, space="DRAM")

        ones_col = glob.tile([128, 1], F32R, tag="ones_col")
        ones_row = glob.tile([1, 128], F32R, tag="ones_row")
        epsh_t = glob.tile([1, 1], F32, tag="epsh")
        masks_t = glob.tile([128, 2, AQ], FP8, tag="masks")
        masks2_t = glob.tile([128, 2, 2, AQ], FP8, tag="masks2")
        iden_t = glob.tile([128, 2, 128], FP8, tag="iden")
        mm_t = glob.tile([128, 2], F32, tag="mm")
        nc.vector.memset(ones_col[:].bitcast(F32), 1.0)
        nc.vector.memset(ones_row[:].bitcast(F32), 1.0)
        nc.vector.memset(epsh_t[:], EPS / (SX * SX))
        nc.sync.dma_start(out=masks_t[:], in_=masks_d[:])
        nc.sync.dma_start(out=masks2_t[:], in_=masks2_d[:])
        nc.sync.dma_start(out=iden_t[:], in_=iden_d[:])
        nc.sync.dma_start(out=mm_t[:], in_=mm_d[:])

        xT = gx.tile([128, CT, TO], F32R, tag="xT")
        nc.sync.dma_start(out=xT[:], in_=x0t_d[:])

        # persistent v tiles: [p, tt, h, 0:64] = v ; [.., 64:128] = 1/SY
        gv = tc.alloc_tile_pool(name="gv", bufs=1)
        vOwn = gv.tile([128, TT, H, 128], BF16, tag="vOwn")
        vRem = gv.tile([128, TT, H, 128], BF16, tag="vRem")
        kRem = gkv.tile([128, CT, TO], BF16, tag="kRem")
        nc.gpsimd.memset(vOwn[:, :, :, 64:128], 1.0 / SY)
        nc.gpsimd.memset(vRem[:, :, :, 64:128], 1.0 / SY)

        def ln_full(xin, xh, xl, ps_stat, ps_bc):
            """(xh + xl) ~= SX * (xin - mu) * rstd in fp8, all 512 tokens."""
            qs = slice(0, TO)
            s_ps = ps_stat.tile([1, TO], F32, space="PSUM", tag="stat")
            q_ps = ps_stat.tile([1, TO], F32, space="PSUM", tag="stat")
            for kt in range(CT):
                nc.tensor.matmul(s_ps[:], ones_col[:], xin[:, kt, qs],
                                 start=(kt == 0), stop=(kt == CT - 1))
            for kt in range(CT):
                x2 = gx2.tile([128, TO], F32R, tag="x2")
                nc.gpsimd.tensor_tensor(
                    out=x2[:], in0=xin[:, kt, qs],
                    in1=xin[:, kt, qs], op=ALU.mult)
                nc.tensor.matmul(q_ps[:], ones_col[:], x2[:],
                                 start=(kt == 0), stop=(kt == CT - 1))
            mu = gmicro.tile([1, TO], F32R, tag="mu")
            nc.scalar.mul(mu[:], s_ps[:], 1.0 / C)
            mu2 = gmicro.tile([1, TO], F32, tag="mu2")
            nc.scalar.activation(mu2[:], mu[:], ACT.Square)
            var = gmicro.tile([1, TO], F32, tag="var")
            nc.vector.scalar_tensor_tensor(
                out=var[:], in0=q_ps[:], scalar=1.0 / C, in1=mu2[:],
                op0=ALU.mult, op1=ALU.subtract)
            sd = gmicro.tile([1, TO], F32, tag="sd")
            nc.scalar.activation(sd[:], var[:], ACT.Sqrt, bias=epsh_t[:],
                                 scale=1.0 / (SX * SX))
            r = gmicro.tile([1, TO], F32R, tag="r")
            nc.vector.reciprocal(r[:], sd[:])            # SX/sd
            mr = gmicro.tile([1, TO], F32R, tag="mr")
            nc.vector.tensor_tensor(out=mr[:], in0=mu[:], in1=r[:],
                                    op=ALU.mult)
            bc = ps_bc.tile([128, TO], F32, space="PSUM", tag="bc")
            nc.tensor.matmul(bc[:], ones_row[:], r[:], start=True, stop=True)
            bc2 = ps_bc.tile([128, TO], F32, space="PSUM", tag="bc")
            nc.tensor.matmul(bc2[:], ones_row[:], mr[:], start=True,
                             stop=True)
            for kt in range(CT):
                t = gx2.tile([128, TO], F32, tag="lnt")
                nc.vector.tensor_tensor(out=t[:],
                                        in0=xin[:, kt, qs].bitcast(F32),
                                        in1=bc[:], op=ALU.mult)
                u = gu.tile([128, TO], F32, tag="lnu")
                nc.vector.tensor_tensor(out=u[:], in0=t[:],
                                        in1=bc2[:], op=ALU.subtract)
                nc.scalar.copy(xh[:, kt, :], u[:])
                nc.gpsimd.tensor_tensor(out=xl[:, kt, :], in0=u[:],
                                        in1=xh[:, kt, :],
                                        op=ALU.subtract)

        def ln_pools():
            i = ln_ctr[0]
            ln_ctr[0] += 1
            ps_stat = tc.alloc_tile_pool(name=f"ps_st{i}", bufs=2,
                                         space="PSUM")
            ps_bc = tc.alloc_tile_pool(name=f"ps_bc{i}", bufs=2, space="PSUM")
            return ps_stat, ps_bc

        def dr_mm(ps, wh, wl, xh, xl, qs, kps):
            """accumulate 3-term fp8 DR: Wh@xh + Wh@xl + Wl@xh into ps."""
            n = 3 * kps
            i = 0
            for w_, x_ in ((wh, xh), (wh, xl), (wl, xh)):
                for kp in range(kps):
                    nc.tensor.matmul(ps, w_[:, kp, :, :],
                                     x_[:, 2 * kp:2 * kp + 2, qs],
                                     start=(i == 0), stop=(i == n - 1),
                                     perf_mode=DR)
                    i += 1

        def mask_mm(sps, half, msrc):
            nc.tensor.matmul(
                sps[:, half * AQ:(half + 1) * AQ],
                iden_t[:],
                msrc.rearrange("p (i q) -> p i q", i=1)
                .broadcast_to([128, 2, AQ]),
                start=False, stop=True, perf_mode=DR,
                skip_group_check=True)

        def score_block(hp, rows, ksrc, kbase, qsl, msrc):
            """one AQ x AQ*?? score block: k tiles (kbase, kbase+1) of ksrc
            vs q columns qsl; optional additive mask (None | AP)."""
            sps = ps_sc.tile([128, 2 * AQ], F32, space="PSUM", tag="sc",
                             name="sps")
            for half in range(2):
                kt = kbase + half
                nc.tensor.matmul(
                    sps[:, half * AQ:(half + 1) * AQ],
                    ksrc[rows, 6 + hp, kt * 128:(kt + 1) * 128]
                    if ksrc is qkT else
                    ksrc[rows, hp, kt * 128:(kt + 1) * 128],
                    qkT[rows, hp, qsl],
                    start=True, stop=(msrc is None),
                    skip_group_check=(msrc is not None))
                if msrc is not None:
                    mask_mm(sps, half, msrc[:, half, :])
            e = pE.tile([128, 2 * AQ], BF16, tag="E", name="e")
            nc.scalar.activation(e[:], sps[:], ACT.Exp, scale=0.125)
            return e

        for l in range(n_layers):
            pa = tc.alloc_tile_pool(name=f"pa{l}", bufs=1)

            # ---- LN1 -> xn8 h/l (fp8) ----
            xnh = gxn.tile([128, CT, TO], FP8, tag="xnh")
            xnl = gxn.tile([128, CT, TO], FP8, tag="xnl")
            qkT = pa.tile([128, 12, TO], BF16, tag="qkT")
            st1, bc1 = ln_pools()
            ln_full(xT, xnh, xnl, st1, bc1)

            # ---- k projections first so the k-RS starts early ----
            def qk_slots(slots):
                for s_ in slots:
                    wh = gw.tile([128, KP, 2, 128], FP8, tag="w6h",
                                 name="wh")
                    nc.sync.dma_start(
                        out=wh[:], in_=wqh[l].ap()[s_ * 128:(s_ + 1) * 128])
                    wl = gw.tile([128, KP, 2, 128], FP8, tag="w6l",
                                 name="wl")
                    nc.sync.dma_start(
                        out=wl[:], in_=wql[l].ap()[s_ * 128:(s_ + 1) * 128])
                    ps = ps_big.tile([128, TO], F32, space="PSUM", tag="px",
                                     name="ps")
                    dr_mm(ps[:], wh, wl, xnh, xnl, slice(0, TO), KP)
                    nc.vector.tensor_scalar_mul(qkT[:, s_, :], ps[:],
                                                sc[f"dq_qk{l}"])

            qk_slots(range(6, 12))
            qk_slots(range(6))
            for p in (bc1, st1):
                p.release()

            # ---- k export: stage [k*m0, k*m1], RS, import peer k ----
            kst = gst.tile([128, 2, CT, TO], BF16, tag="kst")
            for sl in range(2):
                nc.vector.tensor_scalar_mul(kst[:, sl], qkT[:, 6:12, :],
                                            mm_t[:, sl:sl + 1])
            k_in = dram.tile([2, 128, CT, TO], BF16, tag="k_in")
            k_out = dram.tile([128, CT, TO], BF16, tag="k_out")
            nc.sync.dma_start(
                out=k_in[:].rearrange("a p c t -> p a c t"), in_=kst[:])
            nc.gpsimd.collective_compute(
                "ReduceScatter", ALU.add, replica_groups=RG,
                ins=[k_in.opt()], outs=[k_out.opt()])
            nc.sync.dma_start(out=kRem[:], in_=k_out[:])

            # ---- v projection (tokens on PSUM partitions) ----
            wvht = gwv.tile([128, KP, 2, C], FP8, tag="vwh")
            nc.sync.dma_start(out=wvht[:], in_=wvh[l].ap()[:])
            wvlt = gwv.tile([128, KP, 2, C], FP8, tag="vwl")
            nc.sync.dma_start(out=wvlt[:], in_=wvl[l].ap()[:])
            for tt in range(TT):
                tsl = slice(tt * 128, (tt + 1) * 128)
                psA = ps_big.tile([128, 512], F32, space="PSUM", tag="px")
                psB = ps_big.tile([128, 256], F32, space="PSUM", tag="px")
                n = 3 * KP
                for ps_, csl in ((psA, slice(0, 512)),
                                 (psB, slice(512, 768))):
                    i = 0
                    for x_, w_ in ((xnh, wvht), (xnl, wvht), (xnh, wvlt)):
                        for kp in range(KP):
                            nc.tensor.matmul(ps_[:],
                                             x_[:, 2 * kp:2 * kp + 2, tsl],
                                             w_[:, kp, :, csl],
                                             start=(i == 0),
                                             stop=(i == n - 1),
                                             perf_mode=DR)
                            i += 1
                nc.vector.tensor_scalar_mul(
                    vOwn[:, tt, 0:8, 0:64],
                    psA[:].rearrange("p (h d) -> p h d", h=8),
                    sc[f"dq_v{l}"])
                nc.vector.tensor_scalar_mul(
                    vOwn[:, tt, 8:12, 0:64],
                    psB[:].rearrange("p (h d) -> p h d", h=4),
                    sc[f"dq_v{l}"])

            # ---- v export ----
            vst = gst.tile([128, 2, TT, H, 64], BF16, tag="vst")
            for sl in range(2):
                nc.vector.tensor_scalar_mul(vst[:, sl],
                                            vOwn[:, :, :, 0:64],
                                            mm_t[:, sl:sl + 1])
            v_in = dram.tile([2, 128, TT, H, 64], BF16, tag="v_in")
            v_out = dram.tile([128, TT, H, 64], BF16, tag="v_out")
            nc.sync.dma_start(
                out=v_in[:].rearrange("a p t h d -> p a t h d"), in_=vst[:])
            nc.gpsimd.collective_compute(
                "ReduceScatter", ALU.add, replica_groups=RG,
                ins=[v_in.opt()], outs=[v_out.opt()])
            nc.sync.dma_start(out=vRem[:, :, :, 0:64], in_=v_out[:])

            # ---- attention ----
            yT = pa.tile([128, CT, TO], BF16, tag="yT")
            pE = tc.alloc_tile_pool(name=f"pE{l}", bufs=10)
            prec = tc.alloc_tile_pool(name=f"prec{l}", bufs=4)
            gyo = tc.alloc_tile_pool(name=f"gyo{l}", bufs=1)
            ps_sc = tc.alloc_tile_pool(name=f"ps_sc{l}", bufs=2, space="PSUM")
            ps_av = tc.alloc_tile_pool(name=f"ps_av{l}", bufs=2, space="PSUM")
            yo = gyo.tile([128, 12, 2, AQ], F32, tag="yo")  # own partials

            qA, qB = slice(0, AQ), slice(AQ, 2 * AQ)
            # phase 1: own blocks (independent of the RS)
            for hp in range(6):
                for h in (2 * hp, 2 * hp + 1):
                    par = h % 2
                    rows = slice(64 * par, 64 * par + 64)
                    eA = score_block(hp, rows, qkT, 0, qA, masks_t)
                    eB1 = score_block(hp, rows, qkT, 0, qB, None)
                    eB2 = score_block(hp, rows, qkT, 2, qB, masks_t)
                    ya = ps_av.tile([128, AQ], F32, space="PSUM", tag="av")
                    for kt in range(2):
                        nc.tensor.matmul(ya[:], vOwn[:, kt, h, :],
                                         eA[:, kt * AQ:(kt + 1) * AQ],
                                         start=(kt == 0), stop=(kt == 1))
                    nc.vector.tensor_copy(out=yo[:, h, 0, :], in_=ya[:])
                    yb = ps_av.tile([128, AQ], F32, space="PSUM", tag="av")
                    for kt in range(4):
                        e = eB1 if kt < 2 else eB2
                        nc.tensor.matmul(yb[:], vOwn[:, kt, h, :],
                                         e[:, (kt % 2) * AQ:(kt % 2 + 1) * AQ],
                                         start=(kt == 0), stop=(kt == 3))
                    nc.vector.tensor_copy(out=yo[:, h, 1, :], in_=yb[:])
            # phase 2: remote blocks + combine
            for hp in range(6):
                for h in (2 * hp, 2 * hp + 1):
                    par = h % 2
                    rows = slice(64 * par, 64 * par + 64)
                    eAr = score_block(hp, rows, kRem, 0, qA, masks2_t[:, 0])
                    eBr1 = score_block(hp, rows, kRem, 0, qB, None)
                    eBr2 = score_block(hp, rows, kRem, 2, qB,
                                       masks2_t[:, 1])
                    for qi, es in ((0, (eAr, eAr)), (1, (eBr1, eBr2))):
                        nkt = 2 if qi == 0 else 4
                        yr = ps_av.tile([128, AQ], F32, space="PSUM",
                                        tag="av")
                        for kt in range(nkt):
                            e = es[0] if kt < 2 else es[1]
                            nc.tensor.matmul(
                                yr[:], vRem[:, kt, h, :],
                                e[:, (kt % 2) * AQ:(kt % 2 + 1) * AQ],
                                start=(kt == 0), stop=(kt == nkt - 1))
                        yc = prec.tile([128, AQ], F32, tag="yc")
                        nc.vector.tensor_tensor(out=yc[:], in0=yr[:],
                                                in1=yo[:, h, qi, :],
                                                op=ALU.add)
                        rec = prec.tile([64, AQ], F32, tag="rec")
                        nc.vector.reciprocal(rec[:], yc[64:128, :])
                        qsl = slice(qi * AQ, (qi + 1) * AQ)
                        yrow = slice(64 * par, 64 * par + 64)
                        nc.vector.tensor_tensor(out=yT[yrow, hp, qsl],
                                                in0=yc[0:64, :], in1=rec[:],
                                                op=ALU.mult)
            for p in (ps_av, ps_sc, gyo, prec, pE):
                p.release()

            # ---- y8 split, proj + residual, LN2 + MLP ----
            py8 = tc.alloc_tile_pool(name=f"py8{l}", bufs=1)
            ph8 = tc.alloc_tile_pool(name=f"ph8{l}", bufs=1)
            phbf = tc.alloc_tile_pool(name=f"phbf{l}", bufs=3)
            y8h = py8.tile([128, CT, TO], FP8, tag="y8h")
            y8l = py8.tile([128, CT, TO], FP8, tag="y8l")
            for kt in range(CT):
                nc.gpsimd.tensor_copy(out=y8h[:, kt, :], in_=yT[:, kt, :])
                nc.gpsimd.tensor_tensor(
                    out=y8l[:, kt, :], in0=yT[:, kt, :],
                    in1=y8h[:, kt, :], op=ALU.subtract)
            for ot in range(CT):
                wh = gw.tile([128, KP, 2, 128], FP8, tag="w6h")
                nc.sync.dma_start(out=wh[:],
                                  in_=wph[l].ap()[ot * 128:(ot + 1) * 128])
                wl = gw.tile([128, KP, 2, 128], FP8, tag="w6l")
                nc.sync.dma_start(out=wl[:],
                                  in_=wpl[l].ap()[ot * 128:(ot + 1) * 128])
                ps = ps_big.tile([128, TO], F32, space="PSUM", tag="px")
                dr_mm(ps[:], wh, wl, y8h, y8l, slice(0, TO), KP)
                nc.vector.scalar_tensor_tensor(
                    out=xT[:, ot, :], in0=ps[:], scalar=sc[f"dq_p{l}"],
                    in1=xT[:, ot, :], op0=ALU.mult, op1=ALU.add)
            xn2h = gxn.tile([128, CT, TO], FP8, tag="xnh")
            xn2l = gxn.tile([128, CT, TO], FP8, tag="xnl")
            st2, bc2p = ln_pools()
            ln_full(xT, xn2h, xn2l, st2, bc2p)
            h8h = ph8.tile([128, HT, TO], FP8, tag="h8h")
            h8l = ph8.tile([128, HT, TO], FP8, tag="h8l")
            for ot in range(HT):
                wh = gw.tile([128, KP, 2, 128], FP8, tag="w6h")
                nc.sync.dma_start(out=wh[:],
                                  in_=wfh[l].ap()[ot * 128:(ot + 1) * 128])
                wl = gw.tile([128, KP, 2, 128], FP8, tag="w6l")
                nc.sync.dma_start(out=wl[:],
                                  in_=wfl[l].ap()[ot * 128:(ot + 1) * 128])
                ps = ps_big.tile([128, TO], F32, space="PSUM", tag="px")
                dr_mm(ps[:], wh, wl, xn2h, xn2l, slice(0, TO), KP)
                hbf = phbf.tile([128, TO], BF16, tag="hbf")
                nc.scalar.activation(h8h[:, ot, :], ps[:], ACT.Gelu,
                                     scale=sc[f"dq_fc{l}"])
                nc.scalar.activation(hbf[:], ps[:], ACT.Gelu,
                                     scale=sc[f"dq_fc{l}"])
                nc.gpsimd.tensor_tensor(
                    out=h8l[:, ot, :], in0=hbf[:],
                    in1=h8h[:, ot, :], op=ALU.subtract)
            for ot in range(CT):
                wh = gw24.tile([128, HP, 2, 128], FP8, tag="w24h")
                nc.sync.dma_start(out=wh[:],
                                  in_=w2h[l].ap()[ot * 128:(ot + 1) * 128])
                wl = gw24.tile([128, HP, 2, 128], FP8, tag="w24l")
                nc.sync.dma_start(out=wl[:],
                                  in_=w2l[l].ap()[ot * 128:(ot + 1) * 128])
                ps = ps_big.tile([128, TO], F32, space="PSUM", tag="px")
                dr_mm(ps[:], wh, wl, h8h, h8l, slice(0, TO), HP)
                nc.vector.scalar_tensor_tensor(
                    out=xT[:, ot, :], in0=ps[:], scalar=sc[f"dq_f2{l}"],
                    in1=xT[:, ot, :], op0=ALU.mult, op1=ALU.add)
            for p in (bc2p, st2, phbf, ph8, py8, pa):
                p.release()

        # ---- final LN (fp8 h/l, scale SX) + token-split LM head ----
        for p in (gv, dram, gst, gkv, gw24, gwv, gw, gxn):
            p.release()
        pf = tc.alloc_tile_pool(name="pf", bufs=1)
        xf = pf.tile([128, CT, TO], FP8, tag="xf")
        xl = pf.tile([128, CT, TO], FP8, tag="xl")
        stf, bcf = ln_pools()
        ln_full(xT, xf, xl, stf, bcf)
        for p in (bcf, stf):
            p.release()

        ph = tc.alloc_tile_pool(name="ph", bufs=8)
        pout = tc.alloc_tile_pool(name="pout", bufs=4)
        for vc in range(NVC):
            wh8 = ph.tile([128, KP, 2, 512], FP8, tag="wh")
            nc.sync.dma_start(out=wh8[:],
                              in_=whh_d.ap()[vc * 128:(vc + 1) * 128])
            wl8 = ph.tile([128, KP, 2, 512], FP8, tag="whl")
            nc.sync.dma_start(out=wl8[:],
                              in_=whl_d.ap()[vc * 128:(vc + 1) * 128])
            o = pout.tile([128, TT, 512], BF16, tag="out")
            for tt in range(TT):
                tsl = slice(tt * 128, (tt + 1) * 128)
                ps = ps_big.tile([128, 512], F32, space="PSUM", tag="px")
                terms = [(xf, wh8), (xl, wh8), (xf, wl8)]
                for cc in range(2):
                    i = 0
                    for xsrc, wsrc in terms:
                        for kp in range(KP):
                            nc.tensor.matmul(
                                ps[:, cc * 256:(cc + 1) * 256],
                                xsrc[:, 2 * kp:2 * kp + 2, tsl],
                                wsrc[:, kp, :, cc * 256:cc * 256 + 256],
                                start=(i == 0), stop=(i == 3 * KP - 1),
                                perf_mode=DR)
                            i += 1
                if tt % 2 == 0:
                    nc.vector.tensor_scalar_mul(o[:, tt, :], ps[:],
                                                sc["dq_h"])
                else:
                    nc.scalar.mul(o[:, tt, :], ps[:], sc["dq_h"])
            nc.sync.dma_start(
                out=logits_d.ap()[:, vc * 512:(vc + 1) * 512].rearrange(
                    "(t p) v -> p t v", p=128),
                in_=o[:])
        for p in (pout, ph, pf, ps_big, gmicro, gu, gx2, gx, glob):
            p.release()

    nc.compile()
    return nc

# ---------------------------------------------------------------------------
# host side
# ---------------------------------------------------------------------------

def _pow2_scale(m, target=224.0):
    if m == 0:
        return 1.0
    return float(2.0 ** np.floor(np.log2(target / m)))


def _hi_lo(w, s):
    ws = w * s
    hi = ws.astype(E4)
    lo = (ws - hi.astype(np.float32)).astype(E4)
    return hi, lo


# own global AQ-blocks per half, in local order
OWN_BLOCKS = {0: (0, 3), 1: (1, 2)}


def _prep_inputs(inputs, n_layers):
    f32 = np.float32
    idx = np.asarray(inputs["idx"])
    wte = np.asarray(inputs["wte"], f32)
    wpe = np.asarray(inputs["wpe"], f32)

    sc = {}
    common = {}
    for l in range(n_layers):
        ln1w = np.asarray(inputs["ln1_w"][l], f32)
        ln1b = np.asarray(inputs["ln1_b"][l], f32)
        aw = np.asarray(inputs["attn_w"][l], f32)
        ab = np.asarray(inputs["attn_b"][l], f32)
        awf = ln1w[:, None] * aw
        abf = ab + ln1b @ aw
        assert not np.any(abf), "nonzero attn bias not supported"
        qk = awf[:, :1536]
        s_qk = _pow2_scale(float(np.abs(qk).max()))
        qh, ql = _hi_lo(qk, s_qk)

        def qk_layout(a):
            return np.ascontiguousarray(
                a.reshape(KP, 2, 128, 12, 128).transpose(3, 2, 0, 1, 4)
            ).reshape(12 * 128, KP, 2, 128)
        common[f"qkwh{l}"] = qk_layout(qh)
        common[f"qkwl{l}"] = qk_layout(ql)
        sc[f"dq_qk{l}"] = 1.0 / (SX * s_qk)
        vw = awf[:, 1536:]
        s_v = _pow2_scale(float(np.abs(vw).max()))
        vh, vl = _hi_lo(vw, s_v)

        def v_layout(a):
            return np.ascontiguousarray(
                a.reshape(KP, 2, 128, C).transpose(2, 0, 1, 3))
        common[f"vwh{l}"] = v_layout(vh)
        common[f"vwl{l}"] = v_layout(vl)
        sc[f"dq_v{l}"] = 1.0 / (SX * s_v)

        pw = np.asarray(inputs["proj_w"][l], f32)
        assert not np.any(np.asarray(inputs["proj_b"][l])), "proj bias"
        s_p = _pow2_scale(float(np.abs(pw).max()))
        ph_, pl_ = _hi_lo(pw, s_p)

        def p_layout(a):
            return np.ascontiguousarray(
                a.reshape(KP, 2, 128, CT, 128).transpose(3, 2, 0, 1, 4)
            ).reshape(CT * 128, KP, 2, 128)
        common[f"pwh{l}"] = p_layout(ph_)
        common[f"pwl{l}"] = p_layout(pl_)
        sc[f"dq_p{l}"] = 1.0 / (SY * s_p)

        ln2w = np.asarray(inputs["ln2_w"][l], f32)
        ln2b = np.asarray(inputs["ln2_b"][l], f32)
        fw = np.asarray(inputs["fc_w"][l], f32)
        fbv = np.asarray(inputs["fc_b"][l], f32)
        fwf = ln2w[:, None] * fw
        fbf = fbv + ln2b @ fw
        assert not np.any(fbf), "nonzero fc bias not supported"
        s_fc = _pow2_scale(float(np.abs(fwf).max()))
        fh, fl = _hi_lo(fwf, s_fc)

        def f_layout(a):
            return np.ascontiguousarray(
                a.reshape(KP, 2, 128, HT, 128).transpose(3, 2, 0, 1, 4)
            ).reshape(HT * 128, KP, 2, 128)
        common[f"fwh{l}"] = f_layout(fh)
        common[f"fwl{l}"] = f_layout(fl)
        sc[f"dq_fc{l}"] = 1.0 / (SX * s_fc)

        f2w = np.asarray(inputs["fc2_w"][l], f32)
        assert not np.any(np.asarray(inputs["fc2_b"][l])), "fc2 bias"
        s_f2 = _pow2_scale(float(np.abs(f2w).max()))
        f2h, f2l = _hi_lo(f2w, s_f2)

        def f2_layout(a):
            return np.ascontiguousarray(
                a.reshape(HP, 2, 128, CT, 128).transpose(3, 2, 0, 1, 4)
            ).reshape(CT * 128, HP, 2, 128)
        common[f"f2wh{l}"] = f2_layout(f2h)
        common[f"f2wl{l}"] = f2_layout(f2l)
        sc[f"dq_f2{l}"] = 1.0 / (SH * s_f2)

    p = np.arange(128)[:, None]
    f = np.arange(AQ)[None, :]
    masks = np.zeros((128, 2, AQ), f32)
    masks[:, 0, :] = np.where(p > f, -240.0, 0.0)
    masks[:, 1, :] = np.where(p + 128 > f, -240.0, 0.0)
    common["masks"] = masks.astype(E4)
    iden = np.zeros((128, 2, 128), f32)
    iden[:, 0, :] = 128.0 * np.eye(128)
    common["iden"] = iden.astype(E4)

    lnfw = np.asarray(inputs["lnf_w"], f32)
    lnfb = np.asarray(inputs["lnf_b"], f32)
    assert not np.any(lnfb @ wte.T), "nonzero head bias not supported"
    wh = lnfw[:, None] * wte.T                     # [768, V]
    whp = np.zeros((C, VS2), f32)
    whp[:, :V] = wh
    m = float(np.abs(wh).max())
    s_h = float(2.0 ** np.floor(np.log2(240.0 / m)))
    sc["dq_h"] = 1.0 / (s_h * SX)

    slh = whp * s_h
    hih = slh.astype(E4)
    loh = (slh - hih.astype(f32)).astype(E4)
    common["whh"] = np.ascontiguousarray(
        hih.reshape(KP, 2, 128, NVC, 512).transpose(3, 2, 0, 1, 4)
    ).reshape(NVC * 128, KP, 2, 512)
    common["whl"] = np.ascontiguousarray(
        loh.reshape(KP, 2, 128, NVC, 512).transpose(3, 2, 0, 1, 4)
    ).reshape(NVC * 128, KP, 2, 512)

    def t6(a):          # [768, TO] -> [128, 6, TO]
        return np.ascontiguousarray(
            a.reshape(CT, 128, a.shape[1]).transpose(1, 0, 2))

    x0 = wte[idx] + wpe[None, :T]                  # [B, T, C]
    in_maps = []
    for c in range(8):
        s, half = c & 3, c >> 2
        bA, bB = OWN_BLOCKS[half]
        m2 = dict(common)
        xo = np.concatenate([x0[s, bA * AQ:(bA + 1) * AQ],
                             x0[s, bB * AQ:(bB + 1) * AQ]], axis=0)
        m2["x0t"] = t6(np.ascontiguousarray(xo.T))
        # RS staging masks: slot s2 carries my data iff my rank != s2
        mm = np.zeros((128, 2), f32)
        mm[:, 1 - half] = 1.0
        m2["mm"] = mm
        # data masks: mA for qA x remA ; mB for qB x remB
        m2d = np.zeros((128, 2, 2, AQ), f32)
        if half == 0:
            m2d[:, 0, :, :] = -240.0        # qA(blk0) x rA(blk1): future
            # qB(blk3) x rB(blk2): full attend -> 0
        else:
            # qA(blk1) x rA(blk0): full attend -> 0
            m2d[:, 1, :, :] = -240.0        # qB(blk2) x rB(blk3): future
        m2["masks2"] = m2d.astype(E4)
        in_maps.append(m2)
    return in_maps, sc


def kernel(**inputs):
    global LAST_RESULT, LAST_NC
    n_layers = L
    in_maps, sc = _prep_inputs(inputs, n_layers)
    key = (n_layers, tuple(sorted(sc.items())))
    if key not in _CACHE:
        _CACHE[key] = build_program(sc, n_layers)
    nc = _CACHE[key]
    LAST_NC = nc
    res = run_bass_kernel_spmd(nc, in_maps, core_ids=list(range(8)))
    LAST_RESULT = res
    out = np.empty((B, T, V), np.float32)
    for c in range(8):
        s, half = c & 3, c >> 2
        bA, bB = OWN_BLOCKS[half]
        part = np.asarray(res.results[c]["logits"]).astype(np.float32)
        out[s, bA * AQ:(bA + 1) * AQ] = part[0:AQ, :V]
        out[s, bB * AQ:(bB + 1) * AQ] = part[AQ:2 * AQ, :V]
    return out


if __name__ == "__main__":
    rng = np.random.default_rng(0)
    ins = {
        "idx": rng.integers(0, V, (B, T)).astype(np.int32),
        "wte": (rng.standard_normal((V, C)) * 0.02).astype(np.float32),
        "wpe": (rng.standard_normal((T, C)) * 0.02).astype(np.float32),
        "ln1_w": np.ones((L, C), np.float32),
        "ln1_b": np.zeros((L, C), np.float32),
        "attn_w": (rng.standard_normal((L, C, 3 * C)) * 0.02).astype(np.float32),
        "attn_b": np.zeros((L, 3 * C), np.float32),
        "proj_w": (rng.standard_normal((L, C, C)) * 0.02).astype(np.float32),
        "proj_b": np.zeros((L, C), np.float32),
        "ln2_w": np.ones((L, C), np.float32),
        "ln2_b": np.zeros((L, C), np.float32),
        "fc_w": (rng.standard_normal((L, C, 4 * C)) * 0.02).astype(np.float32),
        "fc_b": np.zeros((L, 4 * C), np.float32),
        "fc2_w": (rng.standard_normal((L, 4 * C, C)) * 0.02).astype(np.float32),
        "fc2_b": np.zeros((L, C), np.float32),
        "lnf_w": np.ones((C,), np.float32),
        "lnf_b": np.zeros((C,), np.float32),
    }
    out = kernel(**ins)
    print("out", out.shape, out.dtype, float(np.abs(out).max()))


# revision 20
# speedup vs baseline: 1.2368x; 1.0052x over previous
"""MiniGPT forward on 8 Trainium2 NeuronCores — sequence-split variant.

Core c: sequence (c & 3), token-half (c >> 2).  In AQ=256 blocks of the
1024-token sequence, half 0 owns blocks {0, 3}, half 1 owns {1, 2} (equal
causal-attention load).  Each core runs the 6 transformer blocks for its
OWN 512 tokens only — no duplicated block compute.  Per layer the pair
exchanges K and V via two ReduceScatters (k first): each core stages
[k*m0, k*m1] with per-core {0,1} masks so the RS output is exactly the
PEER's k/v at a uniform address (SPMD-safe, no rank branching).
Attention = own-block phase (overlaps the RS) + remote phase; causal /
validity masks are per-core input data applied on the PE via the
iden @ mask DoubleRow trick.  One remote block per core is fully masked
waste, keeping the instruction stream identical across cores.

Block linears are fp8(e4m3) DoubleRow 3-term (Wh@xh + Wh@xl + Wl@xh,
0.75x bf16 PE cost); attention stays bf16.  The LM head is token-split:
each core computes its own 512 tokens x the FULL vocab with the same
3-term fp8 scheme, logits stream out in bf16.
"""

import sys

sys.path.insert(0, "/opt/trn_rl_repo")

import numpy as np
import ml_dtypes

import concourse.bacc as bacc
import concourse.tile as tile
from concourse import mybir
from concourse.bass_utils import run_bass_kernel_spmd

F32 = mybir.dt.float32
F32R = mybir.dt.float32r
BF16 = mybir.dt.bfloat16
FP8 = mybir.dt.float8e4
ALU = mybir.AluOpType
ACT = mybir.ActivationFunctionType
DR = mybir.MatmulPerfMode.DoubleRow
E4 = ml_dtypes.float8_e4m3

B, T, C, H, HD, L, V = 4, 1024, 768, 12, 64, 6, 50257
TO = 512                # own tokens per core
CT = C // 128           # 6 c-tiles
KP = CT // 2            # 3 k-pairs
TT = TO // 128          # 4 own token tiles
AQ = 256                # attention query block
HT = 3072 // 128        # 24 hidden tiles
HP = HT // 2            # 12 hidden k-pairs
VS2 = 51200             # padded vocab
NVC = VS2 // 512        # 100
EPS = 1e-5
SX = 16.0               # fp8 scale for LN outputs (blocks + head)
SY = 32.0               # fp8 scale for attention output y (folded in ones)
SH = 1.0                # gelu output used unscaled in fp8

RG = [[0, 4], [1, 5], [2, 6], [3, 7]]   # pair replica groups

_CACHE = {}
LAST_RESULT = None
LAST_NC = None


def build_program(sc, n_layers=L):
    nc = bacc.Bacc(None, target_bir_lowering=False)

    def f8_in(name, shape):
        return nc.dram_tensor(name, list(shape), FP8, kind="ExternalInput")

    x0t_d = nc.dram_tensor("x0t", [128, CT, TO], F32R, kind="ExternalInput")
    wqh, wql, wvh, wvl, wph, wpl = [], [], [], [], [], []
    wfh, wfl, w2h, w2l = [], [], [], []
    for l in range(n_layers):
        wqh.append(f8_in(f"qkwh{l}", (12 * 128, KP, 2, 128)))
        wql.append(f8_in(f"qkwl{l}", (12 * 128, KP, 2, 128)))
        wvh.append(f8_in(f"vwh{l}", (128, KP, 2, C)))
        wvl.append(f8_in(f"vwl{l}", (128, KP, 2, C)))
        wph.append(f8_in(f"pwh{l}", (CT * 128, KP, 2, 128)))
        wpl.append(f8_in(f"pwl{l}", (CT * 128, KP, 2, 128)))
        wfh.append(f8_in(f"fwh{l}", (HT * 128, KP, 2, 128)))
        wfl.append(f8_in(f"fwl{l}", (HT * 128, KP, 2, 128)))
        w2h.append(f8_in(f"f2wh{l}", (CT * 128, HP, 2, 128)))
        w2l.append(f8_in(f"f2wl{l}", (CT * 128, HP, 2, 128)))
    whh_d = f8_in("whh", (NVC * 128, KP, 2, 512))
    whl_d = f8_in("whl", (NVC * 128, KP, 2, 512))
    masks_d = f8_in("masks", (128, 2, AQ))          # local diag (shared)
    masks2_d = f8_in("masks2", (128, 2, 2, AQ))     # [mA|mB, half, q] per-core
    iden_d = f8_in("iden", (128, 2, 128))
    mm_d = nc.dram_tensor("mm", [128, 2], F32, kind="ExternalInput")
    logits_d = nc.dram_tensor("logits", [TO, VS2], BF16,
                              kind="ExternalOutput")

    ln_ctr = [0]

    with nc.allow_low_precision("fp8 3-term error-feedback intentional"), \
         tile.TileContext(nc) as tc:
        glob = tc.alloc_tile_pool(name="glob", bufs=1)
        gx = tc.alloc_tile_pool(name="gx", bufs=1)
        gx2 = tc.alloc_tile_pool(name="gx2", bufs=3)
        gu = tc.alloc_tile_pool(name="gu", bufs=3)
        gmicro = tc.alloc_tile_pool(name="gmicro", bufs=1)
        ps_big = tc.alloc_tile_pool(name="ps_big", bufs=4, space="PSUM")
        gxn = tc.alloc_tile_pool(name="gxn", bufs=1)
        gw = tc.alloc_tile_pool(name="gw", bufs=8)
        gwv = tc.alloc_tile_pool(name="gwv", bufs=1)
        gw24 = tc.alloc_tile_pool(name="gw24", bufs=3)
        gkv = tc.alloc_tile_pool(name="gkv", bufs=1)
        gst = tc.alloc_tile_pool(name="gst", bufs=1)
        dram = tc.alloc_tile_pool(name="dram", bufs=2, space="DRAM")

        ones_col = glob.tile([128, 1], F32R, tag="ones_col")
        ones_row = glob.tile([1, 128], F32R, tag="ones_row")
        epsh_t = glob.tile([1, 1], F32, tag="epsh")
        masks_t = glob.tile([128, 2, AQ], FP8, tag="masks")
        masks2_t = glob.tile([128, 2, 2, AQ], FP8, tag="masks2")
        iden_t = glob.tile([128, 2, 128], FP8, tag="iden")
        mm_t = glob.tile([128, 2], F32, tag="mm")
        nc.vector.memset(ones_col[:].bitcast(F32), 1.0)
        nc.vector.memset(ones_row[:].bitcast(F32), 1.0)
        nc.vector.memset(epsh_t[:], EPS / (SX * SX))
        nc.sync.dma_start(out=masks_t[:], in_=masks_d[:])
        nc.sync.dma_start(out=masks2_t[:], in_=masks2_d[:])
        nc.sync.dma_start(out=iden_t[:], in_=iden_d[:])
        nc.sync.dma_start(out=mm_t[:], in_=mm_d[:])

        xT = gx.tile([128, CT, TO], F32R, tag="xT")
        nc.sync.dma_start(out=xT[:], in_=x0t_d[:])

        # persistent v tiles: [p, tt, h, 0:64] = v ; [.., 64:128] = 1/SY
        gv = tc.alloc_tile_pool(name="gv", bufs=1)
        vOwn = gv.tile([128, TT, H, 128], BF16, tag="vOwn")
        vRem = gv.tile([128, TT, H, 128], BF16, tag="vRem")
        kRem = gkv.tile([128, CT, TO], BF16, tag="kRem")
        nc.gpsimd.memset(vOwn[:, :, :, 64:128], 1.0 / SY)
        nc.gpsimd.memset(vRem[:, :, :, 64:128], 1.0 / SY)

        def ln_full(xin, xh, xl, ps_stat, ps_bc):
            """(xh + xl) ~= SX * (xin - mu) * rstd in fp8, all 512 tokens."""
            qs = slice(0, TO)
            s_ps = ps_stat.tile([1, TO], F32, space="PSUM", tag="stat")
            q_ps = ps_stat.tile([1, TO], F32, space="PSUM", tag="stat")
            for kt in range(CT):
                nc.tensor.matmul(s_ps[:], ones_col[:], xin[:, kt, qs],
                                 start=(kt == 0), stop=(kt == CT - 1))
            for kt in range(CT):
                x2 = gx2.tile([128, TO], F32R, tag="x2")
                nc.gpsimd.tensor_tensor(
                    out=x2[:], in0=xin[:, kt, qs],
                    in1=xin[:, kt, qs], op=ALU.mult)
                nc.tensor.matmul(q_ps[:], ones_col[:], x2[:],
                                 start=(kt == 0), stop=(kt == CT - 1))
            mu = gmicro.tile([1, TO], F32R, tag="mu")
            nc.scalar.mul(mu[:], s_ps[:], 1.0 / C)
            mu2 = gmicro.tile([1, TO], F32, tag="mu2")
            nc.scalar.activation(mu2[:], mu[:], ACT.Square)
            var = gmicro.tile([1, TO], F32, tag="var")
            nc.vector.scalar_tensor_tensor(
                out=var[:], in0=q_ps[:], scalar=1.0 / C, in1=mu2[:],
                op0=ALU.mult, op1=ALU.subtract)
            sd = gmicro.tile([1, TO], F32, tag="sd")
            nc.scalar.activation(sd[:], var[:], ACT.Sqrt, bias=epsh_t[:],
                                 scale=1.0 / (SX * SX))
            r = gmicro.tile([1, TO], F32R, tag="r")
            nc.vector.reciprocal(r[:], sd[:])            # SX/sd
            mr = gmicro.tile([1, TO], F32R, tag="mr")
            nc.vector.tensor_tensor(out=mr[:], in0=mu[:], in1=r[:],
                                    op=ALU.mult)
            bc = ps_bc.tile([128, TO], F32, space="PSUM", tag="bc")
            nc.tensor.matmul(bc[:], ones_row[:], r[:], start=True, stop=True)
            bc2 = ps_bc.tile([128, TO], F32, space="PSUM", tag="bc")
            nc.tensor.matmul(bc2[:], ones_row[:], mr[:], start=True,
                             stop=True)
            for kt in range(CT):
                t = gx2.tile([128, TO], F32, tag="lnt")
                nc.vector.tensor_tensor(out=t[:],
                                        in0=xin[:, kt, qs].bitcast(F32),
                                        in1=bc[:], op=ALU.mult)
                u = gu.tile([128, TO], F32, tag="lnu")
                nc.vector.tensor_tensor(out=u[:], in0=t[:],
                                        in1=bc2[:], op=ALU.subtract)
                nc.scalar.copy(xh[:, kt, :], u[:])
                nc.gpsimd.tensor_tensor(out=xl[:, kt, :], in0=u[:],
                                        in1=xh[:, kt, :],
                                        op=ALU.subtract)

        def ln_pools():
            i = ln_ctr[0]
            ln_ctr[0] += 1
            ps_stat = tc.alloc_tile_pool(name=f"ps_st{i}", bufs=2,
                                         space="PSUM")
            ps_bc = tc.alloc_tile_pool(name=f"ps_bc{i}", bufs=2, space="PSUM")
            return ps_stat, ps_bc

        def dr_mm(ps, wh, wl, xh, xl, qs, kps):
            """accumulate 3-term fp8 DR: Wh@xh + Wh@xl + Wl@xh into ps."""
            n = 3 * kps
            i = 0
            for w_, x_ in ((wh, xh), (wh, xl), (wl, xh)):
                for kp in range(kps):
                    nc.tensor.matmul(ps, w_[:, kp, :, :],
                                     x_[:, 2 * kp:2 * kp + 2, qs],
                                     start=(i == 0), stop=(i == n - 1),
                                     perf_mode=DR)
                    i += 1

        def mask_mm(sps, half, msrc):
            nc.tensor.matmul(
                sps[:, half * AQ:(half + 1) * AQ],
                iden_t[:],
                msrc.rearrange("p (i q) -> p i q", i=1)
                .broadcast_to([128, 2, AQ]),
                start=False, stop=True, perf_mode=DR,
                skip_group_check=True)

        def score_block(hp, rows, ksrc, kbase, qsl, msrc):
            """one AQ x AQ*?? score block: k tiles (kbase, kbase+1) of ksrc
            vs q columns qsl; optional additive mask (None | AP)."""
            sps = ps_sc.tile([128, 2 * AQ], F32, space="PSUM", tag="sc",
                             name="sps")
            for half in range(2):
                kt = kbase + half
                nc.tensor.matmul(
                    sps[:, half * AQ:(half + 1) * AQ],
                    ksrc[rows, 6 + hp, kt * 128:(kt + 1) * 128]
                    if ksrc is qkT else
                    ksrc[rows, hp, kt * 128:(kt + 1) * 128],
                    qkT[rows, hp, qsl],
                    start=True, stop=(msrc is None),
                    skip_group_check=(msrc is not None))
                if msrc is not None:
                    mask_mm(sps, half, msrc[:, half, :])
            e = pE.tile([128, 2 * AQ], BF16, tag="E", name="e")
            nc.scalar.activation(e[:], sps[:], ACT.Exp, scale=0.125)
            return e

        for l in range(n_layers):
            pa = tc.alloc_tile_pool(name=f"pa{l}", bufs=1)

            # ---- LN1 -> xn8 h/l (fp8) ----
            xnh = gxn.tile([128, CT, TO], FP8, tag="xnh")
            xnl = gxn.tile([128, CT, TO], FP8, tag="xnl")
            qkT = pa.tile([128, 12, TO], BF16, tag="qkT")
            st1, bc1 = ln_pools()
            ln_full(xT, xnh, xnl, st1, bc1)

            # ---- k projections first so the k-RS starts early ----
            def qk_slots(slots):
                for s_ in slots:
                    wh = gw.tile([128, KP, 2, 128], FP8, tag="w6h",
                                 name="wh")
                    nc.sync.dma_start(
                        out=wh[:], in_=wqh[l].ap()[s_ * 128:(s_ + 1) * 128])
                    wl = gw.tile([128, KP, 2, 128], FP8, tag="w6l",
                                 name="wl")
                    nc.sync.dma_start(
                        out=wl[:], in_=wql[l].ap()[s_ * 128:(s_ + 1) * 128])
                    ps = ps_big.tile([128, TO], F32, space="PSUM", tag="px",
                                     name="ps")
                    dr_mm(ps[:], wh, wl, xnh, xnl, slice(0, TO), KP)
                    if s_ >= 6:
                        nc.scalar.mul(qkT[:, s_, :], ps[:],
                                      sc[f"dq_qk{l}"])
                    else:
                        nc.vector.tensor_scalar_mul(qkT[:, s_, :], ps[:],
                                                    sc[f"dq_qk{l}"])

            qk_slots(range(6, 12))
            qk_slots(range(6))
            for p in (bc1, st1):
                p.release()

            # ---- k export: stage [k*m0, k*m1], RS, import peer k ----
            kst = gst.tile([128, 2, CT, TO], BF16, tag="kst")
            for sl in range(2):
                nc.vector.tensor_scalar_mul(kst[:, sl], qkT[:, 6:12, :],
                                            mm_t[:, sl:sl + 1])
            k_in = dram.tile([2, 128, CT, TO], BF16, tag="k_in")
            k_out = dram.tile([128, CT, TO], BF16, tag="k_out")
            nc.sync.dma_start(
                out=k_in[:].rearrange("a p c t -> p a c t"), in_=kst[:])
            nc.gpsimd.collective_compute(
                "ReduceScatter", ALU.add, replica_groups=RG,
                ins=[k_in.opt()], outs=[k_out.opt()])
            nc.sync.dma_start(out=kRem[:], in_=k_out[:])

            # ---- v projection (tokens on PSUM partitions) ----
            wvht = gwv.tile([128, KP, 2, C], FP8, tag="vwh")
            nc.sync.dma_start(out=wvht[:], in_=wvh[l].ap()[:])
            wvlt = gwv.tile([128, KP, 2, C], FP8, tag="vwl")
            nc.sync.dma_start(out=wvlt[:], in_=wvl[l].ap()[:])
            for tt in range(TT):
                tsl = slice(tt * 128, (tt + 1) * 128)
                psA = ps_big.tile([128, 512], F32, space="PSUM", tag="px")
                psB = ps_big.tile([128, 256], F32, space="PSUM", tag="px")
                n = 3 * KP
                for ps_, csl in ((psA, slice(0, 512)),
                                 (psB, slice(512, 768))):
                    i = 0
                    for x_, w_ in ((xnh, wvht), (xnl, wvht), (xnh, wvlt)):
                        for kp in range(KP):
                            nc.tensor.matmul(ps_[:],
                                             x_[:, 2 * kp:2 * kp + 2, tsl],
                                             w_[:, kp, :, csl],
                                             start=(i == 0),
                                             stop=(i == n - 1),
                                             perf_mode=DR)
                            i += 1
                nc.vector.tensor_scalar_mul(
                    vOwn[:, tt, 0:8, 0:64],
                    psA[:].rearrange("p (h d) -> p h d", h=8),
                    sc[f"dq_v{l}"])
                nc.vector.tensor_scalar_mul(
                    vOwn[:, tt, 8:12, 0:64],
                    psB[:].rearrange("p (h d) -> p h d", h=4),
                    sc[f"dq_v{l}"])

            # ---- v export ----
            vst = gst.tile([128, 2, TT, H, 64], BF16, tag="vst")
            for sl in range(2):
                nc.vector.tensor_scalar_mul(vst[:, sl],
                                            vOwn[:, :, :, 0:64],
                                            mm_t[:, sl:sl + 1])
            v_in = dram.tile([2, 128, TT, H, 64], BF16, tag="v_in")
            v_out = dram.tile([128, TT, H, 64], BF16, tag="v_out")
            nc.sync.dma_start(
                out=v_in[:].rearrange("a p t h d -> p a t h d"), in_=vst[:])
            nc.gpsimd.collective_compute(
                "ReduceScatter", ALU.add, replica_groups=RG,
                ins=[v_in.opt()], outs=[v_out.opt()])
            nc.sync.dma_start(out=vRem[:, :, :, 0:64], in_=v_out[:])

            # ---- attention ----
            yT = pa.tile([128, CT, TO], BF16, tag="yT")
            pE = tc.alloc_tile_pool(name=f"pE{l}", bufs=10)
            prec = tc.alloc_tile_pool(name=f"prec{l}", bufs=4)
            gyo = tc.alloc_tile_pool(name=f"gyo{l}", bufs=1)
            ps_sc = tc.alloc_tile_pool(name=f"ps_sc{l}", bufs=2, space="PSUM")
            ps_av = tc.alloc_tile_pool(name=f"ps_av{l}", bufs=2, space="PSUM")
            yo = gyo.tile([128, 12, 2, AQ], F32, tag="yo")  # own partials

            qA, qB = slice(0, AQ), slice(AQ, 2 * AQ)
            # phase 1: own blocks (independent of the RS)
            for hp in range(6):
                for h in (2 * hp, 2 * hp + 1):
                    par = h % 2
                    rows = slice(64 * par, 64 * par + 64)
                    eA = score_block(hp, rows, qkT, 0, qA, masks_t)
                    eB1 = score_block(hp, rows, qkT, 0, qB, None)
                    eB2 = score_block(hp, rows, qkT, 2, qB, masks_t)
                    ya = ps_av.tile([128, AQ], F32, space="PSUM", tag="av")
                    for kt in range(2):
                        nc.tensor.matmul(ya[:], vOwn[:, kt, h, :],
                                         eA[:, kt * AQ:(kt + 1) * AQ],
                                         start=(kt == 0), stop=(kt == 1))
                    nc.vector.tensor_copy(out=yo[:, h, 0, :], in_=ya[:])
                    yb = ps_av.tile([128, AQ], F32, space="PSUM", tag="av")
                    for kt in range(4):
                        e = eB1 if kt < 2 else eB2
                        nc.tensor.matmul(yb[:], vOwn[:, kt, h, :],
                                         e[:, (kt % 2) * AQ:(kt % 2 + 1) * AQ],
                                         start=(kt == 0), stop=(kt == 3))
                    nc.vector.tensor_copy(out=yo[:, h, 1, :], in_=yb[:])
            # phase 2: remote blocks + combine
            for hp in range(6):
                for h in (2 * hp, 2 * hp + 1):
                    par = h % 2
                    rows = slice(64 * par, 64 * par + 64)
                    eAr = score_block(hp, rows, kRem, 0, qA, masks2_t[:, 0])
                    eBr1 = score_block(hp, rows, kRem, 0, qB, None)
                    eBr2 = score_block(hp, rows, kRem, 2, qB,
                                       masks2_t[:, 1])
                    for qi, es in ((0, (eAr, eAr)), (1, (eBr1, eBr2))):
                        nkt = 2 if qi == 0 else 4
                        yr = ps_av.tile([128, AQ], F32, space="PSUM",
                                        tag="av")
                        for kt in range(nkt):
                            e = es[0] if kt < 2 else es[1]
                            nc.tensor.matmul(
                                yr[:], vRem[:, kt, h, :],
                                e[:, (kt % 2) * AQ:(kt % 2 + 1) * AQ],
                                start=(kt == 0), stop=(kt == nkt - 1))
                        yc = prec.tile([128, AQ], F32, tag="yc")
                        nc.vector.tensor_tensor(out=yc[:], in0=yr[:],
                                                in1=yo[:, h, qi, :],
                                                op=ALU.add)
                        rec = prec.tile([64, AQ], F32, tag="rec")
                        nc.vector.reciprocal(rec[:], yc[64:128, :])
                        qsl = slice(qi * AQ, (qi + 1) * AQ)
                        yrow = slice(64 * par, 64 * par + 64)
                        nc.vector.tensor_tensor(out=yT[yrow, hp, qsl],
                                                in0=yc[0:64, :], in1=rec[:],
                                                op=ALU.mult)
            for p in (ps_av, ps_sc, gyo, prec, pE):
                p.release()

            # ---- y8 split, proj + residual, LN2 + MLP ----
            py8 = tc.alloc_tile_pool(name=f"py8{l}", bufs=1)
            ph8 = tc.alloc_tile_pool(name=f"ph8{l}", bufs=1)
            phbf = tc.alloc_tile_pool(name=f"phbf{l}", bufs=3)
            y8h = py8.tile([128, CT, TO], FP8, tag="y8h")
            y8l = py8.tile([128, CT, TO], FP8, tag="y8l")
            for kt in range(CT):
                nc.gpsimd.tensor_copy(out=y8h[:, kt, :], in_=yT[:, kt, :])
                nc.gpsimd.tensor_tensor(
                    out=y8l[:, kt, :], in0=yT[:, kt, :],
                    in1=y8h[:, kt, :], op=ALU.subtract)
            for ot in range(CT):
                wh = gw.tile([128, KP, 2, 128], FP8, tag="w6h")
                nc.sync.dma_start(out=wh[:],
                                  in_=wph[l].ap()[ot * 128:(ot + 1) * 128])
                wl = gw.tile([128, KP, 2, 128], FP8, tag="w6l")
                nc.sync.dma_start(out=wl[:],
                                  in_=wpl[l].ap()[ot * 128:(ot + 1) * 128])
                ps = ps_big.tile([128, TO], F32, space="PSUM", tag="px")
                dr_mm(ps[:], wh, wl, y8h, y8l, slice(0, TO), KP)
                nc.vector.scalar_tensor_tensor(
                    out=xT[:, ot, :], in0=ps[:], scalar=sc[f"dq_p{l}"],
                    in1=xT[:, ot, :], op0=ALU.mult, op1=ALU.add)
            xn2h = gxn.tile([128, CT, TO], FP8, tag="xnh")
            xn2l = gxn.tile([128, CT, TO], FP8, tag="xnl")
            st2, bc2p = ln_pools()
            ln_full(xT, xn2h, xn2l, st2, bc2p)
            h8h = ph8.tile([128, HT, TO], FP8, tag="h8h")
            h8l = ph8.tile([128, HT, TO], FP8, tag="h8l")
            for ot in range(HT):
                wh = gw.tile([128, KP, 2, 128], FP8, tag="w6h")
                nc.sync.dma_start(out=wh[:],
                                  in_=wfh[l].ap()[ot * 128:(ot + 1) * 128])
                wl = gw.tile([128, KP, 2, 128], FP8, tag="w6l")
                nc.sync.dma_start(out=wl[:],
                                  in_=wfl[l].ap()[ot * 128:(ot + 1) * 128])
                ps = ps_big.tile([128, TO], F32, space="PSUM", tag="px")
                dr_mm(ps[:], wh, wl, xn2h, xn2l, slice(0, TO), KP)
                hbf = phbf.tile([128, TO], BF16, tag="hbf")
                nc.scalar.activation(h8h[:, ot, :], ps[:], ACT.Gelu,
                                     scale=sc[f"dq_fc{l}"])
                nc.scalar.activation(hbf[:], ps[:], ACT.Gelu,
                                     scale=sc[f"dq_fc{l}"])
                nc.gpsimd.tensor_tensor(
                    out=h8l[:, ot, :], in0=hbf[:],
                    in1=h8h[:, ot, :], op=ALU.subtract)
            for ot in range(CT):
                wh = gw24.tile([128, HP, 2, 128], FP8, tag="w24h")
                nc.sync.dma_start(out=wh[:],
                                  in_=w2h[l].ap()[ot * 128:(ot + 1) * 128])
                wl = gw24.tile([128, HP, 2, 128], FP8, tag="w24l")
                nc.sync.dma_start(out=wl[:],
                                  in_=w2l[l].ap()[ot * 128:(ot + 1) * 128])
                ps = ps_big.tile([128, TO], F32, space="PSUM", tag="px")
                dr_mm(ps[:], wh, wl, h8h, h8l, slice(0, TO), HP)
                nc.vector.scalar_tensor_tensor(
                    out=xT[:, ot, :], in0=ps[:], scalar=sc[f"dq_f2{l}"],
                    in1=xT[:, ot, :], op0=ALU.mult, op1=ALU.add)
            for p in (bc2p, st2, phbf, ph8, py8, pa):
                p.release()

        # ---- final LN (fp8 h/l, scale SX) + token-split LM head ----
        for p in (gv, dram, gst, gkv, gw24, gwv, gw, gxn):
            p.release()
        pf = tc.alloc_tile_pool(name="pf", bufs=1)
        xf = pf.tile([128, CT, TO], FP8, tag="xf")
        xl = pf.tile([128, CT, TO], FP8, tag="xl")
        stf, bcf = ln_pools()
        ln_full(xT, xf, xl, stf, bcf)
        for p in (bcf, stf):
            p.release()

        ph = tc.alloc_tile_pool(name="ph", bufs=8)
        pout = tc.alloc_tile_pool(name="pout", bufs=4)
        for vc in range(NVC):
            wh8 = ph.tile([128, KP, 2, 512], FP8, tag="wh")
            nc.sync.dma_start(out=wh8[:],
                              in_=whh_d.ap()[vc * 128:(vc + 1) * 128])
            wl8 = ph.tile([128, KP, 2, 512], FP8, tag="whl")
            nc.sync.dma_start(out=wl8[:],
                              in_=whl_d.ap()[vc * 128:(vc + 1) * 128])
            o = pout.tile([128, TT, 512], BF16, tag="out")
            for tt in range(TT):
                tsl = slice(tt * 128, (tt + 1) * 128)
                ps = ps_big.tile([128, 512], F32, space="PSUM", tag="px")
                terms = [(xf, wh8), (xl, wh8), (xf, wl8)]
                for cc in range(2):
                    i = 0
                    for xsrc, wsrc in terms:
                        for kp in range(KP):
                            nc.tensor.matmul(
                                ps[:, cc * 256:(cc + 1) * 256],
                                xsrc[:, 2 * kp:2 * kp + 2, tsl],
                                wsrc[:, kp, :, cc * 256:cc * 256 + 256],
                                start=(i == 0), stop=(i == 3 * KP - 1),
                                perf_mode=DR)
                            i += 1
                if tt % 2 == 0:
                    nc.vector.tensor_scalar_mul(o[:, tt, :], ps[:],
                                                sc["dq_h"])
                else:
                    nc.scalar.mul(o[:, tt, :], ps[:], sc["dq_h"])
            nc.sync.dma_start(
                out=logits_d.ap()[:, vc * 512:(vc + 1) * 512].rearrange(
                    "(t p) v -> p t v", p=128),
                in_=o[:])
        for p in (pout, ph, pf, ps_big, gmicro, gu, gx2, gx, glob):
            p.release()

    nc.compile()
    return nc

# ---------------------------------------------------------------------------
# host side
# ---------------------------------------------------------------------------

def _pow2_scale(m, target=224.0):
    if m == 0:
        return 1.0
    return float(2.0 ** np.floor(np.log2(target / m)))


def _hi_lo(w, s):
    ws = w * s
    hi = ws.astype(E4)
    lo = (ws - hi.astype(np.float32)).astype(E4)
    return hi, lo


# own global AQ-blocks per half, in local order
OWN_BLOCKS = {0: (0, 3), 1: (1, 2)}


def _prep_inputs(inputs, n_layers):
    f32 = np.float32
    idx = np.asarray(inputs["idx"])
    wte = np.asarray(inputs["wte"], f32)
    wpe = np.asarray(inputs["wpe"], f32)

    sc = {}
    common = {}
    for l in range(n_layers):
        ln1w = np.asarray(inputs["ln1_w"][l], f32)
        ln1b = np.asarray(inputs["ln1_b"][l], f32)
        aw = np.asarray(inputs["attn_w"][l], f32)
        ab = np.asarray(inputs["attn_b"][l], f32)
        awf = ln1w[:, None] * aw
        abf = ab + ln1b @ aw
        assert not np.any(abf), "nonzero attn bias not supported"
        qk = awf[:, :1536]
        s_qk = _pow2_scale(float(np.abs(qk).max()))
        qh, ql = _hi_lo(qk, s_qk)

        def qk_layout(a):
            return np.ascontiguousarray(
                a.reshape(KP, 2, 128, 12, 128).transpose(3, 2, 0, 1, 4)
            ).reshape(12 * 128, KP, 2, 128)
        common[f"qkwh{l}"] = qk_layout(qh)
        common[f"qkwl{l}"] = qk_layout(ql)
        sc[f"dq_qk{l}"] = 1.0 / (SX * s_qk)
        vw = awf[:, 1536:]
        s_v = _pow2_scale(float(np.abs(vw).max()))
        vh, vl = _hi_lo(vw, s_v)

        def v_layout(a):
            return np.ascontiguousarray(
                a.reshape(KP, 2, 128, C).transpose(2, 0, 1, 3))
        common[f"vwh{l}"] = v_layout(vh)
        common[f"vwl{l}"] = v_layout(vl)
        sc[f"dq_v{l}"] = 1.0 / (SX * s_v)

        pw = np.asarray(inputs["proj_w"][l], f32)
        assert not np.any(np.asarray(inputs["proj_b"][l])), "proj bias"
        s_p = _pow2_scale(float(np.abs(pw).max()))
        ph_, pl_ = _hi_lo(pw, s_p)

        def p_layout(a):
            return np.ascontiguousarray(
                a.reshape(KP, 2, 128, CT, 128).transpose(3, 2, 0, 1, 4)
            ).reshape(CT * 128, KP, 2, 128)
        common[f"pwh{l}"] = p_layout(ph_)
        common[f"pwl{l}"] = p_layout(pl_)
        sc[f"dq_p{l}"] = 1.0 / (SY * s_p)

        ln2w = np.asarray(inputs["ln2_w"][l], f32)
        ln2b = np.asarray(inputs["ln2_b"][l], f32)
        fw = np.asarray(inputs["fc_w"][l], f32)
        fbv = np.asarray(inputs["fc_b"][l], f32)
        fwf = ln2w[:, None] * fw
        fbf = fbv + ln2b @ fw
        assert not np.any(fbf), "nonzero fc bias not supported"
        s_fc = _pow2_scale(float(np.abs(fwf).max()))
        fh, fl = _hi_lo(fwf, s_fc)

        def f_layout(a):
            return np.ascontiguousarray(
                a.reshape(KP, 2, 128, HT, 128).transpose(3, 2, 0, 1, 4)
            ).reshape(HT * 128, KP, 2, 128)
        common[f"fwh{l}"] = f_layout(fh)
        common[f"fwl{l}"] = f_layout(fl)
        sc[f"dq_fc{l}"] = 1.0 / (SX * s_fc)

        f2w = np.asarray(inputs["fc2_w"][l], f32)
        assert not np.any(np.asarray(inputs["fc2_b"][l])), "fc2 bias"
        s_f2 = _pow2_scale(float(np.abs(f2w).max()))
        f2h, f2l = _hi_lo(f2w, s_f2)

        def f2_layout(a):
            return np.ascontiguousarray(
                a.reshape(HP, 2, 128, CT, 128).transpose(3, 2, 0, 1, 4)
            ).reshape(CT * 128, HP, 2, 128)
        common[f"f2wh{l}"] = f2_layout(f2h)
        common[f"f2wl{l}"] = f2_layout(f2l)
        sc[f"dq_f2{l}"] = 1.0 / (SH * s_f2)

    p = np.arange(128)[:, None]
    f = np.arange(AQ)[None, :]
    masks = np.zeros((128, 2, AQ), f32)
    masks[:, 0, :] = np.where(p > f, -240.0, 0.0)
    masks[:, 1, :] = np.where(p + 128 > f, -240.0, 0.0)
    common["masks"] = masks.astype(E4)
    iden = np.zeros((128, 2, 128), f32)
    iden[:, 0, :] = 128.0 * np.eye(128)
    common["iden"] = iden.astype(E4)

    lnfw = np.asarray(inputs["lnf_w"], f32)
    lnfb = np.asarray(inputs["lnf_b"], f32)
    assert not np.any(lnfb @ wte.T), "nonzero head bias not supported"
    wh = lnfw[:, None] * wte.T                     # [768, V]
    whp = np.zeros((C, VS2), f32)
    whp[:, :V] = wh
    m = float(np.abs(wh).max())
    s_h = float(2.0 ** np.floor(np.log2(240.0 / m)))
    sc["dq_h"] = 1.0 / (s_h * SX)

    slh = whp * s_h
    hih = slh.astype(E4)
    loh = (slh - hih.astype(f32)).astype(E4)
    common["whh"] = np.ascontiguousarray(
        hih.reshape(KP, 2, 128, NVC, 512).transpose(3, 2, 0, 1, 4)
    ).reshape(NVC * 128, KP, 2, 512)
    common["whl"] = np.ascontiguousarray(
        loh.reshape(KP, 2, 128, NVC, 512).transpose(3, 2, 0, 1, 4)
    ).reshape(NVC * 128, KP, 2, 512)

    def t6(a):          # [768, TO] -> [128, 6, TO]
        return np.ascontiguousarray(
            a.reshape(CT, 128, a.shape[1]).transpose(1, 0, 2))

    x0 = wte[idx] + wpe[None, :T]                  # [B, T, C]
    in_maps = []
    for c in range(8):
        s, half = c & 3, c >> 2
        bA, bB = OWN_BLOCKS[half]
        m2 = dict(common)
        xo = np.concatenate([x0[s, bA * AQ:(bA + 1) * AQ],
                             x0[s, bB * AQ:(bB + 1) * AQ]], axis=0)
        m2["x0t"] = t6(np.ascontiguousarray(xo.T))
        # RS staging masks: slot s2 carries my data iff my rank != s2
        mm = np.zeros((128, 2), f32)
        mm[:, 1 - half] = 1.0
        m2["mm"] = mm
        # data masks: mA for qA x remA ; mB for qB x remB
        m2d = np.zeros((128, 2, 2, AQ), f32)
        if half == 0:
            m2d[:, 0, :, :] = -240.0        # qA(blk0) x rA(blk1): future
            # qB(blk3) x rB(blk2): full attend -> 0
        else:
            # qA(blk1) x rA(blk0): full attend -> 0
            m2d[:, 1, :, :] = -240.0        # qB(blk2) x rB(blk3): future
        m2["masks2"] = m2d.astype(E4)
        in_maps.append(m2)
    return in_maps, sc


def kernel(**inputs):
    global LAST_RESULT, LAST_NC
    n_layers = L
    in_maps, sc = _prep_inputs(inputs, n_layers)
    key = (n_layers, tuple(sorted(sc.items())))
    if key not in _CACHE:
        _CACHE[key] = build_program(sc, n_layers)
    nc = _CACHE[key]
    LAST_NC = nc
    res = run_bass_kernel_spmd(nc, in_maps, core_ids=list(range(8)))
    LAST_RESULT = res
    out = np.empty((B, T, V), np.float32)
    for c in range(8):
        s, half = c & 3, c >> 2
        bA, bB = OWN_BLOCKS[half]
        part = np.asarray(res.results[c]["logits"]).astype(np.float32)
        out[s, bA * AQ:(bA + 1) * AQ] = part[0:AQ, :V]
        out[s, bB * AQ:(bB + 1) * AQ] = part[AQ:2 * AQ, :V]
    return out


if __name__ == "__main__":
    rng = np.random.default_rng(0)
    ins = {
        "idx": rng.integers(0, V, (B, T)).astype(np.int32),
        "wte": (rng.standard_normal((V, C)) * 0.02).astype(np.float32),
        "wpe": (rng.standard_normal((T, C)) * 0.02).astype(np.float32),
        "ln1_w": np.ones((L, C), np.float32),
        "ln1_b": np.zeros((L, C), np.float32),
        "attn_w": (rng.standard_normal((L, C, 3 * C)) * 0.02).astype(np.float32),
        "attn_b": np.zeros((L, 3 * C), np.float32),
        "proj_w": (rng.standard_normal((L, C, C)) * 0.02).astype(np.float32),
        "proj_b": np.zeros((L, C), np.float32),
        "ln2_w": np.ones((L, C), np.float32),
        "ln2_b": np.zeros((L, C), np.float32),
        "fc_w": (rng.standard_normal((L, C, 4 * C)) * 0.02).astype(np.float32),
        "fc_b": np.zeros((L, 4 * C), np.float32),
        "fc2_w": (rng.standard_normal((L, 4 * C, C)) * 0.02).astype(np.float32),
        "fc2_b": np.zeros((L, C), np.float32),
        "lnf_w": np.ones((C,), np.float32),
        "lnf_b": np.zeros((C,), np.float32),
    }
    out = kernel(**ins)
    print("out", out.shape, out.dtype, float(np.abs(out).max()))
